# revision 1
# baseline (speedup 1.0000x reference)
"""GCA model (retrieval_knn) Trainium2 kernel: 8 NeuronCores, token-sharded.

Sharding: core c -> (batch b=c//4, quarter q=c%4): 512 contiguous tokens.
KV and chunk encodings all-gathered within each batch's 4-core group.
Precision: fp32 matmuls on the top-k-selection path (layer 0 + qe paths),
float32r (full-rate) for layer-1 attention/FFN and the logits matmul.
"""
import numpy as np
from contextlib import ExitStack

import concourse.bass as bass
import concourse.tile as tile
import concourse.mybir as mybir
from concourse import bacc
from concourse.bass_utils import run_bass_kernel_spmd

dt = mybir.dt
AF = mybir.ActivationFunctionType
ALU = mybir.AluOpType

B, S, E, H, NH, L, V = 2, 2048, 1024, 1024, 8, 2, 32000
CS, K = 128, 8
HD = H // NH
SCALE = HD ** -0.5
TPC = 512            # tokens per core
NQT = TPC // 128     # 4 q-tiles per core
NC = S // CS         # 16 chunks
NKT = S // 128       # 16 key tiles
GROUPS = [[0, 1, 2, 3], [4, 5, 6, 7]]

_CACHE = {}


def _col3(wap, msl0, msl1):
    """DRAM [K, M] -> [128, K//128, msl1-msl0] lhsT-tile view."""
    return wap.rearrange("(kt kp) n -> kp kt n", kp=128)[:, :, msl0:msl1]


def _emit_ln(nc, pool, h_ap, out_ap):
    """LayerNorm of [128, H] h_ap -> out_ap (gamma=1, beta=0 fast path)."""
    sq = pool.tile([128, H], dt.float32, name="ln_sq", tag="ln_sq")
    ss = pool.tile([128, 1], dt.float32, name="ln_ss", tag="ln_ss")
    nc.scalar.activation(sq[:], h_ap, AF.Square, accum_out=ss[:])
    s = pool.tile([128, 1], dt.float32, name="ln_s", tag="ln_s")
    nc.vector.reduce_sum(s[:], h_ap, axis=mybir.AxisListType.X)
    mean = pool.tile([128, 1], dt.float32, name="ln_m", tag="ln_m")
    nc.vector.tensor_scalar(mean[:], s[:], 1.0 / H, None, ALU.mult)
    msq = pool.tile([128, 1], dt.float32, name="ln_msq", tag="ln_msq")
    nc.vector.tensor_tensor(msq[:], mean[:], mean[:], ALU.mult)
    var = pool.tile([128, 1], dt.float32, name="ln_v", tag="ln_v")
    nc.vector.tensor_scalar(var[:], ss[:], 1.0 / H, 1e-5, ALU.mult, ALU.add)
    nc.vector.tensor_sub(var[:], var[:], msq[:])
    sd = pool.tile([128, 1], dt.float32, name="ln_sd", tag="ln_sd")
    nc.scalar.activation(sd[:], var[:], AF.Sqrt)
    r = pool.tile([128, 1], dt.float32, name="ln_r", tag="ln_r")
    nc.vector.reciprocal(r[:], sd[:])
    # one Newton step: r = r*(1.5 - 0.5*var*r*r)
    r2 = pool.tile([128, 1], dt.float32, name="ln_r2", tag="ln_r2")
    nc.vector.tensor_tensor(r2[:], r[:], r[:], ALU.mult)
    nc.vector.tensor_tensor(r2[:], r2[:], var[:], ALU.mult)
    nc.vector.tensor_scalar(r2[:], r2[:], -0.5, 1.5, ALU.mult, ALU.add)
    nc.vector.tensor_tensor(r[:], r[:], r2[:], ALU.mult)
    nc.vector.tensor_scalar(out_ap, h_ap, mean[:], r[:], ALU.subtract, ALU.mult)


def _build():
    nc = bacc.Bacc("TRN2", target_bir_lowering=False, debug=False, num_devices=8)

    def din(name, shape, dtype=dt.float32):
        return nc.dram_tensor(name, shape, dtype, kind="ExternalInput").ap()

    ids_d = din("ids_col", [128, NQT], dt.int32)
    pos_d = din("pos", [TPC, E])
    temb_d = din("tok_emb", [V, E])
    inw_d = din("in_w", [E, H])
    qew1_d = din("qe_w1", [H, H // 2])
    qew2_d = din("qe_w2", [H // 2, H])
    chw1_d = din("ch_w1", [H, H // 2])
    chw2_d = din("ch_w2", [H // 2, H])
    qw_d = [din(f"l{i}_q_w", [H, H]) for i in range(L)]
    kw_d = [din(f"l{i}_k_w", [H, H]) for i in range(L)]
    vw_d = [din(f"l{i}_v_w", [H, H]) for i in range(L)]
    ow_d = [din(f"l{i}_o_w", [H, H]) for i in range(L)]
    fw1_d = [din(f"l{i}_f_w1", [H, 4 * H]) for i in range(L)]
    fw2_d = [din(f"l{i}_f_w2", [4 * H, H]) for i in range(L)]
    outw_d = din("out_w", [H, V])
    idn_d = din("idn", [128, 128])
    cmean_d = din("c_mean", [128, 1])
    rkinit_d = din("rank_init", [128, NC])

    logits_d = nc.dram_tensor("logits", [TPC, V], dt.float32, kind="ExternalOutput").ap()

    with ExitStack() as ctx:
        tc = ctx.enter_context(tile.TileContext(nc))
        P = ctx.enter_context(tc.tile_pool(name="persist", bufs=1))
        dramp = ctx.enter_context(tc.tile_pool(name="dramp", bufs=1, space="DRAM"))

        idn_t = P.tile([128, 128], dt.float32, name="idn_t")
        nc.sync.dma_start(idn_t[:], idn_d)
        idn_r = P.tile([128, 128], dt.float32r, name="idn_r")
        nc.vector.tensor_copy(idn_r[:], idn_t[:])
        cmean_t = P.tile([128, 1], dt.float32, name="cmean_t")
        nc.sync.dma_start(cmean_t[:], cmean_d)
        rkinit_t = P.tile([128, NC], dt.float32, name="rkinit_t")
        nc.sync.dma_start(rkinit_t[:], rkinit_d)

        h_t = P.tile([128, NQT, H], dt.float32, name="h_t")          # residual [tok, H]
        ceT_t = P.tile([128, 8, NC], dt.float32, name="ceT_t")       # [hp, htile, chunk]
        maskb_t = P.tile([128, NQT, NC], dt.float32, name="maskb_t")

        # ---------------- embeddings + in_w ----------------
        with tc.tile_pool(name="emb", bufs=1) as embp, \
             tc.tile_pool(name="embps", bufs=1, space="PSUM") as embps:
            ids_t = embp.tile([128, NQT], dt.int32, name="ids_t")
            nc.sync.dma_start(ids_t[:], ids_d)
            emb_t = embp.tile([128, NQT, E], dt.float32, name="emb_t")
            for j in range(NQT):
                nc.gpsimd.indirect_dma_start(
                    out=emb_t[:, j, :], out_offset=None, in_=temb_d,
                    in_offset=bass.IndirectOffsetOnAxis(ap=ids_t[:, j:j + 1], axis=0))
                pos_t = embp.tile([128, E], dt.float32, name="pos_t", tag="pos", bufs=2)
                nc.sync.dma_start(pos_t[:], pos_d[j * 128:(j + 1) * 128, :])
                nc.vector.tensor_add(emb_t[:, j, :], emb_t[:, j, :], pos_t[:])
            embT_t = embp.tile([128, 8, TPC], dt.float32, name="embT_t")
            for kt in range(8):
                for j in range(NQT):
                    tp = embps.tile([128, 128], dt.float32, name="tp_e", tag="tp", bufs=3)
                    nc.tensor.transpose(tp[:], emb_t[:, j, kt * 128:(kt + 1) * 128], idn_t[:])
                    nc.scalar.copy(embT_t[:, kt, j * 128:(j + 1) * 128], tp[:])
            inw_sb = embp.tile([128, 8, H], dt.float32, name="inw_sb")
            nc.sync.dma_start(inw_sb[:], inw_d.rearrange("(kt kp) n -> kp kt n", kp=128))
            for j in range(NQT):
                for nh in range(2):
                    ps = embps.tile([128, 512], dt.float32, name="ps_h0", tag="ps", bufs=4)
                    for kt in range(8):
                        nc.tensor.matmul(ps[:], embT_t[:, kt, j * 128:(j + 1) * 128],
                                         inw_sb[:, kt, nh * 512:(nh + 1) * 512],
                                         start=(kt == 0), stop=(kt == 7))
                    nc.scalar.copy(h_t[:, j, nh * 512:(nh + 1) * 512], ps[:])

        # ---------------- chunk encodings (fp32) ----------------
        with tc.tile_pool(name="ch", bufs=1) as chp, \
             tc.tile_pool(name="chps", bufs=2, space="PSUM") as chps:
            avg_dram = dramp.tile([NQT, H], dt.float32, name="avg_dram")
            for j in range(NQT):
                for nh in range(2):
                    ps = chps.tile([1, 512], dt.float32, name="ps_av", tag="psa")
                    nc.tensor.matmul(ps[:], cmean_t[:], h_t[:, j, nh * 512:(nh + 1) * 512],
                                     start=True, stop=True)
                    av1 = chp.tile([1, 512], dt.float32, name="av1", tag="av1", bufs=2)
                    nc.vector.tensor_copy(av1[:], ps[:])
                    nc.sync.dma_start(avg_dram[j:j + 1, nh * 512:(nh + 1) * 512], av1[:])
            avg_t = chp.tile([NQT, H], dt.float32, name="avg_t")
            nc.sync.dma_start(avg_t[:], avg_dram[:])
            avgT_t = chp.tile([128, 8, NQT], dt.float32, name="avgT_t")
            for kt in range(8):
                tp = chps.tile([128, NQT], dt.float32, name="tp_a", tag="tpa")
                nc.tensor.transpose(tp[:, :], avg_t[:, kt * 128:(kt + 1) * 128], idn_t[:NQT, :NQT])
                nc.vector.tensor_copy(avgT_t[:, kt, :], tp[:, :])
            hid_t = chp.tile([128, 4, NQT], dt.float32, name="hid_t")
            w1 = chp.tile([128, 8, 512], dt.float32, name="chw1_t")
            nc.sync.dma_start(w1[:], chw1_d.rearrange("(kt kp) n -> kp kt n", kp=128))
            for m in range(4):
                ps = chps.tile([128, NQT], dt.float32, name="ps_c1", tag="psc")
                for kt in range(8):
                    nc.tensor.matmul(ps[:], w1[:, kt, m * 128:(m + 1) * 128],
                                     avgT_t[:, kt, :], start=(kt == 0), stop=(kt == 7))
                nc.scalar.activation(hid_t[:, m, :], ps[:], AF.Relu)
            w2 = chp.tile([128, 4, 1024], dt.float32, name="chw2_t")
            nc.sync.dma_start(w2[:], chw2_d.rearrange("(kt kp) n -> kp kt n", kp=128))
            ce_loc = chp.tile([128, 8, NQT], dt.float32, name="ce_loc")
            for m in range(8):
                ps = chps.tile([128, NQT], dt.float32, name="ps_c2", tag="psc")
                for kt in range(4):
                    nc.tensor.matmul(ps[:], w2[:, kt, m * 128:(m + 1) * 128],
                                     hid_t[:, kt, :], start=(kt == 0), stop=(kt == 3))
                nc.vector.tensor_copy(ce_loc[:, m, :], ps[:])
            ce_in = dramp.tile([128, 8 * NQT], dt.float32, name="ce_in")
            ce_out = dramp.tile([4, 128, 8 * NQT], dt.float32, name="ce_out")
            nc.sync.dma_start(ce_in[:], ce_loc[:].rearrange("p a b -> p (a b)"))
            nc.gpsimd.collective_compute(
                "AllGather", ALU.bypass, replica_groups=GROUPS,
                ins=[ce_in[:].opt()], outs=[ce_out[:].opt()])
            for t in range(8):
                nc.sync.dma_start(
                    ceT_t[:, t, :].rearrange("p (r c) -> p r c", r=4),
                    ce_out[:, :, t * NQT:(t + 1) * NQT].rearrange("r p c -> p r c"))

        kv_dram = []
        for i in range(L):
            kt_in = dramp.tile([128, NH * TPC], dt.float32, name=f"kt_in{i}")
            kt_out = dramp.tile([4, 128, NH * TPC], dt.float32, name=f"kt_out{i}")
            v_in = dramp.tile([TPC, H], dt.float32, name=f"v_in{i}")
            v_out = dramp.tile([4, TPC, H], dt.float32, name=f"v_out{i}")
            kv_dram.append((kt_in, kt_out, v_in, v_out))

        for li in range(L):
            f32 = (li == 0)
            mdt = dt.float32 if f32 else dt.float32r

            def wload(pool, view, n, name, ktiles=8, rnd=(not f32), bufs=2):
                wt = pool.tile([128, ktiles, n], dt.float32, name=name + "_f", tag=name, bufs=bufs)
                nc.sync.dma_start(wt[:], view)
                if rnd:
                    wr = pool.tile([128, ktiles, n], dt.float32r, name=name + "_r",
                                   tag=name + "r", bufs=bufs)
                    nc.vector.tensor_copy(wr[:], wt[:])
                    return wr
                return wt

            # ---- LN1 + x1T ----
            x1T_t = P.tile([128, 8, TPC], mdt, name=f"x1T_{li}", tag="x1T", bufs=1)
            with tc.tile_pool(name=f"ln1_{li}", bufs=2) as lp, \
                 tc.tile_pool(name=f"ln1ps{li}", bufs=4, space="PSUM") as lps:
                for j in range(NQT):
                    x1 = lp.tile([128, H], dt.float32, name="x1", tag="x1")
                    _emit_ln(nc, lp, h_t[:, j, :], x1)
                    for kt in range(8):
                        tp = lps.tile([128, 128], dt.float32, name="tp_x", tag="tp")
                        nc.tensor.transpose(tp[:], x1[:, kt * 128:(kt + 1) * 128], idn_t[:])
                        nc.vector.tensor_copy(x1T_t[:, kt, j * 128:(j + 1) * 128], tp[:])

            # ---- QKV projections + KV all-gather ----
            qT_t = P.tile([128, 8, TPC], mdt, name=f"qT_{li}", tag="qT", bufs=1)
            kt_in, kt_out, v_in, v_out = kv_dram[li]
            with tc.tile_pool(name=f"qkv{li}", bufs=1) as pp, \
                 tc.tile_pool(name=f"qkvps{li}", bufs=4, space="PSUM") as pps:
                kt_in3 = kt_in[:].rearrange("p (a b) -> p a b", a=NH)
                v_in3 = v_in[:].rearrange("(a p) b -> p a b", p=128)
                for m in range(8):
                    wq = wload(pp, _col3(qw_d[li], m * 128, (m + 1) * 128), 128, "wq")
                    ps = pps.tile([128, TPC], dt.float32, name="ps_qp", tag="ps")
                    for kt in range(8):
                        nc.tensor.matmul(ps[:], wq[:, kt, :], x1T_t[:, kt, :],
                                         start=(kt == 0), stop=(kt == 7))
                    nc.vector.tensor_copy(qT_t[:, m, :], ps[:])
                    wk = wload(pp, _col3(kw_d[li], m * 128, (m + 1) * 128), 128, "wk")
                    ps2 = pps.tile([128, TPC], dt.float32, name="ps_kp", tag="ps")
                    for kt in range(8):
                        nc.tensor.matmul(ps2[:], wk[:, kt, :], x1T_t[:, kt, :],
                                         start=(kt == 0), stop=(kt == 7))
                    kslc = pp.tile([128, TPC], dt.float32, name="kslc", tag="kslc", bufs=2)
                    nc.scalar.copy(kslc[:], ps2[:])
                    nc.sync.dma_start(kt_in3[:, m, :], kslc[:])
                nc.gpsimd.collective_compute("AllGather", ALU.bypass, replica_groups=GROUPS,
                                             ins=[kt_in[:].opt()], outs=[kt_out[:].opt()])
                for nh2 in range(2):
                    wv = wload(pp, _col3(vw_d[li], nh2 * 512, (nh2 + 1) * 512), 512, "wv", bufs=1)
                    for j in range(NQT):
                        ps3 = pps.tile([128, 512], dt.float32, name="ps_vp", tag="ps")
                        for kt in range(8):
                            nc.tensor.matmul(ps3[:], x1T_t[:, kt, j * 128:(j + 1) * 128],
                                             wv[:, kt, :], start=(kt == 0), stop=(kt == 7))
                        vslc = pp.tile([128, 512], dt.float32, name="vslc", tag="vslc", bufs=2)
                        nc.scalar.copy(vslc[:], ps3[:])
                        nc.sync.dma_start(v_in3[:, j, nh2 * 512:(nh2 + 1) * 512], vslc[:])
                nc.gpsimd.collective_compute("AllGather", ALU.bypass, replica_groups=GROUPS,
                                             ins=[v_in[:].opt()], outs=[v_out[:].opt()])

            # ---- hT + qe MLP + scores + top-k mask (always fp32) ----
            with tc.tile_pool(name=f"qe{li}", bufs=1) as qp, \
                 tc.tile_pool(name=f"qeps{li}", bufs=1, space="PSUM") as qps:
                hT_t = qp.tile([128, 8, TPC], dt.float32, name="hT_t")
                for kt in range(8):
                    for j in range(NQT):
                        tp = qps.tile([128, 128], dt.float32, name="tp_h", tag="tp", bufs=2)
                        nc.tensor.transpose(tp[:], h_t[:, j, kt * 128:(kt + 1) * 128], idn_t[:])
                        nc.scalar.copy(hT_t[:, kt, j * 128:(j + 1) * 128], tp[:])
                qe1_t = qp.tile([128, 4, TPC], dt.float32, name="qe1_t")
                for m in range(4):
                    w = wload(qp, _col3(qew1_d, m * 128, (m + 1) * 128), 128, "qw1", rnd=False)
                    ps = qps.tile([128, TPC], dt.float32, name="ps_q1", tag="ps", bufs=3)
                    for kt in range(8):
                        nc.tensor.matmul(ps[:], w[:, kt, :], hT_t[:, kt, :],
                                         start=(kt == 0), stop=(kt == 7))
                    nc.scalar.activation(qe1_t[:, m, :], ps[:], AF.Relu)
                qeT_t = qp.tile([128, 8, TPC], dt.float32, name="qeT_t")
                for m in range(8):
                    w = wload(qp, _col3(qew2_d, m * 128, (m + 1) * 128), 128, "qw2", ktiles=4, rnd=False)
                    ps = qps.tile([128, TPC], dt.float32, name="ps_q2", tag="ps", bufs=3)
                    for kt in range(4):
                        nc.tensor.matmul(ps[:], w[:, kt, :], qe1_t[:, kt, :],
                                         start=(kt == 0), stop=(kt == 3))
                    nc.scalar.copy(qeT_t[:, m, :], ps[:])
                for j in range(NQT):
                    ps = qps.tile([128, NC], dt.float32, name="ps_sc", tag="pssc", bufs=2)
                    for kt in range(8):
                        nc.tensor.matmul(ps[:], qeT_t[:, kt, j * 128:(j + 1) * 128],
                                         ceT_t[:, kt, :], start=(kt == 0), stop=(kt == 7))
                    sc = qp.tile([128, NC], dt.float32, name="sc", tag="sc", bufs=2)
                    nc.vector.tensor_copy(sc[:], ps[:])
                    rank = qp.tile([128, NC], dt.float32, name="rank", tag="rank", bufs=2)
                    nc.vector.tensor_copy(rank[:], rkinit_t[:])
                    for d in range(1, NC):
                        ge = qp.tile([128, NC - d], dt.float32, name="ge", tag="ge", bufs=2)
                        nc.vector.tensor_tensor(ge[:], sc[:, :NC - d], sc[:, d:], ALU.is_ge)
                        nc.vector.tensor_add(rank[:, d:], rank[:, d:], ge[:])
                        nc.vector.tensor_sub(rank[:, :NC - d], rank[:, :NC - d], ge[:])
                    m01 = qp.tile([128, NC], dt.float32, name="m01", tag="m01", bufs=2)
                    nc.vector.tensor_scalar(m01[:], rank[:], 7.5, None, ALU.is_le)
                    bias_c = 5e29 if f32 else 1e30   # tanh path folds the 0.5x
                    nc.vector.tensor_scalar(maskb_t[:, j, :], m01[:], 1.0, bias_c,
                                            ALU.subtract, ALU.mult)

            # ---- attention (straight scores, per-partition mask bias) ----
            aoT_t = P.tile([128, 8, TPC], mdt, name=f"aoT_{li}", tag="aoT", bufs=1)
            with tc.tile_pool(name=f"att{li}", bufs=1) as ap, \
                 tc.tile_pool(name=f"attw{li}", bufs=2) as awp, \
                 tc.tile_pool(name=f"attps{li}", bufs=1, space="PSUM") as aps, \
                 tc.tile_pool(name=f"attps2{li}", bufs=2, space="PSUM") as aps2, \
                 tc.tile_pool(name=f"attps3{li}", bufs=2, space="PSUM") as aps3:
                kv_bufs = 2 if f32 else 1
                for hh in range(NH):
                    kT_h = awp.tile([128, S], dt.float32, name="kT_h", tag="kT_h", bufs=kv_bufs)
                    nc.sync.dma_start(
                        kT_h[:].rearrange("p (r t) -> p r t", r=4),
                        kt_out[:, :, hh * TPC:(hh + 1) * TPC].rearrange("r p t -> p r t"))
                    v_h = awp.tile([128, NKT, HD], dt.float32, name="v_h", tag="v_h", bufs=kv_bufs)
                    nc.sync.dma_start(
                        v_h[:], v_out[:].rearrange("r (a p) b -> p (r a) b", p=128)[:, :, hh * HD:(hh + 1) * HD])
                    if not f32:
                        kT_hr = awp.tile([128, S], dt.float32r, name="kT_hr", tag="kT_hr")
                        nc.vector.tensor_copy(kT_hr[:], kT_h[:])
                        kT_h = kT_hr
                        v_hr = awp.tile([128, NKT, HD], dt.float32r, name="v_hr", tag="v_hr")
                        nc.vector.tensor_copy(v_hr[:], v_h[:])
                        v_h = v_hr
                    wT_sb = ap.tile([128, NKT, TPC], mdt, name="wT_sb", tag="wT_sb")
                    for j in range(NQT):
                        ps = aps.tile([128, S], dt.float32, name="ps_qk", tag="qk", bufs=1)
                        for n4 in range(4):
                            nc.tensor.matmul(ps[:, n4 * 512:(n4 + 1) * 512],
                                             qT_t[:, hh, j * 128:(j + 1) * 128],
                                             kT_h[:, n4 * 512:(n4 + 1) * 512],
                                             start=True, stop=True)
                        ssum = ap.tile([128, 1], dt.float32, name="ssum", tag="ssum", bufs=2)
                        if f32:
                            t_sb = ap.tile([128, S], dt.float32, name="t_sb", tag="t_sb", bufs=2)
                            for c in range(NC):
                                nc.scalar.activation(t_sb[:, c * 128:(c + 1) * 128],
                                                     ps[:, c * 128:(c + 1) * 128],
                                                     AF.Tanh, bias=maskb_t[:, j, c:c + 1],
                                                     scale=0.5 * SCALE)
                            wn = ap.tile([128, S], dt.float32, name="wn", tag="wn", bufs=1)
                            den = ap.tile([128, S], dt.float32, name="den", tag="den", bufs=1)
                            nc.vector.tensor_scalar(den[:], t_sb[:], 1.0, None, ALU.subtract)
                            nc.vector.reciprocal(den[:], den[:])
                            # t_sb <- 1 + t (in place), then wn = num*rec with rowsums
                            nc.vector.tensor_scalar(t_sb[:], t_sb[:], 1.0, None, ALU.add)
                            nc.vector.scalar_tensor_tensor(wn[:], t_sb[:], 1.0, den[:],
                                                           ALU.mult, ALU.mult, accum_out=ssum[:])
                            nc.vector.reciprocal(ssum[:], ssum[:])
                            nc.vector.tensor_scalar(wn[:], wn[:], ssum[:], None, ALU.mult)
                            wn_m = wn
                        else:
                            we = ap.tile([128, S], dt.float32, name="we", tag="we", bufs=2)
                            sparts = ap.tile([128, NC], dt.float32, name="sparts", tag="sparts", bufs=2)
                            for c in range(NC):
                                nc.scalar.activation(we[:, c * 128:(c + 1) * 128],
                                                     ps[:, c * 128:(c + 1) * 128],
                                                     AF.Exp, bias=maskb_t[:, j, c:c + 1],
                                                     scale=SCALE, accum_out=sparts[:, c:c + 1])
                            nc.vector.reduce_sum(ssum[:], sparts[:], axis=mybir.AxisListType.X)
                            nc.vector.reciprocal(ssum[:], ssum[:])
                            wn_m = ap.tile([128, S], dt.float32r, name="wn_r", tag="wn_r", bufs=1)
                            nc.vector.tensor_scalar(wn_m[:], we[:], ssum[:], None, ALU.mult)
                        for c in range(NKT):
                            tp = aps2.tile([128, 128], mdt, name="tp_w", tag="tp")
                            nc.tensor.transpose(tp[:], wn_m[:, c * 128:(c + 1) * 128],
                                                idn_t[:] if f32 else idn_r[:])
                            nc.scalar.copy(wT_sb[:, c, j * 128:(j + 1) * 128], tp[:])
                    pao = aps3.tile([128, TPC], dt.float32, name="ps_ao", tag="ao")
                    for c in range(NKT):
                        nc.tensor.matmul(pao[:], v_h[:, c, :], wT_sb[:, c, :],
                                         start=(c == 0), stop=(c == NKT - 1))
                    nc.vector.tensor_copy(aoT_t[:, hh, :], pao[:])

            # ---- o-projection + residual add ----
            with tc.tile_pool(name=f"opj{li}", bufs=2) as op, \
                 tc.tile_pool(name=f"opjps{li}", bufs=1, space="PSUM") as ops:
                for m in range(8):
                    w = wload(op, _col3(ow_d[li], m * 128, (m + 1) * 128), 128, "wo")
                    ps = ops.tile([128, TPC], dt.float32, name="ps_o", tag="ps", bufs=3)
                    for kt in range(8):
                        nc.tensor.matmul(ps[:], w[:, kt, :], aoT_t[:, kt, :],
                                         start=(kt == 0), stop=(kt == 7))
                    hdT = op.tile([128, TPC], dt.float32, name="hdT", tag="hdT")
                    nc.scalar.copy(hdT[:], ps[:])
                    for j in range(NQT):
                        tp = ops.tile([128, 128], dt.float32, name="tp_o", tag="tp", bufs=3)
                        nc.tensor.transpose(tp[:], hdT[:, j * 128:(j + 1) * 128], idn_t[:])
                        nc.vector.tensor_add(h_t[:, j, m * 128:(m + 1) * 128],
                                             h_t[:, j, m * 128:(m + 1) * 128], tp[:])

            # ---- LN2 + x2T ----
            x2T_t = P.tile([128, 8, TPC], mdt, name=f"x2T_{li}", tag="x2T", bufs=1)
            with tc.tile_pool(name=f"ln2_{li}", bufs=2) as lp2, \
                 tc.tile_pool(name=f"ln2ps{li}", bufs=4, space="PSUM") as lps2:
                for j in range(NQT):
                    x2 = lp2.tile([128, H], dt.float32, name="x2", tag="x2")
                    _emit_ln(nc, lp2, h_t[:, j, :], x2)
                    for kt in range(8):
                        tp = lps2.tile([128, 128], dt.float32, name="tp_x2", tag="tp")
                        nc.tensor.transpose(tp[:], x2[:, kt * 128:(kt + 1) * 128], idn_t[:])
                        nc.vector.tensor_copy(x2T_t[:, kt, j * 128:(j + 1) * 128], tp[:])

            # ---- FFN ----
            with tc.tile_pool(name=f"ffn{li}", bufs=1) as fp, \
                 tc.tile_pool(name=f"ffnw{li}", bufs=2) as fwp, \
                 tc.tile_pool(name=f"ffnps{li}", bufs=1, space="PSUM") as fps:
                gl_sb = fp.tile([128, 32, TPC], mdt, name="gl_sb")
                for ms in range(32):
                    w1 = wload(fwp, _col3(fw1_d[li], ms * 128, (ms + 1) * 128), 128, "w1")
                    psg = fps.tile([128, TPC], dt.float32, name="ps_g", tag="psg", bufs=3)
                    for kt in range(8):
                        nc.tensor.matmul(psg[:], w1[:, kt, :], x2T_t[:, kt, :],
                                         start=(kt == 0), stop=(kt == 7))
                    nc.scalar.activation(gl_sb[:, ms, :], psg[:], AF.Gelu)
                for m in range(8):
                    acc = fps.tile([128, TPC], dt.float32, name="acc", tag="acc", bufs=2)
                    for half in range(2):
                        w2 = wload(fwp, _col3(fw2_d[li], m * 128, (m + 1) * 128)[:, half * 16:(half + 1) * 16, :],
                                   128, "w2", ktiles=16, bufs=1)
                        for kt in range(16):
                            g = half * 16 + kt
                            nc.tensor.matmul(acc[:], w2[:, kt, :], gl_sb[:, g, :],
                                             start=(g == 0), stop=(g == 31))
                    hdT = fp.tile([128, TPC], dt.float32, name="fhdT", tag="fhdT", bufs=2)
                    nc.scalar.copy(hdT[:], acc[:])
                    for j in range(NQT):
                        tp = fps.tile([128, 128], dt.float32, name="tp_f2", tag="tp", bufs=2)
                        nc.tensor.transpose(tp[:], hdT[:, j * 128:(j + 1) * 128], idn_t[:])
                        nc.vector.tensor_add(h_t[:, j, m * 128:(m + 1) * 128],
                                             h_t[:, j, m * 128:(m + 1) * 128], tp[:])

        # ---------------- logits ----------------
        with tc.tile_pool(name="lg", bufs=1) as gp, \
             tc.tile_pool(name="lgw", bufs=2) as gwp, \
             tc.tile_pool(name="lgps", bufs=1, space="PSUM") as gps:
            hTf = gp.tile([128, 8, TPC], dt.float32r, name="hTf")
            for kt in range(8):
                for j in range(NQT):
                    tp = gps.tile([128, 128], dt.float32, name="tp_hf", tag="tp", bufs=2)
                    nc.tensor.transpose(tp[:], h_t[:, j, kt * 128:(kt + 1) * 128], idn_t[:])
                    nc.vector.tensor_copy(hTf[:, kt, j * 128:(j + 1) * 128], tp[:])
            ntiles = [(n * 512, 512) for n in range(V // 512)]
            if V % 512:
                ntiles.append((V - V % 512, V % 512))
            for (noff, nsz) in ntiles:
                wf = gwp.tile([128, 8, 512], dt.float32, name="ow_f", tag="ow", bufs=2)
                nc.sync.dma_start(wf[:, :, :nsz],
                                  outw_d.rearrange("(kt kp) n -> kp kt n", kp=128)[:, :, noff:noff + nsz])
                wr = gwp.tile([128, 8, 512], dt.float32r, name="ow_r", tag="owr", bufs=2)
                nc.vector.tensor_copy(wr[:, :, :nsz], wf[:, :, :nsz])
                for j in range(NQT):
                    ps = gps.tile([128, 512], dt.float32, name="ps_lg", tag="ps", bufs=4)
                    for kt in range(8):
                        nc.tensor.matmul(ps[:, :nsz], hTf[:, kt, j * 128:(j + 1) * 128],
                                         wr[:, kt, :nsz], start=(kt == 0), stop=(kt == 7))
                    ot = gp.tile([128, 512], dt.float32, name="ot", tag="ot", bufs=4)
                    nc.vector.tensor_copy(ot[:, :nsz], ps[:, :nsz])
                    nc.sync.dma_start(logits_d[j * 128:(j + 1) * 128, noff:noff + nsz],
                                      ot[:, :nsz])

    nc.compile()
    return nc


def _prep_inputs(inputs):
    f32 = lambda x: np.ascontiguousarray(np.asarray(x, dtype=np.float32))
    ids = np.asarray(inputs["input_ids"]).astype(np.int32)
    common = {
        "tok_emb": f32(inputs["tok_emb"]), "in_w": f32(inputs["in_w"]),
        "qe_w1": f32(inputs["qe_w1"]), "qe_w2": f32(inputs["qe_w2"]),
        "ch_w1": f32(inputs["ch_w1"]), "ch_w2": f32(inputs["ch_w2"]),
        "out_w": f32(inputs["out_w"]),
        "idn": np.eye(128, dtype=np.float32),
        "c_mean": np.full((128, 1), 1.0 / CS, dtype=np.float32),
        "rank_init": np.ascontiguousarray(
            np.broadcast_to(NC - 1 - np.arange(NC, dtype=np.float32), (128, NC))),
    }
    for i in range(L):
        for nm in ["q_w", "k_w", "v_w", "o_w", "f_w1", "f_w2"]:
            common[f"l{i}_{nm}"] = f32(np.asarray(inputs[nm])[i])
    pos = f32(inputs["pos_emb"])
    in_maps = []
    for c in range(8):
        b, q = c // 4, c % 4
        off = q * TPC
        m = dict(common)
        m["ids_col"] = np.ascontiguousarray(ids[b, off:off + TPC].reshape(NQT, 128).T)
        m["pos"] = np.ascontiguousarray(pos[off:off + TPC])
        in_maps.append(m)
    return in_maps


def kernel(**inputs) -> np.ndarray:
    # biases / LN affine params are zero / one for this model; the kernel
    # implements that fast path (verified here).
    for k in ["in_b", "ch_b1", "ch_b2", "qe_b1", "qe_b2", "q_b", "k_b", "v_b",
              "o_b", "f_b1", "f_b2", "ln1_b", "ln2_b", "out_b"]:
        assert not np.any(np.asarray(inputs[k])), f"nonzero bias {k} unsupported"
    for k in ["ln1_g", "ln2_g"]:
        assert np.all(np.asarray(inputs[k]) == 1.0), f"non-unit {k} unsupported"

    if "nc" not in _CACHE:
        _CACHE["nc"] = _build()
    nc = _CACHE["nc"]
    in_maps = _prep_inputs(inputs)
    res = run_bass_kernel_spmd(nc, in_maps, list(range(8)))
    out = np.empty((B, S, V), dtype=np.float32)
    for c in range(8):
        b, q = c // 4, c % 4
        out[b, q * TPC:(q + 1) * TPC] = res.results[c]["logits"]
    return out



# revision 10
# speedup vs baseline: 1.5697x; 1.5697x over previous
"""GCA model (retrieval_knn) Trainium2 kernel: 8 NeuronCores, token-sharded.

Sharding: core c -> (batch b=c//4, quarter q=c%4): 512 contiguous tokens.
KV and chunk encodings all-gathered within each batch's 4-core group.
Logits are vocab-sharded: final h is all-gathered in the group and each
core computes its batch's full 2048 tokens x an 8000-column vocab slice.

Precision: fp32 matmuls on the top-k-selection path (embeddings/in_w,
chunk MLP, qe MLPs, scores); float32r (full-rate) everywhere else.
Attention computes scores^T [keys, queries] directly (no weight
transposes); the chunk mask is applied as a rank-1 matmul accumulate and
softmax normalization is folded into the PSUM->SBUF copy.
"""
import numpy as np
from contextlib import ExitStack

import concourse.bass as bass
import concourse.tile as tile
import concourse.mybir as mybir
from concourse import bacc
from concourse.bass_utils import run_bass_kernel_spmd

dt = mybir.dt
AF = mybir.ActivationFunctionType
ALU = mybir.AluOpType

B, S, E, H, NH, L, V = 2, 2048, 1024, 1024, 8, 2, 32000
CS, K = 128, 8
HD = H // NH
SCALE = HD ** -0.5
TPC = 512            # tokens per core
NQT = TPC // 128     # 4 q-tiles per core
NC = S // CS         # 16 chunks
NKT = S // 128       # 16 key tiles
VS = V // 4          # vocab slice per core
GROUPS = [[0, 1, 2, 3], [4, 5, 6, 7]]

_CACHE = {}


def _col3(wap, msl0, msl1):
    """DRAM [K, M] -> [128, K//128, msl1-msl0] lhsT-tile view."""
    return wap.rearrange("(kt kp) n -> kp kt n", kp=128)[:, :, msl0:msl1]


def _emit_ln(nc, pool, h_ap, out_ap):
    """LayerNorm of [128, H] h_ap -> out_ap (gamma=1, beta=0 fast path)."""
    sq = pool.tile([128, H], dt.float32, name="ln_sq", tag="ln_sq")
    ss = pool.tile([128, 1], dt.float32, name="ln_ss", tag="ln_ss")
    nc.scalar.activation(sq[:], h_ap, AF.Square, accum_out=ss[:])
    s = pool.tile([128, 1], dt.float32, name="ln_s", tag="ln_s")
    nc.vector.reduce_sum(s[:], h_ap, axis=mybir.AxisListType.X)
    mean = pool.tile([128, 1], dt.float32, name="ln_m", tag="ln_m")
    nc.vector.tensor_scalar(mean[:], s[:], 1.0 / H, None, ALU.mult)
    msq = pool.tile([128, 1], dt.float32, name="ln_msq", tag="ln_msq")
    nc.vector.tensor_tensor(msq[:], mean[:], mean[:], ALU.mult)
    var = pool.tile([128, 1], dt.float32, name="ln_v", tag="ln_v")
    nc.vector.tensor_scalar(var[:], ss[:], 1.0 / H, 1e-5, ALU.mult, ALU.add)
    nc.vector.tensor_sub(var[:], var[:], msq[:])
    sd = pool.tile([128, 1], dt.float32, name="ln_sd", tag="ln_sd")
    nc.scalar.activation(sd[:], var[:], AF.Sqrt)
    r = pool.tile([128, 1], dt.float32, name="ln_r", tag="ln_r")
    nc.vector.reciprocal(r[:], sd[:])
    # one Newton step: r = r*(1.5 - 0.5*var*r*r)
    r2 = pool.tile([128, 1], dt.float32, name="ln_r2", tag="ln_r2")
    nc.vector.tensor_tensor(r2[:], r[:], r[:], ALU.mult)
    nc.vector.tensor_tensor(r2[:], r2[:], var[:], ALU.mult)
    nc.vector.tensor_scalar(r2[:], r2[:], -0.5, 1.5, ALU.mult, ALU.add)
    nc.vector.tensor_tensor(r[:], r[:], r2[:], ALU.mult)
    nc.vector.tensor_scalar(out_ap, h_ap, mean[:], r[:], ALU.subtract, ALU.mult)


def _build():
    nc = bacc.Bacc("TRN2", target_bir_lowering=False, debug=False, num_devices=8)

    def din(name, shape, dtype=dt.float32):
        return nc.dram_tensor(name, shape, dtype, kind="ExternalInput").ap()

    ids_d = din("ids_col", [128, NQT], dt.int32)
    pos_d = din("pos", [TPC, E])
    temb_d = din("tok_emb", [V, E])
    inw_d = din("in_w", [E, H])
    qew1_d = din("qe_w1", [H, H // 2])
    qew2_d = din("qe_w2", [H // 2, H])
    chw1_d = din("ch_w1", [H, H // 2])
    chw2_d = din("ch_w2", [H // 2, H])
    qw_d = [din(f"l{i}_q_w", [H, H], dt.float32r) for i in range(L)]
    kw_d = [din(f"l{i}_k_w", [H, H], dt.float32r) for i in range(L)]
    vw_d = [din(f"l{i}_v_w", [H, H], dt.float32r) for i in range(L)]
    ow_d = [din(f"l{i}_o_w", [H, H], dt.float32r) for i in range(L)]
    fw1_d = [din(f"l{i}_f_w1", [H, 4 * H], dt.float32r) for i in range(L)]
    fw2_d = [din(f"l{i}_f_w2", [4 * H, H], dt.float32r) for i in range(L)]
    outw_d = din("out_w_sl", [H, VS], dt.float32r)
    idn_d = din("idn", [128, 128])
    ones_d = din("ones", [128, 128])
    cmean_d = din("c_mean", [128, 1])
    rkinit_d = din("rank_init", [128, NC])

    logits_d = nc.dram_tensor("logits", [S, VS], dt.float32, kind="ExternalOutput").ap()

    with ExitStack() as ctx:
        tc = ctx.enter_context(tile.TileContext(nc))
        P = ctx.enter_context(tc.tile_pool(name="persist", bufs=1))
        dramp = ctx.enter_context(tc.tile_pool(name="dramp", bufs=1, space="DRAM"))

        idn_t = P.tile([128, 128], dt.float32, name="idn_t")
        nc.sync.dma_start(idn_t[:], idn_d)
        ones_t = P.tile([128, 128], dt.float32, name="ones_t")
        nc.sync.dma_start(ones_t[:], ones_d)
        ones_r = P.tile([128, 128], dt.float32r, name="ones_r")
        nc.vector.tensor_copy(ones_r[:], ones_t[:])
        cmean_t = P.tile([128, 1], dt.float32, name="cmean_t")
        nc.sync.dma_start(cmean_t[:], cmean_d)
        rkinit_t = P.tile([128, NC], dt.float32, name="rkinit_t")
        nc.sync.dma_start(rkinit_t[:], rkinit_d)

        h_t = P.tile([128, NQT, H], dt.float32, name="h_t")          # residual [tok, H]
        ceT_t = P.tile([128, 8, NC], dt.float32, name="ceT_t")       # [hp, htile, chunk]
        maskb_t = P.tile([128, NQT, NC], dt.float32, name="maskb_t")

        # ---------------- embeddings + in_w ----------------
        with tc.tile_pool(name="emb", bufs=1) as embp, \
             tc.tile_pool(name="embps", bufs=1, space="PSUM") as embps:
            ids_t = embp.tile([128, NQT], dt.int32, name="ids_t")
            nc.sync.dma_start(ids_t[:], ids_d)
            emb_t = embp.tile([128, NQT, E], dt.float32, name="emb_t")
            for j in range(NQT):
                nc.gpsimd.indirect_dma_start(
                    out=emb_t[:, j, :], out_offset=None, in_=temb_d,
                    in_offset=bass.IndirectOffsetOnAxis(ap=ids_t[:, j:j + 1], axis=0))
                pos_t = embp.tile([128, E], dt.float32, name="pos_t", tag="pos", bufs=2)
                nc.sync.dma_start(pos_t[:], pos_d[j * 128:(j + 1) * 128, :])
                nc.vector.tensor_add(emb_t[:, j, :], emb_t[:, j, :], pos_t[:])
            embT_t = embp.tile([128, 8, TPC], dt.float32, name="embT_t")
            for kt in range(8):
                for j in range(NQT):
                    tp = embps.tile([128, 128], dt.float32, name="tp_e", tag="tp", bufs=3)
                    nc.tensor.transpose(tp[:], emb_t[:, j, kt * 128:(kt + 1) * 128], idn_t[:])
                    nc.scalar.copy(embT_t[:, kt, j * 128:(j + 1) * 128], tp[:])
            inw_sb = embp.tile([128, 8, H], dt.float32, name="inw_sb")
            nc.sync.dma_start(inw_sb[:], inw_d.rearrange("(kt kp) n -> kp kt n", kp=128))
            for j in range(NQT):
                for nh in range(2):
                    ps = embps.tile([128, 512], dt.float32, name="ps_h0", tag="ps", bufs=4)
                    for kt in range(8):
                        nc.tensor.matmul(ps[:], embT_t[:, kt, j * 128:(j + 1) * 128],
                                         inw_sb[:, kt, nh * 512:(nh + 1) * 512],
                                         start=(kt == 0), stop=(kt == 7))
                    nc.scalar.copy(h_t[:, j, nh * 512:(nh + 1) * 512], ps[:])

        # ---------------- chunk encodings (fp32) ----------------
        with tc.tile_pool(name="ch", bufs=1) as chp, \
             tc.tile_pool(name="chps", bufs=2, space="PSUM") as chps:
            avg_dram = dramp.tile([NQT, H], dt.float32, name="avg_dram")
            for j in range(NQT):
                for nh in range(2):
                    ps = chps.tile([1, 512], dt.float32, name="ps_av", tag="psa")
                    nc.tensor.matmul(ps[:], cmean_t[:], h_t[:, j, nh * 512:(nh + 1) * 512],
                                     start=True, stop=True)
                    av1 = chp.tile([1, 512], dt.float32, name="av1", tag="av1", bufs=2)
                    nc.vector.tensor_copy(av1[:], ps[:])
                    nc.sync.dma_start(avg_dram[j:j + 1, nh * 512:(nh + 1) * 512], av1[:])
            avg_t = chp.tile([NQT, H], dt.float32, name="avg_t")
            nc.sync.dma_start(avg_t[:], avg_dram[:])
            avgT_t = chp.tile([128, 8, NQT], dt.float32, name="avgT_t")
            for kt in range(8):
                tp = chps.tile([128, NQT], dt.float32, name="tp_a", tag="tpa")
                nc.tensor.transpose(tp[:, :], avg_t[:, kt * 128:(kt + 1) * 128], idn_t[:NQT, :NQT])
                nc.vector.tensor_copy(avgT_t[:, kt, :], tp[:, :])
            hid_t = chp.tile([128, 4, NQT], dt.float32, name="hid_t")
            w1 = chp.tile([128, 8, 512], dt.float32, name="chw1_t")
            nc.sync.dma_start(w1[:], chw1_d.rearrange("(kt kp) n -> kp kt n", kp=128))
            for m in range(4):
                ps = chps.tile([128, NQT], dt.float32, name="ps_c1", tag="psc")
                for kt in range(8):
                    nc.tensor.matmul(ps[:], w1[:, kt, m * 128:(m + 1) * 128],
                                     avgT_t[:, kt, :], start=(kt == 0), stop=(kt == 7))
                nc.scalar.activation(hid_t[:, m, :], ps[:], AF.Relu)
            w2 = chp.tile([128, 4, 1024], dt.float32, name="chw2_t")
            nc.sync.dma_start(w2[:], chw2_d.rearrange("(kt kp) n -> kp kt n", kp=128))
            ce_loc = chp.tile([128, 8, NQT], dt.float32, name="ce_loc")
            for m in range(8):
                ps = chps.tile([128, NQT], dt.float32, name="ps_c2", tag="psc")
                for kt in range(4):
                    nc.tensor.matmul(ps[:], w2[:, kt, m * 128:(m + 1) * 128],
                                     hid_t[:, kt, :], start=(kt == 0), stop=(kt == 3))
                nc.vector.tensor_copy(ce_loc[:, m, :], ps[:])
            ce_in = dramp.tile([128, 8 * NQT], dt.float32, name="ce_in")
            ce_out = dramp.tile([4, 128, 8 * NQT], dt.float32, name="ce_out")
            nc.sync.dma_start(ce_in[:], ce_loc[:].rearrange("p a b -> p (a b)"))
            nc.gpsimd.collective_compute(
                "AllGather", ALU.bypass, replica_groups=GROUPS,
                ins=[ce_in[:].opt()], outs=[ce_out[:].opt()])
            for t in range(8):
                nc.sync.dma_start(
                    ceT_t[:, t, :].rearrange("p (r c) -> p r c", r=4),
                    ce_out[:, :, t * NQT:(t + 1) * NQT].rearrange("r p c -> p r c"))

        kv_dram = []
        for i in range(L):
            kt_in = dramp.tile([128, NH * TPC], dt.float32r, name=f"kt_in{i}")
            kt_out = dramp.tile([4, 128, NH * TPC], dt.float32r, name=f"kt_out{i}")
            v_in = dramp.tile([TPC, H], dt.float32r, name=f"v_in{i}")
            v_out = dramp.tile([4, TPC, H], dt.float32r, name=f"v_out{i}")
            kv_dram.append((kt_in, kt_out, v_in, v_out))
        hag_in = dramp.tile([128, 8 * TPC], dt.float32r, name="hag_in")
        hag_out = dramp.tile([4, 128, 8 * TPC], dt.float32r, name="hag_out")

        for li in range(L):
            with tc.tile_pool(name=f"layer{li}", bufs=1) as LP, \
                 tc.tile_pool(name=f"qkao{li}", bufs=1) as QP:
                x1T_t = LP.tile([128, 8, TPC], dt.float32r, name="x1T", tag="xT")
                qT_t = QP.tile([128, 8, TPC], dt.float32r, name="qT")
                aoT_t = QP.tile([128, 8, TPC], dt.float32r, name="aoT")
                mbT_t = QP.tile([16, NQT, 128], dt.float32r, name="mbT")
                mb1_t = QP.tile([1, NC, NQT * 128], dt.float32r, name="mb1")

                # ---- LN1 + x1T ----
                with tc.tile_pool(name=f"ln1_{li}", bufs=2) as lp, \
                     tc.tile_pool(name=f"ln1ps{li}", bufs=4, space="PSUM") as lps:
                    for j in range(NQT):
                        x1 = lp.tile([128, H], dt.float32, name="x1", tag="x1")
                        _emit_ln(nc, lp, h_t[:, j, :], x1)
                        for kt in range(8):
                            tp = lps.tile([128, 128], dt.float32, name="tp_x", tag="tp")
                            nc.tensor.transpose(tp[:], x1[:, kt * 128:(kt + 1) * 128], idn_t[:])
                            nc.vector.tensor_copy(x1T_t[:, kt, j * 128:(j + 1) * 128], tp[:])

                # ---- QKV projections + KV all-gather ----
                kt_in, kt_out, v_in, v_out = kv_dram[li]
                with tc.tile_pool(name=f"qkv{li}", bufs=1) as pp, \
                     tc.tile_pool(name=f"qkvps{li}", bufs=4, space="PSUM") as pps:
                    kt_in3 = kt_in[:].rearrange("p (a b) -> p a b", a=NH)
                    v_in3 = v_in[:].rearrange("(a p) b -> p a b", p=128)
                    for m in range(8):
                        wq = pp.tile([128, 8, 128], dt.float32r, name="wq", tag="wq", bufs=2)
                        nc.sync.dma_start(wq[:], _col3(qw_d[li], m * 128, (m + 1) * 128))
                        ps = pps.tile([128, TPC], dt.float32, name="ps_qp", tag="ps")
                        for kt in range(8):
                            nc.tensor.matmul(ps[:], wq[:, kt, :], x1T_t[:, kt, :],
                                             start=(kt == 0), stop=(kt == 7))
                        nc.vector.tensor_copy(qT_t[:, m, :], ps[:])
                        wk = pp.tile([128, 8, 128], dt.float32r, name="wk", tag="wk", bufs=2)
                        nc.sync.dma_start(wk[:], _col3(kw_d[li], m * 128, (m + 1) * 128))
                        ps2 = pps.tile([128, TPC], dt.float32, name="ps_kp", tag="ps")
                        for kt in range(8):
                            nc.tensor.matmul(ps2[:], wk[:, kt, :], x1T_t[:, kt, :],
                                             start=(kt == 0), stop=(kt == 7))
                        kslc = pp.tile([128, TPC], dt.float32r, name="kslc", tag="kslc", bufs=2)
                        nc.vector.tensor_copy(kslc[:], ps2[:])
                        nc.sync.dma_start(kt_in3[:, m, :], kslc[:])
                    nc.gpsimd.collective_compute("AllGather", ALU.bypass, replica_groups=GROUPS,
                                                 ins=[kt_in[:].opt()], outs=[kt_out[:].opt()])
                    for nh2 in range(2):
                        wv = pp.tile([128, 8, 512], dt.float32r, name="wv", tag="wv", bufs=2)
                        nc.sync.dma_start(wv[:], _col3(vw_d[li], nh2 * 512, (nh2 + 1) * 512))
                        for j in range(NQT):
                            ps3 = pps.tile([128, 512], dt.float32, name="ps_vp", tag="ps")
                            for kt in range(8):
                                nc.tensor.matmul(ps3[:], x1T_t[:, kt, j * 128:(j + 1) * 128],
                                                 wv[:, kt, :], start=(kt == 0), stop=(kt == 7))
                            vslc = pp.tile([128, 512], dt.float32r, name="vslc", tag="vslc", bufs=2)
                            nc.vector.tensor_copy(vslc[:], ps3[:])
                            nc.sync.dma_start(v_in3[:, j, nh2 * 512:(nh2 + 1) * 512], vslc[:])
                    nc.gpsimd.collective_compute("AllGather", ALU.bypass, replica_groups=GROUPS,
                                                 ins=[v_in[:].opt()], outs=[v_out[:].opt()])

                # ---- hT + qe MLP + scores + top-k mask (fp32) ----
                with tc.tile_pool(name=f"qe{li}", bufs=1) as qp, \
                     tc.tile_pool(name=f"qeps{li}", bufs=1, space="PSUM") as qps:
                    hT_t = qp.tile([128, 8, TPC], dt.float32, name="hT_t")
                    for kt in range(8):
                        for j in range(NQT):
                            tp = qps.tile([128, 128], dt.float32, name="tp_h", tag="tp", bufs=2)
                            nc.tensor.transpose(tp[:], h_t[:, j, kt * 128:(kt + 1) * 128], idn_t[:])
                            nc.scalar.copy(hT_t[:, kt, j * 128:(j + 1) * 128], tp[:])
                    qe1_t = qp.tile([128, 4, TPC], dt.float32, name="qe1_t")
                    for m in range(4):
                        w = qp.tile([128, 8, 128], dt.float32, name="qw1", tag="qw1", bufs=2)
                        nc.sync.dma_start(w[:], _col3(qew1_d, m * 128, (m + 1) * 128))
                        ps = qps.tile([128, TPC], dt.float32, name="ps_q1", tag="ps", bufs=3)
                        for kt in range(8):
                            nc.tensor.matmul(ps[:], w[:, kt, :], hT_t[:, kt, :],
                                             start=(kt == 0), stop=(kt == 7))
                        nc.scalar.activation(qe1_t[:, m, :], ps[:], AF.Relu)
                    qeT_t = qp.tile([128, 8, TPC], dt.float32, name="qeT_t")
                    for m in range(8):
                        w = qp.tile([128, 4, 128], dt.float32, name="qw2", tag="qw2", bufs=2)
                        nc.sync.dma_start(w[:], _col3(qew2_d, m * 128, (m + 1) * 128))
                        ps = qps.tile([128, TPC], dt.float32, name="ps_q2", tag="ps", bufs=3)
                        for kt in range(4):
                            nc.tensor.matmul(ps[:], w[:, kt, :], qe1_t[:, kt, :],
                                             start=(kt == 0), stop=(kt == 3))
                        nc.scalar.copy(qeT_t[:, m, :], ps[:])
                    for j in range(NQT):
                        ps = qps.tile([128, NC], dt.float32, name="ps_sc", tag="pssc", bufs=2)
                        for kt in range(8):
                            nc.tensor.matmul(ps[:], qeT_t[:, kt, j * 128:(j + 1) * 128],
                                             ceT_t[:, kt, :], start=(kt == 0), stop=(kt == 7))
                        sc = qp.tile([128, NC], dt.float32, name="sc", tag="sc", bufs=2)
                        nc.vector.tensor_copy(sc[:], ps[:])
                        rank = qp.tile([128, NC], dt.float32, name="rank", tag="rank", bufs=2)
                        nc.vector.tensor_copy(rank[:], rkinit_t[:])
                        for d in range(1, NC):
                            ge = qp.tile([128, NC - d], dt.float32, name="ge", tag="ge", bufs=2)
                            nc.vector.tensor_tensor(ge[:], sc[:, :NC - d], sc[:, d:], ALU.is_ge)
                            nc.vector.tensor_add(rank[:, d:], rank[:, d:], ge[:])
                            nc.vector.tensor_sub(rank[:, :NC - d], rank[:, :NC - d], ge[:])
                        m01 = qp.tile([128, NC], dt.float32, name="m01", tag="m01", bufs=2)
                        nc.vector.tensor_scalar(m01[:], rank[:], 7.5, None, ALU.is_le)
                        nc.vector.tensor_scalar(maskb_t[:, j, :], m01[:], 1.0, 1e30,
                                                ALU.subtract, ALU.mult)
                    # mbT[c, j, ii] = maskb[token(j,ii), c] as [1,512] rank-1 rows
                    for j in range(NQT):
                        tpm = qps.tile([16, 128], dt.float32, name="tp_m", tag="tp", bufs=2)
                        nc.tensor.transpose(tpm[:], maskb_t[:, j, :], idn_t[:])
                        nc.vector.tensor_copy(mbT_t[:, j, :], tpm[:])
                    for c in range(NC):
                        nc.sync.dma_start(mb1_t[0:1, c, :],
                                          mbT_t[c:c + 1, :, :].rearrange("c j i -> c (j i)"))

                # ---- attention: scoresT = K^T-major, mask via rank-1, exp, AV ----
                with tc.tile_pool(name=f"att{li}", bufs=1) as ap, \
                     tc.tile_pool(name=f"attw{li}", bufs=2) as awp, \
                     tc.tile_pool(name=f"attqk{li}", bufs=3, space="PSUM") as aps_qk, \
                     tc.tile_pool(name=f"attrs{li}", bufs=2, space="PSUM") as aps_rs, \
                     tc.tile_pool(name=f"attao{li}", bufs=2, space="PSUM") as aps_ao, \
                     tc.tile_pool(name=f"attrb{li}", bufs=1, space="PSUM") as aps_rb:
                    for hh in range(NH):
                        kT_h = awp.tile([128, S], dt.float32r, name="kT_h", tag="kT_h", bufs=2)
                        nc.sync.dma_start(
                            kT_h[:].rearrange("p (r t) -> p r t", r=4),
                            kt_out[:, :, hh * TPC:(hh + 1) * TPC].rearrange("r p t -> p r t"))
                        v_h = awp.tile([128, NKT, HD], dt.float32r, name="v_h", tag="v_h", bufs=2)
                        nc.sync.dma_start(
                            v_h[:], v_out[:].rearrange("r (a p) b -> p (r a) b", p=128)[:, :, hh * HD:(hh + 1) * HD])
                        weT = ap.tile([128, NKT, TPC], dt.float32r, name="weT", tag="weT", bufs=2)
                        rs_ps = aps_rs.tile([1, TPC], dt.float32, name="rs_ps", tag="rs")
                        pao = aps_ao.tile([128, TPC], dt.float32, name="pao", tag="ao")
                        for c in range(NKT):
                            ps_s = aps_qk.tile([128, TPC], dt.float32, name="ps_s", tag="qk")
                            nc.tensor.matmul(ps_s[:], kT_h[:, c * 128:(c + 1) * 128],
                                             qT_t[:, hh, :], start=True, stop=False)
                            nc.tensor.matmul(ps_s[:], ones_r[0:1, :],
                                             mb1_t[0:1, c, :], start=False, stop=True)
                            nc.scalar.activation(weT[:, c, :], ps_s[:], AF.Exp, scale=SCALE)
                            nc.tensor.matmul(rs_ps[:], ones_r[:, 0:1], weT[:, c, :],
                                             start=(c == 0), stop=(c == NKT - 1))
                            nc.tensor.matmul(pao[:], v_h[:, c, :], weT[:, c, :],
                                             start=(c == 0), stop=(c == NKT - 1))
                        rinv_sb = ap.tile([1, TPC], dt.float32, name="rinv", tag="rinv", bufs=2)
                        nc.vector.reciprocal(rinv_sb[:], rs_ps[:])
                        ps_rb = aps_rb.tile([128, TPC], dt.float32, name="ps_rb", tag="rb")
                        nc.tensor.matmul(ps_rb[:], ones_t[0:1, :], rinv_sb[:],
                                         start=True, stop=True)
                        rinvb = ap.tile([128, TPC], dt.float32, name="rinvb", tag="rinvb", bufs=2)
                        nc.vector.tensor_copy(rinvb[:], ps_rb[:])
                        nc.vector.tensor_tensor(aoT_t[:, hh, :], pao[:], rinvb[:], ALU.mult)

                # ---- o-projection direct [tok, feat] + residual add ----
                with tc.tile_pool(name=f"opj{li}", bufs=1) as op, \
                     tc.tile_pool(name=f"opjps{li}", bufs=4, space="PSUM") as ops:
                    wo_sb = op.tile([128, 8, H], dt.float32r, name="wo_sb")
                    nc.sync.dma_start(wo_sb[:], _col3(ow_d[li], 0, H))
                    for j in range(NQT):
                        for mh in range(2):
                            ps_o = ops.tile([128, 512], dt.float32, name="ps_o", tag="ps")
                            for kt in range(8):
                                nc.tensor.matmul(ps_o[:], aoT_t[:, kt, j * 128:(j + 1) * 128],
                                                 wo_sb[:, kt, mh * 512:(mh + 1) * 512],
                                                 start=(kt == 0), stop=(kt == 7))
                            nc.vector.tensor_tensor(h_t[:, j, mh * 512:(mh + 1) * 512],
                                                    h_t[:, j, mh * 512:(mh + 1) * 512],
                                                    ps_o[:], ALU.add)

                # ---- LN2 + x2T ----
                x2T_t = LP.tile([128, 8, TPC], dt.float32r, name="x2T", tag="xT")
                with tc.tile_pool(name=f"ln2_{li}", bufs=2) as lp2, \
                     tc.tile_pool(name=f"ln2ps{li}", bufs=4, space="PSUM") as lps2:
                    for j in range(NQT):
                        x2 = lp2.tile([128, H], dt.float32, name="x2", tag="x2")
                        _emit_ln(nc, lp2, h_t[:, j, :], x2)
                        for kt in range(8):
                            tp = lps2.tile([128, 128], dt.float32, name="tp_x2", tag="tp")
                            nc.tensor.transpose(tp[:], x2[:, kt * 128:(kt + 1) * 128], idn_t[:])
                            nc.vector.tensor_copy(x2T_t[:, kt, j * 128:(j + 1) * 128], tp[:])

                # ---- FFN: w1 -> gelu -> w2 direct [tok, feat] ----
                with tc.tile_pool(name=f"ffn{li}", bufs=1) as fp, \
                     tc.tile_pool(name=f"ffnw{li}", bufs=2) as fwp, \
                     tc.tile_pool(name=f"ffnps{li}", bufs=3, space="PSUM") as fps, \
                     tc.tile_pool(name=f"ffnps2{li}", bufs=4, space="PSUM") as fps2:
                    gl_sb = fp.tile([128, 32, TPC], dt.float32r, name="gl_sb")
                    for ms in range(32):
                        w1s = fwp.tile([128, 8, 128], dt.float32r, name="w1s", tag="w1s")
                        nc.sync.dma_start(w1s[:], _col3(fw1_d[li], ms * 128, (ms + 1) * 128))
                        psg = fps.tile([128, TPC], dt.float32, name="ps_g", tag="psg")
                        for kt in range(8):
                            nc.tensor.matmul(psg[:], w1s[:, kt, :], x2T_t[:, kt, :],
                                             start=(kt == 0), stop=(kt == 7))
                        nc.scalar.activation(gl_sb[:, ms, :], psg[:], AF.Gelu)
                    for mq in range(4):
                        w2q = fwp.tile([128, 32, 256], dt.float32r, name="w2q", tag="w2q", bufs=1)
                        nc.sync.dma_start(w2q[:], _col3(fw2_d[li], mq * 256, (mq + 1) * 256))
                        for j in range(NQT):
                            ps_f = fps2.tile([128, 256], dt.float32, name="ps_f", tag="psf")
                            for kt in range(32):
                                nc.tensor.matmul(ps_f[:], gl_sb[:, kt, j * 128:(j + 1) * 128],
                                                 w2q[:, kt, :], start=(kt == 0), stop=(kt == 31))
                            nc.vector.tensor_tensor(h_t[:, j, mq * 256:(mq + 1) * 256],
                                                    h_t[:, j, mq * 256:(mq + 1) * 256],
                                                    ps_f[:], ALU.add)

        # ---------------- logits: h all-gather + vocab-sharded matmul ----------------
        with tc.tile_pool(name="lg", bufs=1) as gp, \
             tc.tile_pool(name="lgw", bufs=2) as gwp, \
             tc.tile_pool(name="lgps", bufs=2, space="PSUM") as gps, \
             tc.tile_pool(name="lgps2", bufs=6, space="PSUM") as gps2:
            hTf = gp.tile([128, 8, TPC], dt.float32r, name="hTf")
            for kt in range(8):
                for j in range(NQT):
                    tp = gps.tile([128, 128], dt.float32, name="tp_hf", tag="tp")
                    nc.tensor.transpose(tp[:], h_t[:, j, kt * 128:(kt + 1) * 128], idn_t[:])
                    nc.vector.tensor_copy(hTf[:, kt, j * 128:(j + 1) * 128], tp[:])
            nc.sync.dma_start(hag_in[:], hTf[:].rearrange("p a b -> p (a b)"))
            nc.gpsimd.collective_compute("AllGather", ALU.bypass, replica_groups=GROUPS,
                                         ins=[hag_in[:].opt()], outs=[hag_out[:].opt()])
            hT_full = gp.tile([128, 8, S], dt.float32r, name="hT_full")
            for r in range(4):
                nc.sync.dma_start(
                    hT_full[:, :, r * TPC:(r + 1) * TPC],
                    hag_out[r, :, :].rearrange("p (a b) -> p a b", a=8))
            ntiles = [(n * 512, 512) for n in range(VS // 512)]
            if VS % 512:
                ntiles.append((VS - VS % 512, VS % 512))
            for (noff, nsz) in ntiles:
                wf = gwp.tile([128, 8, 512], dt.float32r, name="ow_f", tag="ow", bufs=2)
                nc.sync.dma_start(wf[:, :, :nsz],
                                  outw_d.rearrange("(kt kp) n -> kp kt n", kp=128)[:, :, noff:noff + nsz])
                for tt in range(S // 128):
                    ps = gps2.tile([128, 512], dt.float32, name="ps_lg", tag="ps")
                    for kt in range(8):
                        nc.tensor.matmul(ps[:, :nsz], hT_full[:, kt, tt * 128:(tt + 1) * 128],
                                         wf[:, kt, :nsz], start=(kt == 0), stop=(kt == 7))
                    ot = gp.tile([128, 512], dt.float32, name="ot", tag="ot", bufs=4)
                    nc.vector.tensor_copy(ot[:, :nsz], ps[:, :nsz])
                    nc.sync.dma_start(logits_d[tt * 128:(tt + 1) * 128, noff:noff + nsz],
                                      ot[:, :nsz])

    nc.compile()
    return nc


def _prep_inputs(inputs):
    f32 = lambda x: np.ascontiguousarray(np.asarray(x, dtype=np.float32))
    ids = np.asarray(inputs["input_ids"]).astype(np.int32)
    common = {
        "tok_emb": f32(inputs["tok_emb"]), "in_w": f32(inputs["in_w"]),
        "qe_w1": f32(inputs["qe_w1"]), "qe_w2": f32(inputs["qe_w2"]),
        "ch_w1": f32(inputs["ch_w1"]), "ch_w2": f32(inputs["ch_w2"]),
        "idn": np.eye(128, dtype=np.float32),
        "ones": np.ones((128, 128), dtype=np.float32),
        "c_mean": np.full((128, 1), 1.0 / CS, dtype=np.float32),
        "rank_init": np.ascontiguousarray(
            np.broadcast_to(NC - 1 - np.arange(NC, dtype=np.float32), (128, NC))),
    }
    for i in range(L):
        for nm in ["q_w", "k_w", "v_w", "o_w", "f_w1", "f_w2"]:
            common[f"l{i}_{nm}"] = f32(np.asarray(inputs[nm])[i])
    pos = f32(inputs["pos_emb"])
    outw = f32(inputs["out_w"])
    in_maps = []
    for c in range(8):
        b, q = c // 4, c % 4
        off = q * TPC
        m = dict(common)
        m["ids_col"] = np.ascontiguousarray(ids[b, off:off + TPC].reshape(NQT, 128).T)
        m["pos"] = np.ascontiguousarray(pos[off:off + TPC])
        m["out_w_sl"] = np.ascontiguousarray(outw[:, q * VS:(q + 1) * VS])
        in_maps.append(m)
    return in_maps


def kernel(**inputs) -> np.ndarray:
    # biases / LN affine params are zero / one for this model; the kernel
    # implements that fast path (verified here).
    for k in ["in_b", "ch_b1", "ch_b2", "qe_b1", "qe_b2", "q_b", "k_b", "v_b",
              "o_b", "f_b1", "f_b2", "ln1_b", "ln2_b", "out_b"]:
        assert not np.any(np.asarray(inputs[k])), f"nonzero bias {k} unsupported"
    for k in ["ln1_g", "ln2_g"]:
        assert np.all(np.asarray(inputs[k]) == 1.0), f"non-unit {k} unsupported"

    if "nc" not in _CACHE:
        _CACHE["nc"] = _build()
    nc = _CACHE["nc"]
    in_maps = _prep_inputs(inputs)
    res = run_bass_kernel_spmd(nc, in_maps, list(range(8)))
    out = np.empty((B, S, V), dtype=np.float32)
    for c in range(8):
        b, q = c // 4, c % 4
        out[b, :, q * VS:(q + 1) * VS] = res.results[c]["logits"]
    return out


# revision 32
# speedup vs baseline: 1.6065x; 1.0234x over previous
"""GCA model (retrieval_knn) Trainium2 kernel: 8 NeuronCores, token-sharded.

Sharding: core c -> (batch b=c//4, quarter q=c%4): 512 contiguous tokens.
KV and chunk encodings all-gathered within each batch's 4-core group.
Logits are vocab-sharded: final h is all-gathered in the group and each
core computes its batch's full 2048 tokens x an 8000-column vocab slice.

Precision: fp32 matmuls on the top-k-selection path (embeddings/in_w,
chunk MLP, qe MLPs, scores); float32r (full-rate) everywhere else.
Attention computes scores^T [keys, queries] directly (no weight
transposes); the chunk mask is applied as a rank-1 matmul accumulate and
softmax normalization is folded into the PSUM->SBUF copy.
"""
import numpy as np
from contextlib import ExitStack

import concourse.bass as bass
import concourse.tile as tile
import concourse.mybir as mybir
from concourse import bacc
from concourse.bass_utils import run_bass_kernel_spmd

dt = mybir.dt
AF = mybir.ActivationFunctionType
ALU = mybir.AluOpType

B, S, E, H, NH, L, V = 2, 2048, 1024, 1024, 8, 2, 32000
CS, K = 128, 8
HD = H // NH
SCALE = HD ** -0.5
TPC = 512            # tokens per core
NQT = TPC // 128     # 4 q-tiles per core
NC = S // CS         # 16 chunks
NKT = S // 128       # 16 key tiles
VS = V // 4          # vocab slice per core
GROUPS = [[0, 1, 2, 3], [4, 5, 6, 7]]

_CACHE = {}


def _col3(wap, msl0, msl1):
    """DRAM [K, M] -> [128, K//128, msl1-msl0] lhsT-tile view."""
    return wap.rearrange("(kt kp) n -> kp kt n", kp=128)[:, :, msl0:msl1]


def _emit_ln(nc, pool, h_ap, out_ap):
    """LayerNorm of [128, H] h_ap -> out_ap (gamma=1, beta=0 fast path)."""
    sq = pool.tile([128, H], dt.float32, name="ln_sq", tag="ln_sq")
    ss = pool.tile([128, 1], dt.float32, name="ln_ss", tag="ln_ss")
    nc.scalar.activation(sq[:], h_ap, AF.Square, accum_out=ss[:])
    s = pool.tile([128, 1], dt.float32, name="ln_s", tag="ln_s")
    nc.vector.reduce_sum(s[:], h_ap, axis=mybir.AxisListType.X)
    mean = pool.tile([128, 1], dt.float32, name="ln_m", tag="ln_m")
    nc.vector.tensor_scalar(mean[:], s[:], 1.0 / H, None, ALU.mult)
    msq = pool.tile([128, 1], dt.float32, name="ln_msq", tag="ln_msq")
    nc.vector.tensor_tensor(msq[:], mean[:], mean[:], ALU.mult)
    var = pool.tile([128, 1], dt.float32, name="ln_v", tag="ln_v")
    nc.vector.tensor_scalar(var[:], ss[:], 1.0 / H, 1e-5, ALU.mult, ALU.add)
    nc.vector.tensor_sub(var[:], var[:], msq[:])
    sd = pool.tile([128, 1], dt.float32, name="ln_sd", tag="ln_sd")
    nc.scalar.activation(sd[:], var[:], AF.Sqrt)
    r = pool.tile([128, 1], dt.float32, name="ln_r", tag="ln_r")
    nc.vector.reciprocal(r[:], sd[:])
    # one Newton step: r = r*(1.5 - 0.5*var*r*r)
    r2 = pool.tile([128, 1], dt.float32, name="ln_r2", tag="ln_r2")
    nc.vector.tensor_tensor(r2[:], r[:], r[:], ALU.mult)
    nc.vector.tensor_tensor(r2[:], r2[:], var[:], ALU.mult)
    nc.vector.tensor_scalar(r2[:], r2[:], -0.5, 1.5, ALU.mult, ALU.add)
    nc.vector.tensor_tensor(r[:], r[:], r2[:], ALU.mult)
    nc.vector.tensor_scalar(out_ap, h_ap, mean[:], r[:], ALU.subtract, ALU.mult)


def _build():
    nc = bacc.Bacc("TRN2", target_bir_lowering=False, debug=False, num_devices=8)

    def din(name, shape, dtype=dt.float32):
        return nc.dram_tensor(name, shape, dtype, kind="ExternalInput").ap()

    ids_d = din("ids_col", [128, NQT], dt.int32)
    pos_d = din("pos", [TPC, E])
    temb_d = din("tok_emb", [V, E])
    inw_d = din("in_w", [E, H])
    qew1_d = din("qe_w1", [H, H // 2])
    qew2_d = din("qe_w2", [H // 2, H])
    qew1r_d = din("qe_w1r", [H, H // 2], dt.float32r)
    qew2r_d = din("qe_w2r", [H // 2, H], dt.float32r)
    chw1_d = din("ch_w1", [H, H // 2])
    chw2_d = din("ch_w2", [H // 2, H])
    qw_d = [din(f"l{i}_q_w", [H, H], dt.float32r) for i in range(L)]
    kw_d = [din(f"l{i}_k_w", [H, H], dt.float32r) for i in range(L)]
    vw_d = [din(f"l{i}_v_w", [H, H], dt.float32r) for i in range(L)]
    ow_d = [din(f"l{i}_o_w", [H, H], dt.float32r) for i in range(L)]
    fw1_d = [din(f"l{i}_f_w1", [H, 4 * H], dt.float32r) for i in range(L)]
    fw2_d = [din(f"l{i}_f_w2", [4 * H, H], dt.float32r) for i in range(L)]
    outw_d = din("out_w_sl", [H, VS], dt.float32r)
    idn_d = din("idn", [128, 128])
    ones_d = din("ones", [128, 128])
    cmean_d = din("c_mean", [128, 1])
    rkinit_d = din("rank_init", [128, NC])

    logits_d = nc.dram_tensor("logits", [S, VS], dt.float32, kind="ExternalOutput").ap()

    with ExitStack() as ctx:
        tc = ctx.enter_context(tile.TileContext(nc))
        P = ctx.enter_context(tc.tile_pool(name="persist", bufs=1))
        dramp = ctx.enter_context(tc.tile_pool(name="dramp", bufs=1, space="DRAM"))

        idn_t = P.tile([128, 128], dt.float32, name="idn_t")
        nc.sync.dma_start(idn_t[:], idn_d)
        ones_t = P.tile([128, 128], dt.float32, name="ones_t")
        nc.sync.dma_start(ones_t[:], ones_d)
        ones_r = P.tile([128, 128], dt.float32r, name="ones_r")
        nc.vector.tensor_copy(ones_r[:], ones_t[:])

        cmean_t = P.tile([128, 1], dt.float32, name="cmean_t")
        nc.sync.dma_start(cmean_t[:], cmean_d)
        rkinit_t = P.tile([128, NC], dt.float32, name="rkinit_t")
        nc.sync.dma_start(rkinit_t[:], rkinit_d)

        h_t = P.tile([128, NQT, H], dt.float32, name="h_t")          # residual [tok, H]
        ceT_t = P.tile([128, 8, NC], dt.float32, name="ceT_t")       # [hp, htile, chunk]
        ceT_r = P.tile([128, 8, NC], dt.float32r, name="ceT_r")
        maskb_t = P.tile([128, NQT, NC], dt.float32, name="maskb_t")

        # ---------------- embeddings + in_w ----------------
        with tc.tile_pool(name="emb", bufs=1) as embp, \
             tc.tile_pool(name="embps", bufs=1, space="PSUM") as embps:
            ids_t = embp.tile([128, NQT], dt.int32, name="ids_t")
            nc.sync.dma_start(ids_t[:], ids_d)
            emb_t = embp.tile([128, NQT, E], dt.float32, name="emb_t")
            for j in range(NQT):
                nc.gpsimd.indirect_dma_start(
                    out=emb_t[:, j, :], out_offset=None, in_=temb_d,
                    in_offset=bass.IndirectOffsetOnAxis(ap=ids_t[:, j:j + 1], axis=0))
                pos_t = embp.tile([128, E], dt.float32, name="pos_t", tag="pos", bufs=2)
                nc.sync.dma_start(pos_t[:], pos_d[j * 128:(j + 1) * 128, :])
                nc.vector.tensor_add(emb_t[:, j, :], emb_t[:, j, :], pos_t[:])
            embT_t = embp.tile([128, 8, TPC], dt.float32, name="embT_t")
            for kt in range(8):
                for j in range(NQT):
                    tp = embps.tile([128, 128], dt.float32, name="tp_e", tag="tp", bufs=3)
                    nc.tensor.transpose(tp[:], emb_t[:, j, kt * 128:(kt + 1) * 128], idn_t[:])
                    nc.vector.tensor_copy(embT_t[:, kt, j * 128:(j + 1) * 128], tp[:])
            inw_sb = embp.tile([128, 8, H], dt.float32, name="inw_sb")
            nc.sync.dma_start(inw_sb[:], inw_d.rearrange("(kt kp) n -> kp kt n", kp=128))
            for j in range(NQT):
                for nh in range(2):
                    ps = embps.tile([128, 512], dt.float32, name="ps_h0", tag="ps", bufs=4)
                    for kt in range(8):
                        nc.tensor.matmul(ps[:], embT_t[:, kt, j * 128:(j + 1) * 128],
                                         inw_sb[:, kt, nh * 512:(nh + 1) * 512],
                                         start=(kt == 0), stop=(kt == 7))
                    nc.vector.tensor_copy(h_t[:, j, nh * 512:(nh + 1) * 512], ps[:])

        # ---------------- chunk encodings (fp32) ----------------
        with tc.tile_pool(name="ch", bufs=1) as chp, \
             tc.tile_pool(name="chps", bufs=2, space="PSUM") as chps:
            avg_t = chp.tile([NQT, H], dt.float32, name="avg_t")
            for j in range(NQT):
                for nh in range(2):
                    ps = chps.tile([1, 512], dt.float32, name="ps_av", tag="psa")
                    nc.tensor.matmul(ps[:], cmean_t[:], h_t[:, j, nh * 512:(nh + 1) * 512],
                                     start=True, stop=True)
                    av1 = chp.tile([1, 512], dt.float32, name="av1", tag="av1", bufs=2)
                    nc.vector.tensor_copy(av1[:], ps[:])
                    nc.scalar.dma_start(avg_t[j:j + 1, nh * 512:(nh + 1) * 512], av1[:])
            avgT_t = chp.tile([128, 8, NQT], dt.float32, name="avgT_t")
            for kt in range(8):
                tp = chps.tile([128, NQT], dt.float32, name="tp_a", tag="tpa")
                nc.tensor.transpose(tp[:, :], avg_t[:, kt * 128:(kt + 1) * 128], idn_t[:NQT, :NQT])
                nc.vector.tensor_copy(avgT_t[:, kt, :], tp[:, :])
            hid_t = chp.tile([128, 4, NQT], dt.float32, name="hid_t")
            w1 = chp.tile([128, 8, 512], dt.float32, name="chw1_t")
            nc.sync.dma_start(w1[:], chw1_d.rearrange("(kt kp) n -> kp kt n", kp=128))
            for m in range(4):
                ps = chps.tile([128, NQT], dt.float32, name="ps_c1", tag="psc")
                for kt in range(8):
                    nc.tensor.matmul(ps[:], w1[:, kt, m * 128:(m + 1) * 128],
                                     avgT_t[:, kt, :], start=(kt == 0), stop=(kt == 7))
                nc.scalar.activation(hid_t[:, m, :], ps[:], AF.Relu)
            w2 = chp.tile([128, 4, 1024], dt.float32, name="chw2_t")
            nc.sync.dma_start(w2[:], chw2_d.rearrange("(kt kp) n -> kp kt n", kp=128))
            ce_loc = chp.tile([128, 8, NQT], dt.float32, name="ce_loc")
            for m in range(8):
                ps = chps.tile([128, NQT], dt.float32, name="ps_c2", tag="psc")
                for kt in range(4):
                    nc.tensor.matmul(ps[:], w2[:, kt, m * 128:(m + 1) * 128],
                                     hid_t[:, kt, :], start=(kt == 0), stop=(kt == 3))
                nc.vector.tensor_copy(ce_loc[:, m, :], ps[:])
            ce_in = dramp.tile([128, 8 * NQT], dt.float32, name="ce_in")
            ce_out = dramp.tile([4, 128, 8 * NQT], dt.float32, name="ce_out")
            nc.sync.dma_start(ce_in[:], ce_loc[:].rearrange("p a b -> p (a b)"))
            nc.gpsimd.collective_compute(
                "AllGather", ALU.bypass, replica_groups=GROUPS,
                ins=[ce_in[:].opt()], outs=[ce_out[:].opt()])
            for t in range(8):
                nc.sync.dma_start(
                    ceT_t[:, t, :].rearrange("p (r c) -> p r c", r=4),
                    ce_out[:, :, t * NQT:(t + 1) * NQT].rearrange("r p c -> p r c"))
            nc.vector.tensor_copy(ceT_r[:], ceT_t[:])

        kv_dram = []
        for i in range(L):
            kt_in = dramp.tile([128, NH * TPC], dt.float32r, name=f"kt_in{i}")
            kt_out = dramp.tile([4, 128, NH * TPC], dt.float32r, name=f"kt_out{i}")
            v_in = dramp.tile([TPC, H], dt.float32r, name=f"v_in{i}")
            v_out = dramp.tile([4, TPC, H], dt.float32r, name=f"v_out{i}")
            kv_dram.append((kt_in, kt_out, v_in, v_out))
        hag_in = dramp.tile([128, 8 * TPC], dt.float32r, name="hag_in")
        hag_out = dramp.tile([4, 128, 8 * TPC], dt.float32r, name="hag_out")

        for li in range(L):
            with tc.tile_pool(name=f"layer{li}", bufs=1) as LP:
                qp_cm = tc.tile_pool(name=f"qkao{li}", bufs=1)
                QP = qp_cm.__enter__()
                x1T_t = LP.tile([128, 8, TPC], dt.float32r, name="x1T", tag="xT")
                qT_t = QP.tile([128, 8, TPC], dt.float32r, name="qT")
                aoT_t = QP.tile([128, 8, TPC], dt.float32r, name="aoT")
                mbT_t = QP.tile([16, NQT, 128], dt.float32r, name="mbT")
                mb1_t = QP.tile([1, NC, NQT * 128], dt.float32r, name="mb1")

                # ---- LN1 + x1T ----
                with tc.tile_pool(name=f"ln1_{li}", bufs=2) as lp, \
                     tc.tile_pool(name=f"ln1ps{li}", bufs=4, space="PSUM") as lps:
                    for j in range(NQT):
                        x1 = lp.tile([128, H], dt.float32, name="x1", tag="x1")
                        _emit_ln(nc, lp, h_t[:, j, :], x1)
                        for kt in range(8):
                            tp = lps.tile([128, 128], dt.float32, name="tp_x", tag="tp")
                            nc.tensor.transpose(tp[:], x1[:, kt * 128:(kt + 1) * 128], idn_t[:])
                            nc.vector.tensor_copy(x1T_t[:, kt, j * 128:(j + 1) * 128], tp[:])

                # ---- QKV projections + KV all-gather ----
                kt_in, kt_out, v_in, v_out = kv_dram[li]
                with tc.tile_pool(name=f"qkv{li}", bufs=1) as pp, \
                     tc.tile_pool(name=f"qkvps{li}", bufs=4, space="PSUM") as pps:
                    kt_in3 = kt_in[:].rearrange("p (a b) -> p a b", a=NH)
                    v_in3 = v_in[:].rearrange("(a p) b -> p a b", p=128)
                    for m in range(8):
                        wq = pp.tile([128, 8, 128], dt.float32r, name="wq", tag="wq", bufs=2)
                        nc.sync.dma_start(wq[:], _col3(qw_d[li], m * 128, (m + 1) * 128))
                        ps = pps.tile([128, TPC], dt.float32, name="ps_qp", tag="ps")
                        for kt in range(8):
                            nc.tensor.matmul(ps[:], wq[:, kt, :], x1T_t[:, kt, :],
                                             start=(kt == 0), stop=(kt == 7))
                        nc.vector.tensor_copy(qT_t[:, m, :], ps[:])
                        wk = pp.tile([128, 8, 128], dt.float32r, name="wk", tag="wk", bufs=2)
                        nc.sync.dma_start(wk[:], _col3(kw_d[li], m * 128, (m + 1) * 128))
                        ps2 = pps.tile([128, TPC], dt.float32, name="ps_kp", tag="ps")
                        for kt in range(8):
                            nc.tensor.matmul(ps2[:], wk[:, kt, :], x1T_t[:, kt, :],
                                             start=(kt == 0), stop=(kt == 7))
                        kslc = pp.tile([128, TPC], dt.float32r, name="kslc", tag="kslc", bufs=2)
                        nc.vector.tensor_copy(kslc[:], ps2[:])
                        nc.sync.dma_start(kt_in3[:, m, :], kslc[:])
                    nc.gpsimd.collective_compute("AllGather", ALU.bypass, replica_groups=GROUPS,
                                                 ins=[kt_in[:].opt()], outs=[kt_out[:].opt()])
                    for nh2 in range(2):
                        wv = pp.tile([128, 8, 512], dt.float32r, name="wv", tag="wv", bufs=2)
                        nc.sync.dma_start(wv[:], _col3(vw_d[li], nh2 * 512, (nh2 + 1) * 512))
                        for j in range(NQT):
                            ps3 = pps.tile([128, 512], dt.float32, name="ps_vp", tag="ps")
                            for kt in range(8):
                                nc.tensor.matmul(ps3[:], x1T_t[:, kt, j * 128:(j + 1) * 128],
                                                 wv[:, kt, :], start=(kt == 0), stop=(kt == 7))
                            vslc = pp.tile([128, 512], dt.float32r, name="vslc", tag="vslc", bufs=2)
                            nc.vector.tensor_copy(vslc[:], ps3[:])
                            nc.sync.dma_start(v_in3[:, j, nh2 * 512:(nh2 + 1) * 512], vslc[:])
                    nc.gpsimd.collective_compute("AllGather", ALU.bypass, replica_groups=GROUPS,
                                                 ins=[v_in[:].opt()], outs=[v_out[:].opt()])

                # ---- hT + qe MLP + scores + top-k mask ----
                # selection path: fp32 on layer 0 (fragile margins), fp32r on
                # layer 1 (h already carries fp32r error; verified offline).
                qdt = dt.float32 if li == 0 else dt.float32r
                w1d = qew1_d if li == 0 else qew1r_d
                w2d = qew2_d if li == 0 else qew2r_d
                ceT_u = ceT_t if li == 0 else ceT_r
                with tc.tile_pool(name=f"qe{li}", bufs=1) as qp, \
                     tc.tile_pool(name=f"qeps{li}", bufs=1, space="PSUM") as qps:
                    hT_t = qp.tile([128, 8, TPC], qdt, name="hT_t")
                    for kt in range(8):
                        for j in range(NQT):
                            tp = qps.tile([128, 128], dt.float32, name="tp_h", tag="tp", bufs=2)
                            nc.tensor.transpose(tp[:], h_t[:, j, kt * 128:(kt + 1) * 128], idn_t[:])
                            nc.scalar.copy(hT_t[:, kt, j * 128:(j + 1) * 128], tp[:])
                    qe1_t = qp.tile([128, 4, TPC], qdt, name="qe1_t")
                    for m in range(4):
                        w = qp.tile([128, 8, 128], qdt, name="qw1", tag="qw1", bufs=2)
                        nc.sync.dma_start(w[:], _col3(w1d, m * 128, (m + 1) * 128))
                        ps = qps.tile([128, TPC], dt.float32, name="ps_q1", tag="ps", bufs=3)
                        for kt in range(8):
                            nc.tensor.matmul(ps[:], w[:, kt, :], hT_t[:, kt, :],
                                             start=(kt == 0), stop=(kt == 7))
                        nc.scalar.activation(qe1_t[:, m, :], ps[:], AF.Relu)
                    qeT_t = qp.tile([128, 8, TPC], qdt, name="qeT_t")
                    for m in range(8):
                        w = qp.tile([128, 4, 128], qdt, name="qw2", tag="qw2", bufs=2)
                        nc.sync.dma_start(w[:], _col3(w2d, m * 128, (m + 1) * 128))
                        ps = qps.tile([128, TPC], dt.float32, name="ps_q2", tag="ps", bufs=3)
                        for kt in range(4):
                            nc.tensor.matmul(ps[:], w[:, kt, :], qe1_t[:, kt, :],
                                             start=(kt == 0), stop=(kt == 3))
                        nc.vector.tensor_copy(qeT_t[:, m, :], ps[:])
                    for j in range(NQT):
                        ps = qps.tile([128, NC], dt.float32, name="ps_sc", tag="pssc", bufs=2)
                        for kt in range(8):
                            nc.tensor.matmul(ps[:], qeT_t[:, kt, j * 128:(j + 1) * 128],
                                             ceT_u[:, kt, :], start=(kt == 0), stop=(kt == 7))
                        sc = qp.tile([128, NC], dt.float32, name="sc", tag="sc", bufs=2)
                        nc.vector.tensor_copy(sc[:], ps[:])
                        rank = qp.tile([128, NC], dt.float32, name="rank", tag="rank", bufs=2)
                        nc.vector.tensor_copy(rank[:], rkinit_t[:])
                        for d in range(1, NC):
                            ge = qp.tile([128, NC - d], dt.float32, name="ge", tag="ge", bufs=2)
                            nc.vector.tensor_tensor(ge[:], sc[:, :NC - d], sc[:, d:], ALU.is_ge)
                            nc.vector.tensor_add(rank[:, d:], rank[:, d:], ge[:])
                            nc.vector.tensor_sub(rank[:, :NC - d], rank[:, :NC - d], ge[:])
                        m01 = qp.tile([128, NC], dt.float32, name="m01", tag="m01", bufs=2)
                        nc.vector.tensor_scalar(m01[:], rank[:], 7.5, None, ALU.is_le)
                        nc.vector.tensor_scalar(maskb_t[:, j, :], m01[:], 1.0, 1e30,
                                                ALU.subtract, ALU.mult)
                    # mbT[c, j, ii] = maskb[token(j,ii), c] as [1,512] rank-1 rows
                    for j in range(NQT):
                        tpm = qps.tile([16, 128], dt.float32, name="tp_m", tag="tp", bufs=2)
                        nc.tensor.transpose(tpm[:], maskb_t[:, j, :], idn_t[:])
                        nc.vector.tensor_copy(mbT_t[:, j, :], tpm[:])
                    for c in range(NC):
                        nc.sync.dma_start(mb1_t[0:1, c, :],
                                          mbT_t[c:c + 1, :, :].rearrange("c j i -> c (j i)"))

                # ---- attention: scoresT = K^T-major, mask via rank-1, exp, AV ----
                with tc.tile_pool(name=f"att{li}", bufs=1) as ap, \
                     tc.tile_pool(name=f"attw{li}", bufs=2) as awp, \
                     tc.tile_pool(name=f"attqk{li}", bufs=3, space="PSUM") as aps_qk, \
                     tc.tile_pool(name=f"attrs{li}", bufs=2, space="PSUM") as aps_rs, \
                     tc.tile_pool(name=f"attao{li}", bufs=2, space="PSUM") as aps_ao, \
                     tc.tile_pool(name=f"attrb{li}", bufs=1, space="PSUM") as aps_rb:
                    for hh in range(NH):
                        kT_h = awp.tile([128, S], dt.float32r, name="kT_h", tag="kT_h", bufs=2)
                        for r in range(4):
                            nc.scalar.dma_start(
                                kT_h[:, r * TPC:(r + 1) * TPC],
                                kt_out[r:r + 1, :, hh * TPC:(hh + 1) * TPC].rearrange("r p t -> (r p) t"))
                        v_h = awp.tile([128, NKT, HD], dt.float32r, name="v_h", tag="v_h", bufs=2)
                        for r in range(4):
                            nc.scalar.dma_start(
                                v_h[:, r * 4:(r + 1) * 4, :],
                                v_out[r:r + 1, :, :].rearrange("r (a p) b -> p (r a) b", p=128)[:, :, hh * HD:(hh + 1) * HD])
                        weT = ap.tile([128, NKT, TPC], dt.float32r, name="weT", tag="weT", bufs=2)
                        rs_ps = aps_rs.tile([1, TPC], dt.float32, name="rs_ps", tag="rs")
                        pao = aps_ao.tile([128, TPC], dt.float32, name="pao", tag="ao")
                        for c in range(NKT):
                            ps_s = aps_qk.tile([128, TPC], dt.float32, name="ps_s", tag="qk")
                            nc.tensor.matmul(ps_s[:], kT_h[:, c * 128:(c + 1) * 128],
                                             qT_t[:, hh, :], start=True, stop=False)
                            nc.tensor.matmul(ps_s[:], ones_r[0:1, :],
                                             mb1_t[0:1, c, :], start=False, stop=True)
                            nc.scalar.activation(weT[:, c, :], ps_s[:], AF.Exp, scale=SCALE)
                            nc.tensor.matmul(pao[:], v_h[:, c, :], weT[:, c, :],
                                             start=(c == 0), stop=(c == NKT - 1))
                        # softmax denominators: in-place tree-reduce over chunk
                        # tiles on DVE (after AV consumed weT), then a single
                        # cross-partition ones-matmul.
                        nc.vector.tensor_add(weT[:, 0:8, :], weT[:, 0:8, :], weT[:, 8:16, :])
                        nc.vector.tensor_add(weT[:, 0:4, :], weT[:, 0:4, :], weT[:, 4:8, :])
                        nc.vector.tensor_add(weT[:, 0:2, :], weT[:, 0:2, :], weT[:, 2:4, :])
                        nc.vector.tensor_add(weT[:, 0, :], weT[:, 0, :], weT[:, 1, :])
                        nc.tensor.matmul(rs_ps[:], ones_r[:, 0:1], weT[:, 0, :],
                                         start=True, stop=True)
                        rinv_sb = ap.tile([1, TPC], dt.float32, name="rinv", tag="rinv", bufs=2)
                        nc.vector.reciprocal(rinv_sb[:], rs_ps[:])
                        ps_rb = aps_rb.tile([128, TPC], dt.float32, name="ps_rb", tag="rb")
                        nc.tensor.matmul(ps_rb[:], ones_t[0:1, :], rinv_sb[:],
                                         start=True, stop=True)
                        rinvb = ap.tile([128, TPC], dt.float32, name="rinvb", tag="rinvb", bufs=2)
                        nc.vector.tensor_copy(rinvb[:], ps_rb[:])
                        nc.vector.tensor_tensor(aoT_t[:, hh, :], pao[:], rinvb[:], ALU.mult)

                # ---- o-projection direct [tok, feat] + residual add ----
                with tc.tile_pool(name=f"opj{li}", bufs=1) as op, \
                     tc.tile_pool(name=f"opjps{li}", bufs=4, space="PSUM") as ops:
                    wo_sb = op.tile([128, 8, H], dt.float32r, name="wo_sb")
                    nc.sync.dma_start(wo_sb[:], _col3(ow_d[li], 0, H))
                    for j in range(NQT):
                        for mh in range(2):
                            ps_o = ops.tile([128, 512], dt.float32, name="ps_o", tag="ps")
                            for kt in range(8):
                                nc.tensor.matmul(ps_o[:], aoT_t[:, kt, j * 128:(j + 1) * 128],
                                                 wo_sb[:, kt, mh * 512:(mh + 1) * 512],
                                                 start=(kt == 0), stop=(kt == 7))
                            nc.vector.tensor_tensor(h_t[:, j, mh * 512:(mh + 1) * 512],
                                                    h_t[:, j, mh * 512:(mh + 1) * 512],
                                                    ps_o[:], ALU.add)
                qp_cm.__exit__(None, None, None)

                # ---- LN2 + x2T ----
                x2T_t = LP.tile([128, 8, TPC], dt.float32r, name="x2T", tag="xT")
                with tc.tile_pool(name=f"ln2_{li}", bufs=2) as lp2, \
                     tc.tile_pool(name=f"ln2ps{li}", bufs=4, space="PSUM") as lps2:
                    for j in range(NQT):
                        x2 = lp2.tile([128, H], dt.float32, name="x2", tag="x2")
                        _emit_ln(nc, lp2, h_t[:, j, :], x2)
                        for kt in range(8):
                            tp = lps2.tile([128, 128], dt.float32, name="tp_x2", tag="tp")
                            nc.tensor.transpose(tp[:], x2[:, kt * 128:(kt + 1) * 128], idn_t[:])
                            nc.vector.tensor_copy(x2T_t[:, kt, j * 128:(j + 1) * 128], tp[:])

                # ---- FFN: w1 -> gelu -> w2 direct [tok, feat] ----
                with tc.tile_pool(name=f"ffn{li}", bufs=1) as fp, \
                     tc.tile_pool(name=f"ffnw{li}", bufs=2) as fwp, \
                     tc.tile_pool(name=f"ffnps{li}", bufs=3, space="PSUM") as fps, \
                     tc.tile_pool(name=f"ffnps2{li}", bufs=4, space="PSUM") as fps2:
                    gl_sb = fp.tile([128, 32, TPC], dt.float32r, name="gl_sb")
                    for ms in range(32):
                        w1s = fwp.tile([128, 8, 128], dt.float32r, name="w1s", tag="w1s")
                        nc.sync.dma_start(w1s[:], _col3(fw1_d[li], ms * 128, (ms + 1) * 128))
                        psg = fps.tile([128, TPC], dt.float32, name="ps_g", tag="psg")
                        for kt in range(8):
                            nc.tensor.matmul(psg[:], w1s[:, kt, :], x2T_t[:, kt, :],
                                             start=(kt == 0), stop=(kt == 7))
                        nc.scalar.activation(gl_sb[:, ms, :], psg[:], AF.Gelu)
                    for mq in range(4):
                        w2q = fwp.tile([128, 32, 256], dt.float32r, name="w2q", tag="w2q", bufs=2)
                        nc.sync.dma_start(w2q[:], _col3(fw2_d[li], mq * 256, (mq + 1) * 256))
                        for j in range(NQT):
                            ps_f = fps2.tile([128, 256], dt.float32, name="ps_f", tag="psf")
                            for kt in range(32):
                                nc.tensor.matmul(ps_f[:], gl_sb[:, kt, j * 128:(j + 1) * 128],
                                                 w2q[:, kt, :], start=(kt == 0), stop=(kt == 31))
                            nc.vector.tensor_tensor(h_t[:, j, mq * 256:(mq + 1) * 256],
                                                    h_t[:, j, mq * 256:(mq + 1) * 256],
                                                    ps_f[:], ALU.add)

        # ---------------- logits: h all-gather + vocab-sharded matmul ----------------
        with tc.tile_pool(name="lg", bufs=1) as gp, \
             tc.tile_pool(name="lgw", bufs=2) as gwp, \
             tc.tile_pool(name="lgps", bufs=2, space="PSUM") as gps, \
             tc.tile_pool(name="lgps2", bufs=6, space="PSUM") as gps2:
            hTf = gp.tile([128, 8, TPC], dt.float32r, name="hTf")
            for kt in range(8):
                for j in range(NQT):
                    tp = gps.tile([128, 128], dt.float32, name="tp_hf", tag="tp")
                    nc.tensor.transpose(tp[:], h_t[:, j, kt * 128:(kt + 1) * 128], idn_t[:])
                    nc.vector.tensor_copy(hTf[:, kt, j * 128:(j + 1) * 128], tp[:])
            nc.sync.dma_start(hag_in[:], hTf[:].rearrange("p a b -> p (a b)"))
            nc.gpsimd.collective_compute("AllGather", ALU.bypass, replica_groups=GROUPS,
                                         ins=[hag_in[:].opt()], outs=[hag_out[:].opt()])
            hT_full = gp.tile([128, 8, S], dt.float32r, name="hT_full")
            for r in range(4):
                nc.scalar.dma_start(
                    hT_full[:, :, r * TPC:(r + 1) * TPC],
                    hag_out[r:r + 1, :, :].rearrange("r p (a b) -> (r p) a b", a=8))
            ntiles = [(n * 512, 512) for n in range(VS // 512)]
            if VS % 512:
                ntiles.append((VS - VS % 512, VS % 512))
            for (noff, nsz) in ntiles:
                wf = gwp.tile([128, 8, 512], dt.float32r, name="ow_f", tag="ow", bufs=2)
                nc.sync.dma_start(wf[:, :, :nsz],
                                  outw_d.rearrange("(kt kp) n -> kp kt n", kp=128)[:, :, noff:noff + nsz])
                for tt in range(S // 128):
                    ps = gps2.tile([128, 512], dt.float32, name="ps_lg", tag="ps")
                    for kt in range(8):
                        nc.tensor.matmul(ps[:, :nsz], hT_full[:, kt, tt * 128:(tt + 1) * 128],
                                         wf[:, kt, :nsz], start=(kt == 0), stop=(kt == 7))
                    ot = gp.tile([128, 512], dt.float32, name="ot", tag="ot", bufs=6)
                    nc.vector.tensor_copy(ot[:, :nsz], ps[:, :nsz])
                    nc.scalar.dma_start(logits_d[tt * 128:(tt + 1) * 128, noff:noff + nsz],
                                        ot[:, :nsz])

    nc.compile()
    return nc


def _prep_inputs(inputs):
    f32 = lambda x: np.ascontiguousarray(np.asarray(x, dtype=np.float32))
    ids = np.asarray(inputs["input_ids"]).astype(np.int32)
    common = {
        "tok_emb": f32(inputs["tok_emb"]), "in_w": f32(inputs["in_w"]),
        "qe_w1": f32(inputs["qe_w1"]), "qe_w2": f32(inputs["qe_w2"]),
        "qe_w1r": f32(inputs["qe_w1"]), "qe_w2r": f32(inputs["qe_w2"]),
        "ch_w1": f32(inputs["ch_w1"]), "ch_w2": f32(inputs["ch_w2"]),
        "idn": np.eye(128, dtype=np.float32),
        "ones": np.ones((128, 128), dtype=np.float32),
        "c_mean": np.full((128, 1), 1.0 / CS, dtype=np.float32),
        "rank_init": np.ascontiguousarray(
            np.broadcast_to(NC - 1 - np.arange(NC, dtype=np.float32), (128, NC))),
    }
    for i in range(L):
        for nm in ["q_w", "k_w", "v_w", "o_w", "f_w1", "f_w2"]:
            common[f"l{i}_{nm}"] = f32(np.asarray(inputs[nm])[i])
    pos = f32(inputs["pos_emb"])
    outw = f32(inputs["out_w"])
    in_maps = []
    for c in range(8):
        b, q = c // 4, c % 4
        off = q * TPC
        m = dict(common)
        m["ids_col"] = np.ascontiguousarray(ids[b, off:off + TPC].reshape(NQT, 128).T)
        m["pos"] = np.ascontiguousarray(pos[off:off + TPC])
        m["out_w_sl"] = np.ascontiguousarray(outw[:, q * VS:(q + 1) * VS])
        in_maps.append(m)
    return in_maps


def kernel(**inputs) -> np.ndarray:
    # biases / LN affine params are zero / one for this model; the kernel
    # implements that fast path (verified here).
    for k in ["in_b", "ch_b1", "ch_b2", "qe_b1", "qe_b2", "q_b", "k_b", "v_b",
              "o_b", "f_b1", "f_b2", "ln1_b", "ln2_b", "out_b"]:
        assert not np.any(np.asarray(inputs[k])), f"nonzero bias {k} unsupported"
    for k in ["ln1_g", "ln2_g"]:
        assert np.all(np.asarray(inputs[k]) == 1.0), f"non-unit {k} unsupported"

    if "nc" not in _CACHE:
        _CACHE["nc"] = _build()
    nc = _CACHE["nc"]
    in_maps = _prep_inputs(inputs)
    res = run_bass_kernel_spmd(nc, in_maps, list(range(8)))
    out = np.empty((B, S, V), dtype=np.float32)
    for c in range(8):
        b, q = c // 4, c % 4
        out[b, :, q * VS:(q + 1) * VS] = res.results[c]["logits"]
    return out


# revision 38
# speedup vs baseline: 1.6737x; 1.0418x over previous
"""GCA model (retrieval_knn) Trainium2 kernel: 8 NeuronCores, token-sharded.

Sharding: core c -> (batch b=c//4, quarter q=c%4): 512 contiguous tokens.
KV and chunk encodings all-gathered within each batch's 4-core group.
Logits are vocab-sharded: final h is all-gathered in the group and each
core computes its batch's full 2048 tokens x an 8000-column vocab slice.

Precision: fp32 matmuls on the top-k-selection path (embeddings/in_w,
chunk MLP, qe MLPs, scores); float32r (full-rate) everywhere else.
Attention computes scores^T [keys, queries] directly (no weight
transposes); the chunk mask is applied as a rank-1 matmul accumulate and
softmax normalization is folded into the PSUM->SBUF copy.
"""
import numpy as np
from contextlib import ExitStack

import concourse.bass as bass
import concourse.tile as tile
import concourse.mybir as mybir
from concourse import bacc
from concourse.bass_utils import run_bass_kernel_spmd

dt = mybir.dt
AF = mybir.ActivationFunctionType
ALU = mybir.AluOpType

B, S, E, H, NH, L, V = 2, 2048, 1024, 1024, 8, 2, 32000
CS, K = 128, 8
HD = H // NH
SCALE = HD ** -0.5
TPC = 512            # tokens per core
NQT = TPC // 128     # 4 q-tiles per core
NC = S // CS         # 16 chunks
NKT = S // 128       # 16 key tiles
VS = V // 4          # vocab slice per core
GROUPS = [[0, 1, 2, 3], [4, 5, 6, 7]]

_CACHE = {}


def _col3(wap, msl0, msl1):
    """DRAM [K, M] -> [128, K//128, msl1-msl0] lhsT-tile view."""
    return wap.rearrange("(kt kp) n -> kp kt n", kp=128)[:, :, msl0:msl1]


def _emit_ln(nc, pool, h_ap, out_ap):
    """LayerNorm of [128, H] h_ap -> out_ap (gamma=1, beta=0 fast path)."""
    sq = pool.tile([128, H], dt.float32, name="ln_sq", tag="ln_sq")
    ss = pool.tile([128, 1], dt.float32, name="ln_ss", tag="ln_ss")
    nc.scalar.activation(sq[:], h_ap, AF.Square, accum_out=ss[:])
    s = pool.tile([128, 1], dt.float32, name="ln_s", tag="ln_s")
    nc.vector.reduce_sum(s[:], h_ap, axis=mybir.AxisListType.X)
    mean = pool.tile([128, 1], dt.float32, name="ln_m", tag="ln_m")
    nc.vector.tensor_scalar(mean[:], s[:], 1.0 / H, None, ALU.mult)
    msq = pool.tile([128, 1], dt.float32, name="ln_msq", tag="ln_msq")
    nc.vector.tensor_tensor(msq[:], mean[:], mean[:], ALU.mult)
    var = pool.tile([128, 1], dt.float32, name="ln_v", tag="ln_v")
    nc.vector.tensor_scalar(var[:], ss[:], 1.0 / H, 1e-5, ALU.mult, ALU.add)
    nc.vector.tensor_sub(var[:], var[:], msq[:])
    sd = pool.tile([128, 1], dt.float32, name="ln_sd", tag="ln_sd")
    nc.scalar.activation(sd[:], var[:], AF.Sqrt)
    r = pool.tile([128, 1], dt.float32, name="ln_r", tag="ln_r")
    nc.vector.reciprocal(r[:], sd[:])
    # one Newton step: r = r*(1.5 - 0.5*var*r*r)
    r2 = pool.tile([128, 1], dt.float32, name="ln_r2", tag="ln_r2")
    nc.vector.tensor_tensor(r2[:], r[:], r[:], ALU.mult)
    nc.vector.tensor_tensor(r2[:], r2[:], var[:], ALU.mult)
    nc.vector.tensor_scalar(r2[:], r2[:], -0.5, 1.5, ALU.mult, ALU.add)
    nc.vector.tensor_tensor(r[:], r[:], r2[:], ALU.mult)
    nc.vector.tensor_scalar(out_ap, h_ap, mean[:], r[:], ALU.subtract, ALU.mult)


def _build():
    nc = bacc.Bacc("TRN2", target_bir_lowering=False, debug=False, num_devices=8)

    def din(name, shape, dtype=dt.float32):
        return nc.dram_tensor(name, shape, dtype, kind="ExternalInput").ap()

    ids_d = din("ids_col", [128, NQT], dt.int32)
    pos_d = din("pos", [TPC, E])
    temb_d = din("tok_emb", [V, E])
    inw_d = din("in_w", [E, H])
    qew1_d = din("qe_w1", [H, H // 2])
    qew2_d = din("qe_w2", [H // 2, H])
    qew1r_d = din("qe_w1r", [H, H // 2], dt.float32r)
    qew2r_d = din("qe_w2r", [H // 2, H], dt.float32r)
    chw1_d = din("ch_w1", [H, H // 2])
    chw2_d = din("ch_w2", [H // 2, H])
    qw_d = [din(f"l{i}_q_w", [H, H], dt.float32r) for i in range(L)]
    kw_d = [din(f"l{i}_k_w", [H, H], dt.float32r) for i in range(L)]
    vw_d = [din(f"l{i}_v_w", [H, H], dt.float32r) for i in range(L)]
    ow_d = [din(f"l{i}_o_w", [H, H], dt.float32r) for i in range(L)]
    fw1_d = [din(f"l{i}_f_w1", [H, 4 * H], dt.float32r) for i in range(L)]
    fw2_d = [din(f"l{i}_f_w2", [4 * H, H], dt.float32r) for i in range(L)]
    outw_d = din("out_w_sl", [H, VS], dt.float32r)
    idn_d = din("idn", [128, 128])
    ones_d = din("ones", [128, 128])
    cmean_d = din("c_mean", [128, 1])
    rkinit_d = din("rank_init", [128, NC])

    logits_d = nc.dram_tensor("logits", [S, VS], dt.float32, kind="ExternalOutput").ap()

    with ExitStack() as ctx:
        tc = ctx.enter_context(tile.TileContext(nc))
        P = ctx.enter_context(tc.tile_pool(name="persist", bufs=1))
        dramp = ctx.enter_context(tc.tile_pool(name="dramp", bufs=1, space="DRAM"))

        idn_t = P.tile([128, 128], dt.float32, name="idn_t")
        nc.sync.dma_start(idn_t[:], idn_d)
        ones_t = P.tile([128, 128], dt.float32, name="ones_t")
        nc.sync.dma_start(ones_t[:], ones_d)
        ones_r = P.tile([128, 128], dt.float32r, name="ones_r")
        nc.vector.tensor_copy(ones_r[:], ones_t[:])

        cmean_t = P.tile([128, 1], dt.float32, name="cmean_t")
        nc.sync.dma_start(cmean_t[:], cmean_d)
        rkinit_t = P.tile([128, NC], dt.float32, name="rkinit_t")
        nc.sync.dma_start(rkinit_t[:], rkinit_d)

        h_t = P.tile([128, NQT, H], dt.float32, name="h_t")          # residual [tok, H]
        ceT_t = P.tile([128, 8, NC], dt.float32, name="ceT_t")       # [hp, htile, chunk]
        ceT_r = P.tile([128, 8, NC], dt.float32r, name="ceT_r")
        maskb_t = P.tile([128, NQT, NC], dt.float32, name="maskb_t")

        # ---------------- embeddings + in_w ----------------
        with tc.tile_pool(name="emb", bufs=1) as embp, \
             tc.tile_pool(name="embps", bufs=1, space="PSUM") as embps:
            ids_t = embp.tile([128, NQT], dt.int32, name="ids_t")
            nc.sync.dma_start(ids_t[:], ids_d)
            emb_t = embp.tile([128, NQT, E], dt.float32, name="emb_t")
            for j in range(NQT):
                nc.gpsimd.indirect_dma_start(
                    out=emb_t[:, j, :], out_offset=None, in_=temb_d,
                    in_offset=bass.IndirectOffsetOnAxis(ap=ids_t[:, j:j + 1], axis=0))
                pos_t = embp.tile([128, E], dt.float32, name="pos_t", tag="pos", bufs=2)
                nc.sync.dma_start(pos_t[:], pos_d[j * 128:(j + 1) * 128, :])
                nc.vector.tensor_add(emb_t[:, j, :], emb_t[:, j, :], pos_t[:])
            embT_t = embp.tile([128, 8, TPC], dt.float32, name="embT_t")
            for kt in range(8):
                for j in range(NQT):
                    tp = embps.tile([128, 128], dt.float32, name="tp_e", tag="tp", bufs=3)
                    nc.tensor.transpose(tp[:], emb_t[:, j, kt * 128:(kt + 1) * 128], idn_t[:])
                    nc.vector.tensor_copy(embT_t[:, kt, j * 128:(j + 1) * 128], tp[:])
            inw_sb = embp.tile([128, 8, H], dt.float32, name="inw_sb")
            nc.sync.dma_start(inw_sb[:], inw_d.rearrange("(kt kp) n -> kp kt n", kp=128))
            for j in range(NQT):
                for nh in range(2):
                    ps = embps.tile([128, 512], dt.float32, name="ps_h0", tag="ps", bufs=4)
                    for kt in range(8):
                        nc.tensor.matmul(ps[:], embT_t[:, kt, j * 128:(j + 1) * 128],
                                         inw_sb[:, kt, nh * 512:(nh + 1) * 512],
                                         start=(kt == 0), stop=(kt == 7))
                    nc.vector.tensor_copy(h_t[:, j, nh * 512:(nh + 1) * 512], ps[:])

        # ---------------- chunk encodings (fp32) ----------------
        with tc.tile_pool(name="ch", bufs=1) as chp, \
             tc.tile_pool(name="chps", bufs=2, space="PSUM") as chps:
            avg_t = chp.tile([NQT, H], dt.float32, name="avg_t")
            for j in range(NQT):
                for nh in range(2):
                    ps = chps.tile([1, 512], dt.float32, name="ps_av", tag="psa")
                    nc.tensor.matmul(ps[:], cmean_t[:], h_t[:, j, nh * 512:(nh + 1) * 512],
                                     start=True, stop=True)
                    av1 = chp.tile([1, 512], dt.float32, name="av1", tag="av1", bufs=2)
                    nc.vector.tensor_copy(av1[:], ps[:])
                    nc.scalar.dma_start(avg_t[j:j + 1, nh * 512:(nh + 1) * 512], av1[:])
            avgT_t = chp.tile([128, 8, NQT], dt.float32, name="avgT_t")
            for kt in range(8):
                tp = chps.tile([128, NQT], dt.float32, name="tp_a", tag="tpa")
                nc.tensor.transpose(tp[:, :], avg_t[:, kt * 128:(kt + 1) * 128], idn_t[:NQT, :NQT])
                nc.vector.tensor_copy(avgT_t[:, kt, :], tp[:, :])
            hid_t = chp.tile([128, 4, NQT], dt.float32, name="hid_t")
            w1 = chp.tile([128, 8, 512], dt.float32, name="chw1_t")
            nc.sync.dma_start(w1[:], chw1_d.rearrange("(kt kp) n -> kp kt n", kp=128))
            for m in range(4):
                ps = chps.tile([128, NQT], dt.float32, name="ps_c1", tag="psc")
                for kt in range(8):
                    nc.tensor.matmul(ps[:], w1[:, kt, m * 128:(m + 1) * 128],
                                     avgT_t[:, kt, :], start=(kt == 0), stop=(kt == 7))
                nc.scalar.activation(hid_t[:, m, :], ps[:], AF.Relu)
            w2 = chp.tile([128, 4, 1024], dt.float32, name="chw2_t")
            nc.sync.dma_start(w2[:], chw2_d.rearrange("(kt kp) n -> kp kt n", kp=128))
            ce_loc = chp.tile([128, 8, NQT], dt.float32, name="ce_loc")
            for m in range(8):
                ps = chps.tile([128, NQT], dt.float32, name="ps_c2", tag="psc")
                for kt in range(4):
                    nc.tensor.matmul(ps[:], w2[:, kt, m * 128:(m + 1) * 128],
                                     hid_t[:, kt, :], start=(kt == 0), stop=(kt == 3))
                nc.vector.tensor_copy(ce_loc[:, m, :], ps[:])
            ce_in = dramp.tile([128, 8 * NQT], dt.float32, name="ce_in")
            ce_out = dramp.tile([4, 128, 8 * NQT], dt.float32, name="ce_out")
            nc.sync.dma_start(ce_in[:], ce_loc[:].rearrange("p a b -> p (a b)"))
            nc.gpsimd.collective_compute(
                "AllGather", ALU.bypass, replica_groups=GROUPS,
                ins=[ce_in[:].opt()], outs=[ce_out[:].opt()])
            for t in range(8):
                nc.sync.dma_start(
                    ceT_t[:, t, :].rearrange("p (r c) -> p r c", r=4),
                    ce_out[:, :, t * NQT:(t + 1) * NQT].rearrange("r p c -> p r c"))
            nc.vector.tensor_copy(ceT_r[:], ceT_t[:])

        kv_dram = []
        for i in range(L):
            k_io = [(dramp.tile([128, TPC], dt.float32r, name=f"kt_in{i}_{m}"),
                     dramp.tile([4, 128, TPC], dt.float32r, name=f"kt_out{i}_{m}"))
                    for m in range(NH)]
            v_io = [(dramp.tile([TPC, 512], dt.float32r, name=f"v_in{i}_{n}"),
                     dramp.tile([4, TPC, 512], dt.float32r, name=f"v_out{i}_{n}"))
                    for n in range(2)]
            kv_dram.append((k_io, v_io))
        hag_in = dramp.tile([128, 8 * TPC], dt.float32r, name="hag_in")
        hag_out = dramp.tile([4, 128, 8 * TPC], dt.float32r, name="hag_out")

        for li in range(L):
            with tc.tile_pool(name=f"layer{li}", bufs=1) as LP:
                qp_cm = tc.tile_pool(name=f"qkao{li}", bufs=1)
                QP = qp_cm.__enter__()
                x1T_t = LP.tile([128, 8, TPC], dt.float32r, name="x1T", tag="xT")
                qT_t = QP.tile([128, 8, TPC], dt.float32r, name="qT")
                aoT_t = QP.tile([128, 8, TPC], dt.float32r, name="aoT")
                mbT_t = QP.tile([16, NQT, 128], dt.float32r, name="mbT")
                mbb_t = QP.tile([128, NC, TPC], dt.float32, name="mbb")

                # ---- LN1 + x1T ----
                with tc.tile_pool(name=f"ln1_{li}", bufs=2) as lp, \
                     tc.tile_pool(name=f"ln1ps{li}", bufs=4, space="PSUM") as lps:
                    for j in range(NQT):
                        x1 = lp.tile([128, H], dt.float32, name="x1", tag="x1")
                        _emit_ln(nc, lp, h_t[:, j, :], x1)
                        for kt in range(8):
                            tp = lps.tile([128, 128], dt.float32, name="tp_x", tag="tp")
                            nc.tensor.transpose(tp[:], x1[:, kt * 128:(kt + 1) * 128], idn_t[:])
                            nc.vector.tensor_copy(x1T_t[:, kt, j * 128:(j + 1) * 128], tp[:])

                # ---- QKV projections + per-head/half KV all-gathers ----
                k_io, v_io = kv_dram[li]
                with tc.tile_pool(name=f"qkv{li}", bufs=1) as pp, \
                     tc.tile_pool(name=f"qkvps{li}", bufs=4, space="PSUM") as pps:
                    for m in range(8):
                        wk = pp.tile([128, 8, 128], dt.float32r, name="wk", tag="wk", bufs=2)
                        nc.sync.dma_start(wk[:], _col3(kw_d[li], m * 128, (m + 1) * 128))
                        ps2 = pps.tile([128, TPC], dt.float32, name="ps_kp", tag="ps")
                        for kt in range(8):
                            nc.tensor.matmul(ps2[:], wk[:, kt, :], x1T_t[:, kt, :],
                                             start=(kt == 0), stop=(kt == 7))
                        kslc = pp.tile([128, TPC], dt.float32r, name="kslc", tag="kslc", bufs=2)
                        nc.vector.tensor_copy(kslc[:], ps2[:])
                        nc.sync.dma_start(k_io[m][0][:], kslc[:])
                        nc.gpsimd.collective_compute(
                            "AllGather", ALU.bypass, replica_groups=GROUPS,
                            ins=[k_io[m][0][:].opt()], outs=[k_io[m][1][:].opt()])
                        wq = pp.tile([128, 8, 128], dt.float32r, name="wq", tag="wq", bufs=2)
                        nc.sync.dma_start(wq[:], _col3(qw_d[li], m * 128, (m + 1) * 128))
                        ps = pps.tile([128, TPC], dt.float32, name="ps_qp", tag="ps")
                        for kt in range(8):
                            nc.tensor.matmul(ps[:], wq[:, kt, :], x1T_t[:, kt, :],
                                             start=(kt == 0), stop=(kt == 7))
                        nc.vector.tensor_copy(qT_t[:, m, :], ps[:])
                    for nh2 in range(2):
                        v_in3 = v_io[nh2][0][:].rearrange("(a p) b -> p a b", p=128)
                        wv = pp.tile([128, 8, 512], dt.float32r, name="wv", tag="wv", bufs=2)
                        nc.sync.dma_start(wv[:], _col3(vw_d[li], nh2 * 512, (nh2 + 1) * 512))
                        for j in range(NQT):
                            ps3 = pps.tile([128, 512], dt.float32, name="ps_vp", tag="ps")
                            for kt in range(8):
                                nc.tensor.matmul(ps3[:], x1T_t[:, kt, j * 128:(j + 1) * 128],
                                                 wv[:, kt, :], start=(kt == 0), stop=(kt == 7))
                            vslc = pp.tile([128, 512], dt.float32r, name="vslc", tag="vslc", bufs=2)
                            nc.vector.tensor_copy(vslc[:], ps3[:])
                            nc.sync.dma_start(v_in3[:, j, :], vslc[:])
                        nc.gpsimd.collective_compute(
                            "AllGather", ALU.bypass, replica_groups=GROUPS,
                            ins=[v_io[nh2][0][:].opt()], outs=[v_io[nh2][1][:].opt()])

                # ---- hT + qe MLP + scores + top-k mask ----
                # selection path: fp32 on layer 0 (fragile margins), fp32r on
                # layer 1 (h already carries fp32r error; verified offline).
                qdt = dt.float32 if li == 0 else dt.float32r
                w1d = qew1_d if li == 0 else qew1r_d
                w2d = qew2_d if li == 0 else qew2r_d
                ceT_u = ceT_t if li == 0 else ceT_r
                with tc.tile_pool(name=f"qe{li}", bufs=1) as qp, \
                     tc.tile_pool(name=f"qeps{li}", bufs=1, space="PSUM") as qps:
                    hT_t = qp.tile([128, 8, TPC], qdt, name="hT_t")
                    for kt in range(8):
                        for j in range(NQT):
                            tp = qps.tile([128, 128], dt.float32, name="tp_h", tag="tp", bufs=2)
                            nc.tensor.transpose(tp[:], h_t[:, j, kt * 128:(kt + 1) * 128], idn_t[:])
                            nc.vector.tensor_copy(hT_t[:, kt, j * 128:(j + 1) * 128], tp[:])
                    qe1_t = qp.tile([128, 4, TPC], qdt, name="qe1_t")
                    for m in range(4):
                        w = qp.tile([128, 8, 128], qdt, name="qw1", tag="qw1", bufs=2)
                        nc.sync.dma_start(w[:], _col3(w1d, m * 128, (m + 1) * 128))
                        ps = qps.tile([128, TPC], dt.float32, name="ps_q1", tag="ps", bufs=3)
                        for kt in range(8):
                            nc.tensor.matmul(ps[:], w[:, kt, :], hT_t[:, kt, :],
                                             start=(kt == 0), stop=(kt == 7))
                        nc.scalar.activation(qe1_t[:, m, :], ps[:], AF.Relu)
                    qeT_t = qp.tile([128, 8, TPC], qdt, name="qeT_t")
                    for m in range(8):
                        w = qp.tile([128, 4, 128], qdt, name="qw2", tag="qw2", bufs=2)
                        nc.sync.dma_start(w[:], _col3(w2d, m * 128, (m + 1) * 128))
                        ps = qps.tile([128, TPC], dt.float32, name="ps_q2", tag="ps", bufs=3)
                        for kt in range(4):
                            nc.tensor.matmul(ps[:], w[:, kt, :], qe1_t[:, kt, :],
                                             start=(kt == 0), stop=(kt == 3))
                        nc.vector.tensor_copy(qeT_t[:, m, :], ps[:])
                    for j in range(NQT):
                        ps = qps.tile([128, NC], dt.float32, name="ps_sc", tag="pssc", bufs=2)
                        for kt in range(8):
                            nc.tensor.matmul(ps[:], qeT_t[:, kt, j * 128:(j + 1) * 128],
                                             ceT_u[:, kt, :], start=(kt == 0), stop=(kt == 7))
                        sc = qp.tile([128, NC], dt.float32, name="sc", tag="sc", bufs=2)
                        nc.vector.tensor_copy(sc[:], ps[:])
                        rank = qp.tile([128, NC], dt.float32, name="rank", tag="rank", bufs=2)
                        nc.vector.tensor_copy(rank[:], rkinit_t[:])
                        for d in range(1, NC):
                            ge = qp.tile([128, NC - d], dt.float32, name="ge", tag="ge", bufs=2)
                            nc.vector.tensor_tensor(ge[:], sc[:, :NC - d], sc[:, d:], ALU.is_ge)
                            nc.vector.tensor_add(rank[:, d:], rank[:, d:], ge[:])
                            nc.vector.tensor_sub(rank[:, :NC - d], rank[:, :NC - d], ge[:])
                        m01 = qp.tile([128, NC], dt.float32, name="m01", tag="m01", bufs=2)
                        nc.vector.tensor_scalar(m01[:], rank[:], 7.5, None, ALU.is_le)
                        nc.vector.tensor_scalar(maskb_t[:, j, :], m01[:], 1.0, 1e30,
                                                ALU.subtract, ALU.mult)
                    # mbT[c, j, ii] = maskb[token(j,ii), c] as [1,512] rank-1 rows
                    for j in range(NQT):
                        tpm = qps.tile([16, 128], dt.float32, name="tp_m", tag="tp", bufs=2)
                        nc.tensor.transpose(tpm[:], maskb_t[:, j, :], idn_t[:])
                        nc.vector.tensor_copy(mbT_t[:, j, :], tpm[:])
                    # broadcast mask rows to all partitions: mbb[p, c, i] = maskb[i, c]
                    for c in range(NC):
                        mb1 = qp.tile([1, TPC], dt.float32r, name="mb1", tag="mb1", bufs=2)
                        nc.sync.dma_start(mb1[:],
                                          mbT_t[c:c + 1, :, :].rearrange("c j i -> c (j i)"))
                        psb = qps.tile([128, TPC], dt.float32, name="ps_b", tag="ps", bufs=3)
                        nc.tensor.matmul(psb[:], ones_r[0:1, :], mb1[:], start=True, stop=True)
                        nc.vector.tensor_copy(mbb_t[:, c, :], psb[:])

                # ---- attention: scoresT = K^T-major, mask via rank-1, exp, AV ----
                with tc.tile_pool(name=f"att{li}", bufs=1) as ap, \
                     tc.tile_pool(name=f"attw{li}", bufs=2) as awp, \
                     tc.tile_pool(name=f"attqk{li}", bufs=4, space="PSUM") as aps_qk, \
                     tc.tile_pool(name=f"attrs{li}", bufs=1, space="PSUM") as aps_rs, \
                     tc.tile_pool(name=f"attao{li}", bufs=2, space="PSUM") as aps_ao, \
                     tc.tile_pool(name=f"attrb{li}", bufs=1, space="PSUM") as aps_rb:
                    for hh in range(NH):
                        kT_h = awp.tile([128, S], dt.float32r, name="kT_h", tag="kT_h", bufs=2)
                        for r in range(4):
                            nc.scalar.dma_start(
                                kT_h[:, r * TPC:(r + 1) * TPC],
                                k_io[hh][1][r:r + 1, :, :].rearrange("r p t -> (r p) t"))
                        v_h = awp.tile([128, NKT, HD], dt.float32r, name="v_h", tag="v_h", bufs=2)
                        for r in range(4):
                            nc.scalar.dma_start(
                                v_h[:, r * 4:(r + 1) * 4, :],
                                v_io[hh // 4][1][r:r + 1, :, :].rearrange(
                                    "r (a p) b -> p (r a) b", p=128)[:, :, (hh % 4) * HD:(hh % 4 + 1) * HD])
                        weT = ap.tile([128, NKT, TPC], dt.float32r, name="weT", tag="weT", bufs=2)
                        rs_ps = aps_rs.tile([1, TPC], dt.float32, name="rs_ps", tag="rs")
                        pao = aps_ao.tile([128, TPC], dt.float32, name="pao", tag="ao")
                        for c in range(NKT):
                            ps_s = aps_qk.tile([128, TPC], dt.float32, name="ps_s", tag="qk")
                            nc.tensor.matmul(ps_s[:], kT_h[:, c * 128:(c + 1) * 128],
                                             qT_t[:, hh, :], start=True, stop=True)
                            nc.vector.tensor_tensor(ps_s[:], ps_s[:], mbb_t[:, c, :], ALU.add)
                            nc.scalar.activation(weT[:, c, :], ps_s[:], AF.Exp, scale=SCALE)
                            nc.tensor.matmul(pao[:], v_h[:, c, :], weT[:, c, :],
                                             start=(c == 0), stop=(c == NKT - 1))
                        # softmax denominators: in-place tree-reduce over chunk
                        # tiles on DVE (after AV consumed weT), then a single
                        # cross-partition ones-matmul.
                        nc.vector.tensor_add(weT[:, 0:8, :], weT[:, 0:8, :], weT[:, 8:16, :])
                        nc.vector.tensor_add(weT[:, 0:4, :], weT[:, 0:4, :], weT[:, 4:8, :])
                        nc.vector.tensor_add(weT[:, 0:2, :], weT[:, 0:2, :], weT[:, 2:4, :])
                        nc.vector.tensor_add(weT[:, 0, :], weT[:, 0, :], weT[:, 1, :])
                        nc.tensor.matmul(rs_ps[:], ones_r[:, 0:1], weT[:, 0, :],
                                         start=True, stop=True)
                        rinv_sb = ap.tile([1, TPC], dt.float32, name="rinv", tag="rinv", bufs=2)
                        nc.vector.reciprocal(rinv_sb[:], rs_ps[:])
                        ps_rb = aps_rb.tile([128, TPC], dt.float32, name="ps_rb", tag="rb")
                        nc.tensor.matmul(ps_rb[:], ones_t[0:1, :], rinv_sb[:],
                                         start=True, stop=True)
                        rinvb = ap.tile([128, TPC], dt.float32, name="rinvb", tag="rinvb", bufs=2)
                        nc.vector.tensor_copy(rinvb[:], ps_rb[:])
                        nc.vector.tensor_tensor(aoT_t[:, hh, :], pao[:], rinvb[:], ALU.mult)

                # ---- o-projection direct [tok, feat] + residual add ----
                with tc.tile_pool(name=f"opj{li}", bufs=1) as op, \
                     tc.tile_pool(name=f"opjps{li}", bufs=4, space="PSUM") as ops:
                    wo_sb = op.tile([128, 8, H], dt.float32r, name="wo_sb")
                    nc.sync.dma_start(wo_sb[:], _col3(ow_d[li], 0, H))
                    for j in range(NQT):
                        for mh in range(2):
                            ps_o = ops.tile([128, 512], dt.float32, name="ps_o", tag="ps")
                            for kt in range(8):
                                nc.tensor.matmul(ps_o[:], aoT_t[:, kt, j * 128:(j + 1) * 128],
                                                 wo_sb[:, kt, mh * 512:(mh + 1) * 512],
                                                 start=(kt == 0), stop=(kt == 7))
                            nc.vector.tensor_tensor(h_t[:, j, mh * 512:(mh + 1) * 512],
                                                    h_t[:, j, mh * 512:(mh + 1) * 512],
                                                    ps_o[:], ALU.add)
                qp_cm.__exit__(None, None, None)

                # ---- LN2 + x2T ----
                x2T_t = LP.tile([128, 8, TPC], dt.float32r, name="x2T", tag="xT")
                with tc.tile_pool(name=f"ln2_{li}", bufs=2) as lp2, \
                     tc.tile_pool(name=f"ln2ps{li}", bufs=4, space="PSUM") as lps2:
                    for j in range(NQT):
                        x2 = lp2.tile([128, H], dt.float32, name="x2", tag="x2")
                        _emit_ln(nc, lp2, h_t[:, j, :], x2)
                        for kt in range(8):
                            tp = lps2.tile([128, 128], dt.float32, name="tp_x2", tag="tp")
                            nc.tensor.transpose(tp[:], x2[:, kt * 128:(kt + 1) * 128], idn_t[:])
                            nc.vector.tensor_copy(x2T_t[:, kt, j * 128:(j + 1) * 128], tp[:])

                # ---- FFN: w1 -> gelu -> w2 direct [tok, feat] ----
                with tc.tile_pool(name=f"ffn{li}", bufs=1) as fp, \
                     tc.tile_pool(name=f"ffnw{li}", bufs=2) as fwp, \
                     tc.tile_pool(name=f"ffnps{li}", bufs=3, space="PSUM") as fps, \
                     tc.tile_pool(name=f"ffnps2{li}", bufs=4, space="PSUM") as fps2:
                    gl_sb = fp.tile([128, 32, TPC], dt.float32r, name="gl_sb")
                    for ms in range(32):
                        w1s = fwp.tile([128, 8, 128], dt.float32r, name="w1s", tag="w1s")
                        nc.sync.dma_start(w1s[:], _col3(fw1_d[li], ms * 128, (ms + 1) * 128))
                        psg = fps.tile([128, TPC], dt.float32, name="ps_g", tag="psg")
                        for kt in range(8):
                            nc.tensor.matmul(psg[:], w1s[:, kt, :], x2T_t[:, kt, :],
                                             start=(kt == 0), stop=(kt == 7))
                        nc.scalar.activation(gl_sb[:, ms, :], psg[:], AF.Gelu)
                    for mq in range(4):
                        w2q = fwp.tile([128, 32, 256], dt.float32r, name="w2q", tag="w2q", bufs=2)
                        nc.sync.dma_start(w2q[:], _col3(fw2_d[li], mq * 256, (mq + 1) * 256))
                        for j in range(NQT):
                            ps_f = fps2.tile([128, 256], dt.float32, name="ps_f", tag="psf")
                            for kt in range(32):
                                nc.tensor.matmul(ps_f[:], gl_sb[:, kt, j * 128:(j + 1) * 128],
                                                 w2q[:, kt, :], start=(kt == 0), stop=(kt == 31))
                            nc.vector.tensor_tensor(h_t[:, j, mq * 256:(mq + 1) * 256],
                                                    h_t[:, j, mq * 256:(mq + 1) * 256],
                                                    ps_f[:], ALU.add)

        # ---------------- logits: h all-gather + vocab-sharded matmul ----------------
        with tc.tile_pool(name="lg", bufs=1) as gp, \
             tc.tile_pool(name="lgw", bufs=2) as gwp, \
             tc.tile_pool(name="lgps", bufs=2, space="PSUM") as gps, \
             tc.tile_pool(name="lgps2", bufs=6, space="PSUM") as gps2:
            hTf = gp.tile([128, 8, TPC], dt.float32r, name="hTf")
            for kt in range(8):
                for j in range(NQT):
                    tp = gps.tile([128, 128], dt.float32, name="tp_hf", tag="tp")
                    nc.tensor.transpose(tp[:], h_t[:, j, kt * 128:(kt + 1) * 128], idn_t[:])
                    nc.vector.tensor_copy(hTf[:, kt, j * 128:(j + 1) * 128], tp[:])
            nc.sync.dma_start(hag_in[:], hTf[:].rearrange("p a b -> p (a b)"))
            nc.gpsimd.collective_compute("AllGather", ALU.bypass, replica_groups=GROUPS,
                                         ins=[hag_in[:].opt()], outs=[hag_out[:].opt()])
            hT_full = gp.tile([128, 8, S], dt.float32r, name="hT_full")
            for r in range(4):
                nc.scalar.dma_start(
                    hT_full[:, :, r * TPC:(r + 1) * TPC],
                    hag_out[r:r + 1, :, :].rearrange("r p (a b) -> (r p) a b", a=8))
            ntiles = [(n * 512, 512) for n in range(VS // 512)]
            if VS % 512:
                ntiles.append((VS - VS % 512, VS % 512))
            for (noff, nsz) in ntiles:
                wf = gwp.tile([128, 8, 512], dt.float32r, name="ow_f", tag="ow", bufs=2)
                nc.sync.dma_start(wf[:, :, :nsz],
                                  outw_d.rearrange("(kt kp) n -> kp kt n", kp=128)[:, :, noff:noff + nsz])
                for tt in range(S // 128):
                    ps = gps2.tile([128, 512], dt.float32, name="ps_lg", tag="ps")
                    for kt in range(8):
                        nc.tensor.matmul(ps[:, :nsz], hT_full[:, kt, tt * 128:(tt + 1) * 128],
                                         wf[:, kt, :nsz], start=(kt == 0), stop=(kt == 7))
                    ot = gp.tile([128, 512], dt.float32, name="ot", tag="ot", bufs=6)
                    nc.vector.tensor_copy(ot[:, :nsz], ps[:, :nsz])
                    nc.scalar.dma_start(logits_d[tt * 128:(tt + 1) * 128, noff:noff + nsz],
                                        ot[:, :nsz])

    nc.compile()
    return nc


def _prep_inputs(inputs):
    f32 = lambda x: np.ascontiguousarray(np.asarray(x, dtype=np.float32))
    ids = np.asarray(inputs["input_ids"]).astype(np.int32)
    common = {
        "tok_emb": f32(inputs["tok_emb"]), "in_w": f32(inputs["in_w"]),
        "qe_w1": f32(inputs["qe_w1"]), "qe_w2": f32(inputs["qe_w2"]),
        "qe_w1r": f32(inputs["qe_w1"]), "qe_w2r": f32(inputs["qe_w2"]),
        "ch_w1": f32(inputs["ch_w1"]), "ch_w2": f32(inputs["ch_w2"]),
        "idn": np.eye(128, dtype=np.float32),
        "ones": np.ones((128, 128), dtype=np.float32),
        "c_mean": np.full((128, 1), 1.0 / CS, dtype=np.float32),
        "rank_init": np.ascontiguousarray(
            np.broadcast_to(NC - 1 - np.arange(NC, dtype=np.float32), (128, NC))),
    }
    for i in range(L):
        for nm in ["q_w", "k_w", "v_w", "o_w", "f_w1", "f_w2"]:
            common[f"l{i}_{nm}"] = f32(np.asarray(inputs[nm])[i])
    pos = f32(inputs["pos_emb"])
    outw = f32(inputs["out_w"])
    in_maps = []
    for c in range(8):
        b, q = c // 4, c % 4
        off = q * TPC
        m = dict(common)
        m["ids_col"] = np.ascontiguousarray(ids[b, off:off + TPC].reshape(NQT, 128).T)
        m["pos"] = np.ascontiguousarray(pos[off:off + TPC])
        m["out_w_sl"] = np.ascontiguousarray(outw[:, q * VS:(q + 1) * VS])
        in_maps.append(m)
    return in_maps


def kernel(**inputs) -> np.ndarray:
    # biases / LN affine params are zero / one for this model; the kernel
    # implements that fast path (verified here).
    for k in ["in_b", "ch_b1", "ch_b2", "qe_b1", "qe_b2", "q_b", "k_b", "v_b",
              "o_b", "f_b1", "f_b2", "ln1_b", "ln2_b", "out_b"]:
        assert not np.any(np.asarray(inputs[k])), f"nonzero bias {k} unsupported"
    for k in ["ln1_g", "ln2_g"]:
        assert np.all(np.asarray(inputs[k]) == 1.0), f"non-unit {k} unsupported"

    if "nc" not in _CACHE:
        _CACHE["nc"] = _build()
    nc = _CACHE["nc"]
    in_maps = _prep_inputs(inputs)
    res = run_bass_kernel_spmd(nc, in_maps, list(range(8)))
    out = np.empty((B, S, V), dtype=np.float32)
    for c in range(8):
        b, q = c // 4, c % 4
        out[b, :, q * VS:(q + 1) * VS] = res.results[c]["logits"]
    return out


# revision 42
# speedup vs baseline: 1.7116x; 1.0226x over previous
"""GCA model (retrieval_knn) Trainium2 kernel: 8 NeuronCores, token-sharded.

Sharding: core c -> (batch b=c//4, quarter q=c%4): 512 contiguous tokens.
KV and chunk encodings all-gathered within each batch's 4-core group.
Logits are vocab-sharded: final h is all-gathered in the group and each
core computes its batch's full 2048 tokens x an 8000-column vocab slice.

Precision: fp32 matmuls on the top-k-selection path (embeddings/in_w,
chunk MLP, qe MLPs, scores); float32r (full-rate) everywhere else.
Attention computes scores^T [keys, queries] directly (no weight
transposes); the chunk mask is applied as a rank-1 matmul accumulate and
softmax normalization is folded into the PSUM->SBUF copy.
"""
import numpy as np
from contextlib import ExitStack

import concourse.bass as bass
import concourse.tile as tile
import concourse.mybir as mybir
from concourse import bacc
from concourse.bass_utils import run_bass_kernel_spmd

dt = mybir.dt
AF = mybir.ActivationFunctionType
ALU = mybir.AluOpType

B, S, E, H, NH, L, V = 2, 2048, 1024, 1024, 8, 2, 32000
CS, K = 128, 8
HD = H // NH
SCALE = HD ** -0.5
TPC = 512            # tokens per core
NQT = TPC // 128     # 4 q-tiles per core
NC = S // CS         # 16 chunks
NKT = S // 128       # 16 key tiles
VS = V // 4          # vocab slice per core
GROUPS = [[0, 1, 2, 3], [4, 5, 6, 7]]

_CACHE = {}


def _col3(wap, msl0, msl1):
    """DRAM [K, M] -> [128, K//128, msl1-msl0] lhsT-tile view."""
    return wap.rearrange("(kt kp) n -> kp kt n", kp=128)[:, :, msl0:msl1]


def _emit_ln(nc, pool, h_ap, out_ap):
    """LayerNorm of [128, H] h_ap -> out_ap (gamma=1, beta=0 fast path)."""
    sq = pool.tile([128, H], dt.float32, name="ln_sq", tag="ln_sq")
    ss = pool.tile([128, 1], dt.float32, name="ln_ss", tag="ln_ss")
    nc.scalar.activation(sq[:], h_ap, AF.Square, accum_out=ss[:])
    s = pool.tile([128, 1], dt.float32, name="ln_s", tag="ln_s")
    nc.vector.reduce_sum(s[:], h_ap, axis=mybir.AxisListType.X)
    mean = pool.tile([128, 1], dt.float32, name="ln_m", tag="ln_m")
    nc.vector.tensor_scalar(mean[:], s[:], 1.0 / H, None, ALU.mult)
    msq = pool.tile([128, 1], dt.float32, name="ln_msq", tag="ln_msq")
    nc.vector.tensor_tensor(msq[:], mean[:], mean[:], ALU.mult)
    var = pool.tile([128, 1], dt.float32, name="ln_v", tag="ln_v")
    nc.vector.tensor_scalar(var[:], ss[:], 1.0 / H, 1e-5, ALU.mult, ALU.add)
    nc.vector.tensor_sub(var[:], var[:], msq[:])
    sd = pool.tile([128, 1], dt.float32, name="ln_sd", tag="ln_sd")
    nc.scalar.activation(sd[:], var[:], AF.Sqrt)
    r = pool.tile([128, 1], dt.float32, name="ln_r", tag="ln_r")
    nc.vector.reciprocal(r[:], sd[:])
    # one Newton step: r = r*(1.5 - 0.5*var*r*r)
    r2 = pool.tile([128, 1], dt.float32, name="ln_r2", tag="ln_r2")
    nc.vector.tensor_tensor(r2[:], r[:], r[:], ALU.mult)
    nc.vector.tensor_tensor(r2[:], r2[:], var[:], ALU.mult)
    nc.vector.tensor_scalar(r2[:], r2[:], -0.5, 1.5, ALU.mult, ALU.add)
    nc.vector.tensor_tensor(r[:], r[:], r2[:], ALU.mult)
    nc.vector.tensor_scalar(out_ap, h_ap, mean[:], r[:], ALU.subtract, ALU.mult)


def _build():
    nc = bacc.Bacc("TRN2", target_bir_lowering=False, debug=False, num_devices=8)

    def din(name, shape, dtype=dt.float32):
        return nc.dram_tensor(name, shape, dtype, kind="ExternalInput").ap()

    ids_d = din("ids_col", [128, NQT], dt.int32)
    pos_d = din("pos", [TPC, E])
    temb_d = din("tok_emb", [V, E])
    inw_d = din("in_w", [E, H])
    qew1_d = din("qe_w1", [H, H // 2])
    qew2_d = din("qe_w2", [H // 2, H])
    qew1r_d = din("qe_w1r", [H, H // 2], dt.float32r)
    qew2r_d = din("qe_w2r", [H // 2, H], dt.float32r)
    chw1_d = din("ch_w1", [H, H // 2])
    chw2_d = din("ch_w2", [H // 2, H])
    qw_d = [din(f"l{i}_q_w", [H, H], dt.float32r) for i in range(L)]
    kw_d = [din(f"l{i}_k_w", [H, H], dt.float32r) for i in range(L)]
    vw_d = [din(f"l{i}_v_w", [H, H], dt.float32r) for i in range(L)]
    ow_d = [din(f"l{i}_o_w", [H, H], dt.float32r) for i in range(L)]
    fw1_d = [din(f"l{i}_f_w1", [H, 4 * H], dt.float32r) for i in range(L)]
    fw2_d = [din(f"l{i}_f_w2", [4 * H, H], dt.float32r) for i in range(L)]
    outw_d = din("out_w_sl", [H, VS], dt.float32r)
    idn_d = din("idn", [128, 128])
    ones_d = din("ones", [128, 128])
    cmean_d = din("c_mean", [128, 1])
    rkinit_d = din("rank_init", [128, NC])

    logits_d = nc.dram_tensor("logits", [S, VS], dt.float32, kind="ExternalOutput").ap()

    with ExitStack() as ctx:
        tc = ctx.enter_context(tile.TileContext(nc))
        P = ctx.enter_context(tc.tile_pool(name="persist", bufs=1))
        dramp = ctx.enter_context(tc.tile_pool(name="dramp", bufs=1, space="DRAM"))

        idn_t = P.tile([128, 128], dt.float32, name="idn_t")
        nc.sync.dma_start(idn_t[:], idn_d)
        ones_t = P.tile([128, 128], dt.float32, name="ones_t")
        nc.sync.dma_start(ones_t[:], ones_d)
        ones_r = P.tile([128, 128], dt.float32r, name="ones_r")
        nc.vector.tensor_copy(ones_r[:], ones_t[:])

        cmean_t = P.tile([128, 1], dt.float32, name="cmean_t")
        nc.sync.dma_start(cmean_t[:], cmean_d)
        rkinit_t = P.tile([128, NC], dt.float32, name="rkinit_t")
        nc.sync.dma_start(rkinit_t[:], rkinit_d)

        h_t = P.tile([128, NQT, H], dt.float32, name="h_t")          # residual [tok, H]
        ceT_t = P.tile([128, 8, NC], dt.float32, name="ceT_t")       # [hp, htile, chunk]
        ceT_r = P.tile([128, 8, NC], dt.float32r, name="ceT_r")
        maskb_t = P.tile([128, NQT, NC], dt.float32, name="maskb_t")

        # ---------------- embeddings + in_w ----------------
        with tc.tile_pool(name="emb", bufs=1) as embp, \
             tc.tile_pool(name="embps", bufs=1, space="PSUM") as embps:
            ids_t = embp.tile([128, NQT], dt.int32, name="ids_t")
            nc.sync.dma_start(ids_t[:], ids_d)
            emb_t = embp.tile([128, NQT, E], dt.float32, name="emb_t")
            for j in range(NQT):
                nc.gpsimd.indirect_dma_start(
                    out=emb_t[:, j, :], out_offset=None, in_=temb_d,
                    in_offset=bass.IndirectOffsetOnAxis(ap=ids_t[:, j:j + 1], axis=0))
                pos_t = embp.tile([128, E], dt.float32, name="pos_t", tag="pos", bufs=2)
                nc.sync.dma_start(pos_t[:], pos_d[j * 128:(j + 1) * 128, :])
                nc.vector.tensor_add(emb_t[:, j, :], emb_t[:, j, :], pos_t[:])
            embT_t = embp.tile([128, 8, TPC], dt.float32, name="embT_t")
            for kt in range(8):
                for j in range(NQT):
                    tp = embps.tile([128, 128], dt.float32, name="tp_e", tag="tp", bufs=3)
                    nc.tensor.transpose(tp[:], emb_t[:, j, kt * 128:(kt + 1) * 128], idn_t[:])
                    nc.vector.tensor_copy(embT_t[:, kt, j * 128:(j + 1) * 128], tp[:])
            inw_sb = embp.tile([128, 8, H], dt.float32, name="inw_sb")
            nc.sync.dma_start(inw_sb[:], inw_d.rearrange("(kt kp) n -> kp kt n", kp=128))
            for j in range(NQT):
                for nh in range(2):
                    ps = embps.tile([128, 512], dt.float32, name="ps_h0", tag="ps", bufs=4)
                    for kt in range(8):
                        nc.tensor.matmul(ps[:], embT_t[:, kt, j * 128:(j + 1) * 128],
                                         inw_sb[:, kt, nh * 512:(nh + 1) * 512],
                                         start=(kt == 0), stop=(kt == 7))
                    nc.vector.tensor_copy(h_t[:, j, nh * 512:(nh + 1) * 512], ps[:])

        # ---------------- chunk encodings (fp32) ----------------
        with tc.tile_pool(name="ch", bufs=1) as chp, \
             tc.tile_pool(name="chps", bufs=2, space="PSUM") as chps:
            avg_t = chp.tile([NQT, H], dt.float32, name="avg_t")
            for j in range(NQT):
                for nh in range(2):
                    ps = chps.tile([1, 512], dt.float32, name="ps_av", tag="psa")
                    nc.tensor.matmul(ps[:], cmean_t[:], h_t[:, j, nh * 512:(nh + 1) * 512],
                                     start=True, stop=True)
                    av1 = chp.tile([1, 512], dt.float32, name="av1", tag="av1", bufs=2)
                    nc.vector.tensor_copy(av1[:], ps[:])
                    nc.scalar.dma_start(avg_t[j:j + 1, nh * 512:(nh + 1) * 512], av1[:])
            avgT_t = chp.tile([128, 8, NQT], dt.float32, name="avgT_t")
            for kt in range(8):
                tp = chps.tile([128, NQT], dt.float32, name="tp_a", tag="tpa")
                nc.tensor.transpose(tp[:, :], avg_t[:, kt * 128:(kt + 1) * 128], idn_t[:NQT, :NQT])
                nc.vector.tensor_copy(avgT_t[:, kt, :], tp[:, :])
            hid_t = chp.tile([128, 4, NQT], dt.float32, name="hid_t")
            w1 = chp.tile([128, 8, 512], dt.float32, name="chw1_t")
            nc.sync.dma_start(w1[:], chw1_d.rearrange("(kt kp) n -> kp kt n", kp=128))
            for m in range(4):
                ps = chps.tile([128, NQT], dt.float32, name="ps_c1", tag="psc")
                for kt in range(8):
                    nc.tensor.matmul(ps[:], w1[:, kt, m * 128:(m + 1) * 128],
                                     avgT_t[:, kt, :], start=(kt == 0), stop=(kt == 7))
                nc.scalar.activation(hid_t[:, m, :], ps[:], AF.Relu)
            w2 = chp.tile([128, 4, 1024], dt.float32, name="chw2_t")
            nc.sync.dma_start(w2[:], chw2_d.rearrange("(kt kp) n -> kp kt n", kp=128))
            ce_loc = chp.tile([128, 8, NQT], dt.float32, name="ce_loc")
            for m in range(8):
                ps = chps.tile([128, NQT], dt.float32, name="ps_c2", tag="psc")
                for kt in range(4):
                    nc.tensor.matmul(ps[:], w2[:, kt, m * 128:(m + 1) * 128],
                                     hid_t[:, kt, :], start=(kt == 0), stop=(kt == 3))
                nc.vector.tensor_copy(ce_loc[:, m, :], ps[:])
            ce_in = dramp.tile([128, 8 * NQT], dt.float32, name="ce_in")
            ce_out = dramp.tile([4, 128, 8 * NQT], dt.float32, name="ce_out")
            nc.sync.dma_start(ce_in[:], ce_loc[:].rearrange("p a b -> p (a b)"))
            nc.gpsimd.collective_compute(
                "AllGather", ALU.bypass, replica_groups=GROUPS,
                ins=[ce_in[:].opt()], outs=[ce_out[:].opt()])
            for t in range(8):
                nc.sync.dma_start(
                    ceT_t[:, t, :].rearrange("p (r c) -> p r c", r=4),
                    ce_out[:, :, t * NQT:(t + 1) * NQT].rearrange("r p c -> p r c"))
            nc.vector.tensor_copy(ceT_r[:], ceT_t[:])

        kv_dram = []
        for i in range(L):
            k_io = [(dramp.tile([128, TPC], dt.float32r, name=f"kt_in{i}_{m}"),
                     dramp.tile([4, 128, TPC], dt.float32r, name=f"kt_out{i}_{m}"))
                    for m in range(NH)]
            v_io = [(dramp.tile([TPC, 512], dt.float32r, name=f"v_in{i}_{n}"),
                     dramp.tile([4, TPC, 512], dt.float32r, name=f"v_out{i}_{n}"))
                    for n in range(2)]
            kv_dram.append((k_io, v_io))
        hag_in = dramp.tile([128, 8 * TPC], dt.float32r, name="hag_in")
        hag_out = dramp.tile([4, 128, 8 * TPC], dt.float32r, name="hag_out")

        for li in range(L):
            with tc.tile_pool(name=f"layer{li}", bufs=1) as LP:
                qp_cm = tc.tile_pool(name=f"qkao{li}", bufs=1)
                QP = qp_cm.__enter__()
                x1T_t = LP.tile([128, 8, TPC], dt.float32r, name="x1T", tag="xT")
                qT_t = QP.tile([128, 8, TPC], dt.float32r, name="qT")
                aoT_t = QP.tile([128, 8, TPC], dt.float32r, name="aoT")
                mbT_t = QP.tile([16, NQT, 128], dt.float32r, name="mbT")
                mbb_t = QP.tile([128, NC, TPC], dt.float32, name="mbb")

                # ---- LN1 + x1T ----
                with tc.tile_pool(name=f"ln1_{li}", bufs=2) as lp, \
                     tc.tile_pool(name=f"ln1ps{li}", bufs=4, space="PSUM") as lps:
                    for j in range(NQT):
                        x1 = lp.tile([128, H], dt.float32, name="x1", tag="x1")
                        _emit_ln(nc, lp, h_t[:, j, :], x1)
                        for kt in range(8):
                            tp = lps.tile([128, 128], dt.float32, name="tp_x", tag="tp")
                            nc.tensor.transpose(tp[:], x1[:, kt * 128:(kt + 1) * 128], idn_t[:])
                            nc.vector.tensor_copy(x1T_t[:, kt, j * 128:(j + 1) * 128], tp[:])

                # ---- QKV projections + per-head/half KV all-gathers ----
                k_io, v_io = kv_dram[li]
                with tc.tile_pool(name=f"qkv{li}", bufs=1) as pp, \
                     tc.tile_pool(name=f"qkvps{li}", bufs=4, space="PSUM") as pps:
                    for m in range(8):
                        wk = pp.tile([128, 8, 128], dt.float32r, name="wk", tag="wk", bufs=2)
                        nc.sync.dma_start(wk[:], _col3(kw_d[li], m * 128, (m + 1) * 128))
                        ps2 = pps.tile([128, TPC], dt.float32, name="ps_kp", tag="ps")
                        for kt in range(8):
                            nc.tensor.matmul(ps2[:], wk[:, kt, :], x1T_t[:, kt, :],
                                             start=(kt == 0), stop=(kt == 7))
                        kslc = pp.tile([128, TPC], dt.float32r, name="kslc", tag="kslc", bufs=2)
                        nc.vector.tensor_copy(kslc[:], ps2[:])
                        nc.sync.dma_start(k_io[m][0][:], kslc[:])
                        nc.gpsimd.collective_compute(
                            "AllGather", ALU.bypass, replica_groups=GROUPS,
                            ins=[k_io[m][0][:].opt()], outs=[k_io[m][1][:].opt()])
                        wq = pp.tile([128, 8, 128], dt.float32r, name="wq", tag="wq", bufs=2)
                        nc.sync.dma_start(wq[:], _col3(qw_d[li], m * 128, (m + 1) * 128))
                        ps = pps.tile([128, TPC], dt.float32, name="ps_qp", tag="ps")
                        for kt in range(8):
                            nc.tensor.matmul(ps[:], wq[:, kt, :], x1T_t[:, kt, :],
                                             start=(kt == 0), stop=(kt == 7))
                        nc.vector.tensor_copy(qT_t[:, m, :], ps[:])
                    for nh2 in range(2):
                        v_in3 = v_io[nh2][0][:].rearrange("(a p) b -> p a b", p=128)
                        wv = pp.tile([128, 8, 512], dt.float32r, name="wv", tag="wv", bufs=2)
                        nc.sync.dma_start(wv[:], _col3(vw_d[li], nh2 * 512, (nh2 + 1) * 512))
                        for j in range(NQT):
                            ps3 = pps.tile([128, 512], dt.float32, name="ps_vp", tag="ps")
                            for kt in range(8):
                                nc.tensor.matmul(ps3[:], x1T_t[:, kt, j * 128:(j + 1) * 128],
                                                 wv[:, kt, :], start=(kt == 0), stop=(kt == 7))
                            vslc = pp.tile([128, 512], dt.float32r, name="vslc", tag="vslc", bufs=2)
                            nc.vector.tensor_copy(vslc[:], ps3[:])
                            nc.sync.dma_start(v_in3[:, j, :], vslc[:])
                        nc.gpsimd.collective_compute(
                            "AllGather", ALU.bypass, replica_groups=GROUPS,
                            ins=[v_io[nh2][0][:].opt()], outs=[v_io[nh2][1][:].opt()])

                # ---- hT + qe MLP + scores + top-k mask ----
                # selection path: fp32 on layer 0 (fragile margins), fp32r on
                # layer 1 (h already carries fp32r error; verified offline).
                qdt = dt.float32 if li == 0 else dt.float32r
                w1d = qew1_d if li == 0 else qew1r_d
                w2d = qew2_d if li == 0 else qew2r_d
                ceT_u = ceT_t if li == 0 else ceT_r
                with tc.tile_pool(name=f"qe{li}", bufs=1) as qp, \
                     tc.tile_pool(name=f"qeps{li}", bufs=1, space="PSUM") as qps:
                    hT_t = qp.tile([128, 8, TPC], qdt, name="hT_t")
                    for kt in range(8):
                        for j in range(NQT):
                            tp = qps.tile([128, 128], dt.float32, name="tp_h", tag="tp", bufs=2)
                            nc.tensor.transpose(tp[:], h_t[:, j, kt * 128:(kt + 1) * 128], idn_t[:])
                            nc.vector.tensor_copy(hT_t[:, kt, j * 128:(j + 1) * 128], tp[:])
                    qe1_t = qp.tile([128, 4, TPC], qdt, name="qe1_t")
                    for m in range(4):
                        w = qp.tile([128, 8, 128], qdt, name="qw1", tag="qw1", bufs=2)
                        nc.scalar.dma_start(w[:], _col3(w1d, m * 128, (m + 1) * 128))
                        ps = qps.tile([128, TPC], dt.float32, name="ps_q1", tag="ps", bufs=3)
                        for kt in range(8):
                            nc.tensor.matmul(ps[:], w[:, kt, :], hT_t[:, kt, :],
                                             start=(kt == 0), stop=(kt == 7))
                        nc.scalar.activation(qe1_t[:, m, :], ps[:], AF.Relu)
                    qeT_t = qp.tile([128, 8, TPC], qdt, name="qeT_t")
                    for m in range(8):
                        w = qp.tile([128, 4, 128], qdt, name="qw2", tag="qw2", bufs=2)
                        nc.scalar.dma_start(w[:], _col3(w2d, m * 128, (m + 1) * 128))
                        ps = qps.tile([128, TPC], dt.float32, name="ps_q2", tag="ps", bufs=3)
                        for kt in range(4):
                            nc.tensor.matmul(ps[:], w[:, kt, :], qe1_t[:, kt, :],
                                             start=(kt == 0), stop=(kt == 3))
                        nc.vector.tensor_copy(qeT_t[:, m, :], ps[:])
                    for j in range(NQT):
                        ps = qps.tile([128, NC], dt.float32, name="ps_sc", tag="pssc", bufs=2)
                        for kt in range(8):
                            nc.tensor.matmul(ps[:], qeT_t[:, kt, j * 128:(j + 1) * 128],
                                             ceT_u[:, kt, :], start=(kt == 0), stop=(kt == 7))
                        sc = qp.tile([128, NC], dt.float32, name="sc", tag="sc", bufs=2)
                        nc.vector.tensor_copy(sc[:], ps[:])
                        rank = qp.tile([128, NC], dt.float32, name="rank", tag="rank", bufs=2)
                        nc.vector.tensor_copy(rank[:], rkinit_t[:])
                        for d in range(1, NC):
                            ge = qp.tile([128, NC - d], dt.float32, name="ge", tag="ge", bufs=2)
                            nc.vector.tensor_tensor(ge[:], sc[:, :NC - d], sc[:, d:], ALU.is_ge)
                            nc.vector.tensor_add(rank[:, d:], rank[:, d:], ge[:])
                            nc.vector.tensor_sub(rank[:, :NC - d], rank[:, :NC - d], ge[:])
                        m01 = qp.tile([128, NC], dt.float32, name="m01", tag="m01", bufs=2)
                        nc.vector.tensor_scalar(m01[:], rank[:], 7.5, None, ALU.is_le)
                        nc.vector.tensor_scalar(maskb_t[:, j, :], m01[:], 1.0, 1e30,
                                                ALU.subtract, ALU.mult)
                    # mbT[c, j, ii] = maskb[token(j,ii), c] as [1,512] rank-1 rows
                    for j in range(NQT):
                        tpm = qps.tile([16, 128], dt.float32, name="tp_m", tag="tp", bufs=2)
                        nc.tensor.transpose(tpm[:], maskb_t[:, j, :], idn_t[:])
                        nc.vector.tensor_copy(mbT_t[:, j, :], tpm[:])
                    # broadcast mask rows to all partitions: mbb[p, c, i] = maskb[i, c]
                    for c in range(NC):
                        mb1 = qp.tile([1, TPC], dt.float32r, name="mb1", tag="mb1", bufs=2)
                        nc.sync.dma_start(mb1[:],
                                          mbT_t[c:c + 1, :, :].rearrange("c j i -> c (j i)"))
                        psb = qps.tile([128, TPC], dt.float32, name="ps_b", tag="ps", bufs=3)
                        nc.tensor.matmul(psb[:], ones_r[0:1, :], mb1[:], start=True, stop=True)
                        nc.vector.tensor_copy(mbb_t[:, c, :], psb[:])

                # ---- attention: scoresT = K^T-major, mask via rank-1, exp, AV ----
                with tc.tile_pool(name=f"att{li}", bufs=1) as ap, \
                     tc.tile_pool(name=f"attw{li}", bufs=2) as awp, \
                     tc.tile_pool(name=f"attqk{li}", bufs=2, space="PSUM") as aps_qk, \
                     tc.tile_pool(name=f"attrs{li}", bufs=1, space="PSUM") as aps_rs, \
                     tc.tile_pool(name=f"attao{li}", bufs=2, space="PSUM") as aps_ao, \
                     tc.tile_pool(name=f"attrb{li}", bufs=1, space="PSUM") as aps_rb:
                    for hh in range(NH):
                        kT_h = awp.tile([128, S], dt.float32r, name="kT_h", tag="kT_h", bufs=2)
                        for r in range(4):
                            nc.scalar.dma_start(
                                kT_h[:, r * TPC:(r + 1) * TPC],
                                k_io[hh][1][r:r + 1, :, :].rearrange("r p t -> (r p) t"))
                        v_h = awp.tile([128, NKT, HD], dt.float32r, name="v_h", tag="v_h", bufs=2)
                        for r in range(4):
                            nc.scalar.dma_start(
                                v_h[:, r * 4:(r + 1) * 4, :],
                                v_io[hh // 4][1][r:r + 1, :, :].rearrange(
                                    "r (a p) b -> p (r a) b", p=128)[:, :, (hh % 4) * HD:(hh % 4 + 1) * HD])
                        weT = ap.tile([128, NKT, TPC], dt.float32r, name="weT", tag="weT", bufs=2)
                        rs_ps = aps_rs.tile([1, TPC], dt.float32, name="rs_ps", tag="rs")
                        pao = aps_ao.tile([128, TPC], dt.float32, name="pao", tag="ao")
                        for cp in range(NKT // 2):
                            ps_s = aps_qk.tile([128, 2, TPC], dt.float32, name="ps_s", tag="qk")
                            for u in range(2):
                                nc.tensor.matmul(ps_s[:, u, :],
                                                 kT_h[:, (2 * cp + u) * 128:(2 * cp + u + 1) * 128],
                                                 qT_t[:, hh, :], start=True, stop=True)
                            nc.vector.tensor_tensor(ps_s[:], ps_s[:],
                                                    mbb_t[:, 2 * cp:2 * cp + 2, :], ALU.add)
                            nc.scalar.activation(weT[:, 2 * cp:2 * cp + 2, :], ps_s[:],
                                                 AF.Exp, scale=SCALE)
                            for u in range(2):
                                c = 2 * cp + u
                                nc.tensor.matmul(rs_ps[:], ones_r[:, 0:1], weT[:, c, :],
                                                 start=(c == 0), stop=(c == NKT - 1))
                                nc.tensor.matmul(pao[:], v_h[:, c, :], weT[:, c, :],
                                                 start=(c == 0), stop=(c == NKT - 1))
                        rinv_sb = ap.tile([1, TPC], dt.float32, name="rinv", tag="rinv", bufs=2)
                        nc.vector.reciprocal(rinv_sb[:], rs_ps[:])
                        ps_rb = aps_rb.tile([128, TPC], dt.float32, name="ps_rb", tag="rb")
                        nc.tensor.matmul(ps_rb[:], ones_t[0:1, :], rinv_sb[:],
                                         start=True, stop=True)
                        rinvb = ap.tile([128, TPC], dt.float32, name="rinvb", tag="rinvb", bufs=2)
                        nc.vector.tensor_copy(rinvb[:], ps_rb[:])
                        nc.vector.tensor_tensor(aoT_t[:, hh, :], pao[:], rinvb[:], ALU.mult)

                # ---- o-projection direct [tok, feat] + residual add ----
                with tc.tile_pool(name=f"opj{li}", bufs=1) as op, \
                     tc.tile_pool(name=f"opjps{li}", bufs=4, space="PSUM") as ops:
                    wo_sb = op.tile([128, 8, H], dt.float32r, name="wo_sb")
                    nc.sync.dma_start(wo_sb[:], _col3(ow_d[li], 0, H))
                    for j in range(NQT):
                        for mh in range(2):
                            ps_o = ops.tile([128, 512], dt.float32, name="ps_o", tag="ps")
                            for kt in range(8):
                                nc.tensor.matmul(ps_o[:], aoT_t[:, kt, j * 128:(j + 1) * 128],
                                                 wo_sb[:, kt, mh * 512:(mh + 1) * 512],
                                                 start=(kt == 0), stop=(kt == 7))
                            nc.vector.tensor_tensor(h_t[:, j, mh * 512:(mh + 1) * 512],
                                                    h_t[:, j, mh * 512:(mh + 1) * 512],
                                                    ps_o[:], ALU.add)
                qp_cm.__exit__(None, None, None)

                # ---- LN2 + x2T ----
                x2T_t = LP.tile([128, 8, TPC], dt.float32r, name="x2T", tag="xT")
                with tc.tile_pool(name=f"ln2_{li}", bufs=2) as lp2, \
                     tc.tile_pool(name=f"ln2ps{li}", bufs=4, space="PSUM") as lps2:
                    for j in range(NQT):
                        x2 = lp2.tile([128, H], dt.float32, name="x2", tag="x2")
                        _emit_ln(nc, lp2, h_t[:, j, :], x2)
                        for kt in range(8):
                            tp = lps2.tile([128, 128], dt.float32, name="tp_x2", tag="tp")
                            nc.tensor.transpose(tp[:], x2[:, kt * 128:(kt + 1) * 128], idn_t[:])
                            nc.vector.tensor_copy(x2T_t[:, kt, j * 128:(j + 1) * 128], tp[:])

                # ---- FFN: w1 -> gelu -> w2 direct [tok, feat] ----
                with tc.tile_pool(name=f"ffn{li}", bufs=1) as fp, \
                     tc.tile_pool(name=f"ffnw{li}", bufs=2) as fwp, \
                     tc.tile_pool(name=f"ffnps{li}", bufs=3, space="PSUM") as fps, \
                     tc.tile_pool(name=f"ffnps2{li}", bufs=4, space="PSUM") as fps2:
                    gl_sb = fp.tile([128, 32, TPC], dt.float32r, name="gl_sb")
                    for ms in range(32):
                        w1s = fwp.tile([128, 8, 128], dt.float32r, name="w1s", tag="w1s")
                        nc.sync.dma_start(w1s[:], _col3(fw1_d[li], ms * 128, (ms + 1) * 128))
                        psg = fps.tile([128, TPC], dt.float32, name="ps_g", tag="psg")
                        for kt in range(8):
                            nc.tensor.matmul(psg[:], w1s[:, kt, :], x2T_t[:, kt, :],
                                             start=(kt == 0), stop=(kt == 7))
                        nc.scalar.activation(gl_sb[:, ms, :], psg[:], AF.Gelu)
                    for mq in range(4):
                        w2q = fwp.tile([128, 32, 256], dt.float32r, name="w2q", tag="w2q", bufs=2)
                        nc.sync.dma_start(w2q[:], _col3(fw2_d[li], mq * 256, (mq + 1) * 256))
                        for j in range(NQT):
                            ps_f = fps2.tile([128, 256], dt.float32, name="ps_f", tag="psf")
                            for kt in range(32):
                                nc.tensor.matmul(ps_f[:], gl_sb[:, kt, j * 128:(j + 1) * 128],
                                                 w2q[:, kt, :], start=(kt == 0), stop=(kt == 31))
                            nc.vector.tensor_tensor(h_t[:, j, mq * 256:(mq + 1) * 256],
                                                    h_t[:, j, mq * 256:(mq + 1) * 256],
                                                    ps_f[:], ALU.add)

        # ---------------- logits: h all-gather + vocab-sharded matmul ----------------
        with tc.tile_pool(name="lg", bufs=1) as gp, \
             tc.tile_pool(name="lgw", bufs=2) as gwp, \
             tc.tile_pool(name="lgps", bufs=2, space="PSUM") as gps, \
             tc.tile_pool(name="lgps2", bufs=6, space="PSUM") as gps2:
            hTf = gp.tile([128, 8, TPC], dt.float32r, name="hTf")
            for kt in range(8):
                for j in range(NQT):
                    tp = gps.tile([128, 128], dt.float32, name="tp_hf", tag="tp")
                    nc.tensor.transpose(tp[:], h_t[:, j, kt * 128:(kt + 1) * 128], idn_t[:])
                    nc.vector.tensor_copy(hTf[:, kt, j * 128:(j + 1) * 128], tp[:])
            nc.sync.dma_start(hag_in[:], hTf[:].rearrange("p a b -> p (a b)"))
            nc.gpsimd.collective_compute("AllGather", ALU.bypass, replica_groups=GROUPS,
                                         ins=[hag_in[:].opt()], outs=[hag_out[:].opt()])
            hT_full = gp.tile([128, 8, S], dt.float32r, name="hT_full")
            for r in range(4):
                nc.scalar.dma_start(
                    hT_full[:, :, r * TPC:(r + 1) * TPC],
                    hag_out[r:r + 1, :, :].rearrange("r p (a b) -> (r p) a b", a=8))
            ntiles = [(n * 512, 512) for n in range(VS // 512)]
            if VS % 512:
                ntiles.append((VS - VS % 512, VS % 512))
            for (noff, nsz) in ntiles:
                wf = gwp.tile([128, 8, 512], dt.float32r, name="ow_f", tag="ow", bufs=2)
                nc.sync.dma_start(wf[:, :, :nsz],
                                  outw_d.rearrange("(kt kp) n -> kp kt n", kp=128)[:, :, noff:noff + nsz])
                for tt in range(S // 128):
                    ps = gps2.tile([128, 512], dt.float32, name="ps_lg", tag="ps")
                    for kt in range(8):
                        nc.tensor.matmul(ps[:, :nsz], hT_full[:, kt, tt * 128:(tt + 1) * 128],
                                         wf[:, kt, :nsz], start=(kt == 0), stop=(kt == 7))
                    ot = gp.tile([128, 512], dt.float32, name="ot", tag="ot", bufs=6)
                    nc.vector.tensor_copy(ot[:, :nsz], ps[:, :nsz])
                    nc.scalar.dma_start(logits_d[tt * 128:(tt + 1) * 128, noff:noff + nsz],
                                        ot[:, :nsz])

    nc.compile()
    return nc


def _prep_inputs(inputs):
    f32 = lambda x: np.ascontiguousarray(np.asarray(x, dtype=np.float32))
    ids = np.asarray(inputs["input_ids"]).astype(np.int32)
    common = {
        "tok_emb": f32(inputs["tok_emb"]), "in_w": f32(inputs["in_w"]),
        "qe_w1": f32(inputs["qe_w1"]), "qe_w2": f32(inputs["qe_w2"]),
        "qe_w1r": f32(inputs["qe_w1"]), "qe_w2r": f32(inputs["qe_w2"]),
        "ch_w1": f32(inputs["ch_w1"]), "ch_w2": f32(inputs["ch_w2"]),
        "idn": np.eye(128, dtype=np.float32),
        "ones": np.ones((128, 128), dtype=np.float32),
        "c_mean": np.full((128, 1), 1.0 / CS, dtype=np.float32),
        "rank_init": np.ascontiguousarray(
            np.broadcast_to(NC - 1 - np.arange(NC, dtype=np.float32), (128, NC))),
    }
    for i in range(L):
        for nm in ["q_w", "k_w", "v_w", "o_w", "f_w1", "f_w2"]:
            common[f"l{i}_{nm}"] = f32(np.asarray(inputs[nm])[i])
    pos = f32(inputs["pos_emb"])
    outw = f32(inputs["out_w"])
    in_maps = []
    for c in range(8):
        b, q = c // 4, c % 4
        off = q * TPC
        m = dict(common)
        m["ids_col"] = np.ascontiguousarray(ids[b, off:off + TPC].reshape(NQT, 128).T)
        m["pos"] = np.ascontiguousarray(pos[off:off + TPC])
        m["out_w_sl"] = np.ascontiguousarray(outw[:, q * VS:(q + 1) * VS])
        in_maps.append(m)
    return in_maps


def kernel(**inputs) -> np.ndarray:
    # biases / LN affine params are zero / one for this model; the kernel
    # implements that fast path (verified here).
    for k in ["in_b", "ch_b1", "ch_b2", "qe_b1", "qe_b2", "q_b", "k_b", "v_b",
              "o_b", "f_b1", "f_b2", "ln1_b", "ln2_b", "out_b"]:
        assert not np.any(np.asarray(inputs[k])), f"nonzero bias {k} unsupported"
    for k in ["ln1_g", "ln2_g"]:
        assert np.all(np.asarray(inputs[k]) == 1.0), f"non-unit {k} unsupported"

    if "nc" not in _CACHE:
        _CACHE["nc"] = _build()
    nc = _CACHE["nc"]
    in_maps = _prep_inputs(inputs)
    res = run_bass_kernel_spmd(nc, in_maps, list(range(8)))
    out = np.empty((B, S, V), dtype=np.float32)
    for c in range(8):
        b, q = c // 4, c % 4
        out[b, :, q * VS:(q + 1) * VS] = res.results[c]["logits"]
    return out


# revision 51
# speedup vs baseline: 1.8373x; 1.0734x over previous
"""GCA model (retrieval_knn) Trainium2 kernel: 8 NeuronCores, token-sharded.

Sharding: core c -> (batch b=c//4, quarter q=c%4): 512 contiguous tokens.
KV and chunk encodings all-gathered within each batch's 4-core group.
Logits are vocab-sharded: final h is all-gathered in the group and each
core computes its batch's full 2048 tokens x an 8000-column vocab slice.

Precision: fp32 matmuls on the top-k-selection path (embeddings/in_w,
chunk MLP, qe MLPs, scores); float32r (full-rate) everywhere else.
Attention computes scores^T [keys, queries] directly (no weight
transposes); the chunk mask is applied as a rank-1 matmul accumulate and
softmax normalization is folded into the PSUM->SBUF copy.
"""
import numpy as np
from contextlib import ExitStack

import concourse.bass as bass
import concourse.tile as tile
import concourse.mybir as mybir
from concourse import bacc
from concourse.bass_utils import run_bass_kernel_spmd

dt = mybir.dt
AF = mybir.ActivationFunctionType
ALU = mybir.AluOpType

B, S, E, H, NH, L, V = 2, 2048, 1024, 1024, 8, 2, 32000
CS, K = 128, 8
HD = H // NH
SCALE = HD ** -0.5
TPC = 512            # tokens per core
NQT = TPC // 128     # 4 q-tiles per core
NC = S // CS         # 16 chunks
NKT = S // 128       # 16 key tiles
VS = V // 4          # vocab slice per core
GROUPS = [[0, 1, 2, 3], [4, 5, 6, 7]]

_CACHE = {}


def _col3(wap, msl0, msl1):
    """DRAM [K, M] -> [128, K//128, msl1-msl0] lhsT-tile view."""
    return wap.rearrange("(kt kp) n -> kp kt n", kp=128)[:, :, msl0:msl1]


def _emit_ln(nc, pool, h_ap, out_ap):
    """LayerNorm of [128, H] h_ap -> out_ap (gamma=1, beta=0 fast path)."""
    sq = pool.tile([128, H], dt.float32, name="ln_sq", tag="ln_sq")
    ss = pool.tile([128, 1], dt.float32, name="ln_ss", tag="ln_ss")
    nc.scalar.activation(sq[:], h_ap, AF.Square, accum_out=ss[:])
    s = pool.tile([128, 1], dt.float32, name="ln_s", tag="ln_s")
    nc.vector.reduce_sum(s[:], h_ap, axis=mybir.AxisListType.X)
    mean = pool.tile([128, 1], dt.float32, name="ln_m", tag="ln_m")
    nc.vector.tensor_scalar(mean[:], s[:], 1.0 / H, None, ALU.mult)
    msq = pool.tile([128, 1], dt.float32, name="ln_msq", tag="ln_msq")
    nc.vector.tensor_tensor(msq[:], mean[:], mean[:], ALU.mult)
    var = pool.tile([128, 1], dt.float32, name="ln_v", tag="ln_v")
    nc.vector.tensor_scalar(var[:], ss[:], 1.0 / H, 1e-5, ALU.mult, ALU.add)
    nc.vector.tensor_sub(var[:], var[:], msq[:])
    sd = pool.tile([128, 1], dt.float32, name="ln_sd", tag="ln_sd")
    nc.scalar.activation(sd[:], var[:], AF.Sqrt)
    r = pool.tile([128, 1], dt.float32, name="ln_r", tag="ln_r")
    nc.vector.reciprocal(r[:], sd[:])
    # one Newton step: r = r*(1.5 - 0.5*var*r*r)
    r2 = pool.tile([128, 1], dt.float32, name="ln_r2", tag="ln_r2")
    nc.vector.tensor_tensor(r2[:], r[:], r[:], ALU.mult)
    nc.vector.tensor_tensor(r2[:], r2[:], var[:], ALU.mult)
    nc.vector.tensor_scalar(r2[:], r2[:], -0.5, 1.5, ALU.mult, ALU.add)
    nc.vector.tensor_tensor(r[:], r[:], r2[:], ALU.mult)
    nc.vector.tensor_scalar(out_ap, h_ap, mean[:], r[:], ALU.subtract, ALU.mult)


def _build():
    nc = bacc.Bacc("TRN2", target_bir_lowering=False, debug=False, num_devices=8)

    def din(name, shape, dtype=dt.float32):
        return nc.dram_tensor(name, shape, dtype, kind="ExternalInput").ap()

    ids_d = din("ids_col", [128, NQT], dt.int32)
    pos_d = din("pos", [TPC, E])
    temb_d = din("tok_emb", [V, E])
    inw_d = din("in_w", [E, H])
    qew1_d = din("qe_w1", [H, H // 2])
    qew2_d = din("qe_w2", [H // 2, H])
    qew1r_d = din("qe_w1r", [H, H // 2], dt.float32r)
    qew2r_d = din("qe_w2r", [H // 2, H], dt.float32r)
    chw1_d = din("ch_w1", [H, H // 2])
    chw2_d = din("ch_w2", [H // 2, H])
    qw_d = [din(f"l{i}_q_w", [H, H], dt.float32r) for i in range(L)]
    kw_d = [din(f"l{i}_k_w", [H, H], dt.float32r) for i in range(L)]
    vw_d = [din(f"l{i}_v_w", [H, H], dt.float32r) for i in range(L)]
    ow_d = [din(f"l{i}_o_w", [H, H], dt.float32r) for i in range(L)]
    fw1_d = [din(f"l{i}_f_w1", [H, 4 * H], dt.float32r) for i in range(L)]
    fw2_d = [din(f"l{i}_f_w2", [4 * H, H], dt.float32r) for i in range(L)]
    outw_d = din("out_w_sl", [H, VS], dt.float32r)
    idn_d = din("idn", [128, 128])
    ones_d = din("ones", [128, 128])
    cmean_d = din("c_mean", [128, 1])
    rkinit_d = din("rank_init", [128, NC])

    logits_d = nc.dram_tensor("logits", [S, VS], dt.float32, kind="ExternalOutput").ap()

    with ExitStack() as ctx:
        tc = ctx.enter_context(tile.TileContext(nc))
        P = ctx.enter_context(tc.tile_pool(name="persist", bufs=1))
        dramp = ctx.enter_context(tc.tile_pool(name="dramp", bufs=1, space="DRAM"))

        idn_t = P.tile([128, 128], dt.float32, name="idn_t")
        nc.sync.dma_start(idn_t[:], idn_d)
        ones_t = P.tile([128, 128], dt.float32, name="ones_t")
        nc.sync.dma_start(ones_t[:], ones_d)
        ones_r = P.tile([128, 128], dt.float32r, name="ones_r")
        nc.vector.tensor_copy(ones_r[:], ones_t[:])
        ones_b = P.tile([128, 128], dt.bfloat16, name="ones_b")
        nc.vector.tensor_copy(ones_b[:], ones_t[:])

        cmean_t = P.tile([128, 1], dt.float32, name="cmean_t")
        nc.sync.dma_start(cmean_t[:], cmean_d)
        rkinit_t = P.tile([128, NC], dt.float32, name="rkinit_t")
        nc.sync.dma_start(rkinit_t[:], rkinit_d)

        h_t = P.tile([128, NQT, H], dt.float32, name="h_t")          # residual [tok, H]
        ceT_t = P.tile([128, 8, NC], dt.float32, name="ceT_t")       # [hp, htile, chunk]
        ceT_r = P.tile([128, 8, NC], dt.float32r, name="ceT_r")
        maskb_t = P.tile([128, NQT, NC], dt.float32, name="maskb_t")

        # ---------------- embeddings + in_w ----------------
        with tc.tile_pool(name="emb", bufs=1) as embp, \
             tc.tile_pool(name="embps", bufs=1, space="PSUM") as embps:
            ids_t = embp.tile([128, NQT], dt.int32, name="ids_t")
            nc.sync.dma_start(ids_t[:], ids_d)
            emb_t = embp.tile([128, NQT, E], dt.float32, name="emb_t")
            for j in range(NQT):
                nc.gpsimd.indirect_dma_start(
                    out=emb_t[:, j, :], out_offset=None, in_=temb_d,
                    in_offset=bass.IndirectOffsetOnAxis(ap=ids_t[:, j:j + 1], axis=0))
                pos_t = embp.tile([128, E], dt.float32, name="pos_t", tag="pos", bufs=2)
                nc.sync.dma_start(pos_t[:], pos_d[j * 128:(j + 1) * 128, :])
                nc.vector.tensor_add(emb_t[:, j, :], emb_t[:, j, :], pos_t[:])
            embT_t = embp.tile([128, 8, TPC], dt.float32, name="embT_t")
            for kt in range(8):
                for j in range(NQT):
                    tp = embps.tile([128, 128], dt.float32, name="tp_e", tag="tp", bufs=3)
                    nc.tensor.transpose(tp[:], emb_t[:, j, kt * 128:(kt + 1) * 128], idn_t[:])
                    nc.vector.tensor_copy(embT_t[:, kt, j * 128:(j + 1) * 128], tp[:])
            inw_sb = embp.tile([128, 8, H], dt.float32, name="inw_sb")
            nc.sync.dma_start(inw_sb[:], inw_d.rearrange("(kt kp) n -> kp kt n", kp=128))
            for j in range(NQT):
                for nh in range(2):
                    ps = embps.tile([128, 512], dt.float32, name="ps_h0", tag="ps", bufs=4)
                    for kt in range(8):
                        nc.tensor.matmul(ps[:], embT_t[:, kt, j * 128:(j + 1) * 128],
                                         inw_sb[:, kt, nh * 512:(nh + 1) * 512],
                                         start=(kt == 0), stop=(kt == 7))
                    nc.vector.tensor_copy(h_t[:, j, nh * 512:(nh + 1) * 512], ps[:])

        # ---------------- chunk encodings (fp32) ----------------
        with tc.tile_pool(name="ch", bufs=1) as chp, \
             tc.tile_pool(name="chps", bufs=2, space="PSUM") as chps:
            avg_t = chp.tile([NQT, H], dt.float32, name="avg_t")
            for j in range(NQT):
                for nh in range(2):
                    ps = chps.tile([1, 512], dt.float32, name="ps_av", tag="psa")
                    nc.tensor.matmul(ps[:], cmean_t[:], h_t[:, j, nh * 512:(nh + 1) * 512],
                                     start=True, stop=True)
                    av1 = chp.tile([1, 512], dt.float32, name="av1", tag="av1", bufs=2)
                    nc.vector.tensor_copy(av1[:], ps[:])
                    nc.scalar.dma_start(avg_t[j:j + 1, nh * 512:(nh + 1) * 512], av1[:])
            avgT_t = chp.tile([128, 8, NQT], dt.float32, name="avgT_t")
            for kt in range(8):
                tp = chps.tile([128, NQT], dt.float32, name="tp_a", tag="tpa")
                nc.tensor.transpose(tp[:, :], avg_t[:, kt * 128:(kt + 1) * 128], idn_t[:NQT, :NQT])
                nc.vector.tensor_copy(avgT_t[:, kt, :], tp[:, :])
            hid_t = chp.tile([128, 4, NQT], dt.float32, name="hid_t")
            w1 = chp.tile([128, 8, 512], dt.float32, name="chw1_t")
            nc.sync.dma_start(w1[:], chw1_d.rearrange("(kt kp) n -> kp kt n", kp=128))
            for m in range(4):
                ps = chps.tile([128, NQT], dt.float32, name="ps_c1", tag="psc")
                for kt in range(8):
                    nc.tensor.matmul(ps[:], w1[:, kt, m * 128:(m + 1) * 128],
                                     avgT_t[:, kt, :], start=(kt == 0), stop=(kt == 7))
                nc.scalar.activation(hid_t[:, m, :], ps[:], AF.Relu)
            w2 = chp.tile([128, 4, 1024], dt.float32, name="chw2_t")
            nc.sync.dma_start(w2[:], chw2_d.rearrange("(kt kp) n -> kp kt n", kp=128))
            ce_loc = chp.tile([128, 8, NQT], dt.float32, name="ce_loc")
            for m in range(8):
                ps = chps.tile([128, NQT], dt.float32, name="ps_c2", tag="psc")
                for kt in range(4):
                    nc.tensor.matmul(ps[:], w2[:, kt, m * 128:(m + 1) * 128],
                                     hid_t[:, kt, :], start=(kt == 0), stop=(kt == 3))
                nc.vector.tensor_copy(ce_loc[:, m, :], ps[:])
            ce_in = dramp.tile([128, 8 * NQT], dt.float32, name="ce_in")
            ce_out = dramp.tile([4, 128, 8 * NQT], dt.float32, name="ce_out")
            nc.sync.dma_start(ce_in[:], ce_loc[:].rearrange("p a b -> p (a b)"))
            nc.gpsimd.collective_compute(
                "AllGather", ALU.bypass, replica_groups=GROUPS,
                ins=[ce_in[:].opt()], outs=[ce_out[:].opt()])
            for t in range(8):
                nc.sync.dma_start(
                    ceT_t[:, t, :].rearrange("p (r c) -> p r c", r=4),
                    ce_out[:, :, t * NQT:(t + 1) * NQT].rearrange("r p c -> p r c"))
            nc.vector.tensor_copy(ceT_r[:], ceT_t[:])

        kv_dram = []
        for i in range(L):
            k_io = [(dramp.tile([128, TPC], dt.bfloat16, name=f"kt_in{i}_{m}"),
                     dramp.tile([4, 128, TPC], dt.bfloat16, name=f"kt_out{i}_{m}"))
                    for m in range(NH)]
            v_io = [(dramp.tile([TPC, 512], dt.bfloat16, name=f"v_in{i}_{n}"),
                     dramp.tile([4, TPC, 512], dt.bfloat16, name=f"v_out{i}_{n}"))
                    for n in range(2)]
            kv_dram.append((k_io, v_io))
        hag_in = dramp.tile([128, 8 * TPC], dt.float32r, name="hag_in")
        hag_out = dramp.tile([4, 128, 8 * TPC], dt.float32r, name="hag_out")

        for li in range(L):
            with tc.tile_pool(name=f"layer{li}", bufs=1) as LP:
                qp_cm = tc.tile_pool(name=f"qkao{li}", bufs=1)
                QP = qp_cm.__enter__()
                x1T_t = LP.tile([128, 8, TPC], dt.float32r, name="x1T", tag="xT")
                qT_t = QP.tile([128, 8, TPC], dt.bfloat16, name="qT")
                aoT_t = QP.tile([128, 8, TPC], dt.float32r, name="aoT")
                mbT_t = QP.tile([16, NQT, 128], dt.float32r, name="mbT")
                mbb_t = QP.tile([128, NC, TPC], dt.float32, name="mbb")

                # ---- LN1 + x1T ----
                with tc.tile_pool(name=f"ln1_{li}", bufs=2) as lp, \
                     tc.tile_pool(name=f"ln1ps{li}", bufs=4, space="PSUM") as lps:
                    for j in range(NQT):
                        x1 = lp.tile([128, H], dt.float32, name="x1", tag="x1")
                        _emit_ln(nc, lp, h_t[:, j, :], x1)
                        for kt in range(8):
                            tp = lps.tile([128, 128], dt.float32, name="tp_x", tag="tp")
                            nc.tensor.transpose(tp[:], x1[:, kt * 128:(kt + 1) * 128], idn_t[:])
                            nc.vector.tensor_copy(x1T_t[:, kt, j * 128:(j + 1) * 128], tp[:])

                # ---- QKV projections + per-head/half KV all-gathers ----
                k_io, v_io = kv_dram[li]
                with tc.tile_pool(name=f"qkv{li}", bufs=1) as pp, \
                     tc.tile_pool(name=f"qkvps{li}", bufs=4, space="PSUM") as pps:
                    for m in range(8):
                        wk = pp.tile([128, 8, 128], dt.float32r, name="wk", tag="wk", bufs=2)
                        nc.sync.dma_start(wk[:], _col3(kw_d[li], m * 128, (m + 1) * 128))
                        ps2 = pps.tile([128, TPC], dt.float32, name="ps_kp", tag="ps")
                        for kt in range(8):
                            nc.tensor.matmul(ps2[:], wk[:, kt, :], x1T_t[:, kt, :],
                                             start=(kt == 0), stop=(kt == 7))
                        kslc = pp.tile([128, TPC], dt.bfloat16, name="kslc", tag="kslc", bufs=2)
                        nc.vector.tensor_copy(kslc[:], ps2[:])
                        nc.sync.dma_start(k_io[m][0][:], kslc[:])
                        nc.gpsimd.collective_compute(
                            "AllGather", ALU.bypass, replica_groups=GROUPS,
                            ins=[k_io[m][0][:].opt()], outs=[k_io[m][1][:].opt()])
                        wq = pp.tile([128, 8, 128], dt.float32r, name="wq", tag="wq", bufs=2)
                        nc.sync.dma_start(wq[:], _col3(qw_d[li], m * 128, (m + 1) * 128))
                        ps = pps.tile([128, TPC], dt.float32, name="ps_qp", tag="ps")
                        for kt in range(8):
                            nc.tensor.matmul(ps[:], wq[:, kt, :], x1T_t[:, kt, :],
                                             start=(kt == 0), stop=(kt == 7))
                        nc.vector.tensor_copy(qT_t[:, m, :], ps[:])
                    for nh2 in range(2):
                        v_in3 = v_io[nh2][0][:].rearrange("(a p) b -> p a b", p=128)
                        wv = pp.tile([128, 8, 512], dt.float32r, name="wv", tag="wv", bufs=2)
                        nc.sync.dma_start(wv[:], _col3(vw_d[li], nh2 * 512, (nh2 + 1) * 512))
                        for j in range(NQT):
                            ps3 = pps.tile([128, 512], dt.float32, name="ps_vp", tag="ps")
                            for kt in range(8):
                                nc.tensor.matmul(ps3[:], x1T_t[:, kt, j * 128:(j + 1) * 128],
                                                 wv[:, kt, :], start=(kt == 0), stop=(kt == 7))
                            vslc = pp.tile([128, 512], dt.bfloat16, name="vslc", tag="vslc", bufs=2)
                            nc.vector.tensor_copy(vslc[:], ps3[:])
                            nc.sync.dma_start(v_in3[:, j, :], vslc[:])
                        nc.gpsimd.collective_compute(
                            "AllGather", ALU.bypass, replica_groups=GROUPS,
                            ins=[v_io[nh2][0][:].opt()], outs=[v_io[nh2][1][:].opt()])

                # ---- hT + qe MLP + scores + top-k mask ----
                # selection path: fp32 on layer 0 (fragile margins), fp32r on
                # layer 1 (h already carries fp32r error; verified offline).
                qdt = dt.float32 if li == 0 else dt.float32r
                w1d = qew1_d if li == 0 else qew1r_d
                w2d = qew2_d if li == 0 else qew2r_d
                ceT_u = ceT_t if li == 0 else ceT_r
                with tc.tile_pool(name=f"qe{li}", bufs=1) as qp, \
                     tc.tile_pool(name=f"qeps{li}", bufs=1, space="PSUM") as qps:
                    hT_t = qp.tile([128, 8, TPC], qdt, name="hT_t")
                    for kt in range(8):
                        for j in range(NQT):
                            tp = qps.tile([128, 128], dt.float32, name="tp_h", tag="tp", bufs=2)
                            nc.tensor.transpose(tp[:], h_t[:, j, kt * 128:(kt + 1) * 128], idn_t[:])
                            nc.vector.tensor_copy(hT_t[:, kt, j * 128:(j + 1) * 128], tp[:])
                    qe1_t = qp.tile([128, 4, TPC], qdt, name="qe1_t")
                    for m in range(4):
                        w = qp.tile([128, 8, 128], qdt, name="qw1", tag="qw1", bufs=2)
                        nc.scalar.dma_start(w[:], _col3(w1d, m * 128, (m + 1) * 128))
                        ps = qps.tile([128, TPC], dt.float32, name="ps_q1", tag="ps", bufs=3)
                        for kt in range(8):
                            nc.tensor.matmul(ps[:], w[:, kt, :], hT_t[:, kt, :],
                                             start=(kt == 0), stop=(kt == 7))
                        nc.scalar.activation(qe1_t[:, m, :], ps[:], AF.Relu)
                    qeT_t = qp.tile([128, 8, TPC], qdt, name="qeT_t")
                    for m in range(8):
                        w = qp.tile([128, 4, 128], qdt, name="qw2", tag="qw2", bufs=2)
                        nc.scalar.dma_start(w[:], _col3(w2d, m * 128, (m + 1) * 128))
                        ps = qps.tile([128, TPC], dt.float32, name="ps_q2", tag="ps", bufs=3)
                        for kt in range(4):
                            nc.tensor.matmul(ps[:], w[:, kt, :], qe1_t[:, kt, :],
                                             start=(kt == 0), stop=(kt == 3))
                        nc.vector.tensor_copy(qeT_t[:, m, :], ps[:])
                    for j in range(NQT):
                        ps = qps.tile([128, NC], dt.float32, name="ps_sc", tag="pssc", bufs=2)
                        for kt in range(8):
                            nc.tensor.matmul(ps[:], qeT_t[:, kt, j * 128:(j + 1) * 128],
                                             ceT_u[:, kt, :], start=(kt == 0), stop=(kt == 7))
                        sc = qp.tile([128, NC], dt.float32, name="sc", tag="sc", bufs=2)
                        nc.vector.tensor_copy(sc[:], ps[:])
                        rank = qp.tile([128, NC], dt.float32, name="rank", tag="rank", bufs=2)
                        nc.vector.tensor_copy(rank[:], rkinit_t[:])
                        for d in range(1, NC):
                            ge = qp.tile([128, NC - d], dt.float32, name="ge", tag="ge", bufs=2)
                            nc.vector.tensor_tensor(ge[:], sc[:, :NC - d], sc[:, d:], ALU.is_ge)
                            nc.vector.tensor_add(rank[:, d:], rank[:, d:], ge[:])
                            nc.vector.tensor_sub(rank[:, :NC - d], rank[:, :NC - d], ge[:])
                        m01 = qp.tile([128, NC], dt.float32, name="m01", tag="m01", bufs=2)
                        nc.vector.tensor_scalar(m01[:], rank[:], 7.5, None, ALU.is_le)
                        nc.vector.tensor_scalar(maskb_t[:, j, :], m01[:], 1.0, 1e30,
                                                ALU.subtract, ALU.mult)
                    # mbT[c, j, ii] = maskb[token(j,ii), c] as [1,512] rank-1 rows
                    for j in range(NQT):
                        tpm = qps.tile([16, 128], dt.float32, name="tp_m", tag="tp", bufs=2)
                        nc.tensor.transpose(tpm[:], maskb_t[:, j, :], idn_t[:])
                        nc.vector.tensor_copy(mbT_t[:, j, :], tpm[:])
                    # broadcast mask rows to all partitions: mbb[p, c, i] = maskb[i, c]
                    for c in range(NC):
                        mb1 = qp.tile([1, TPC], dt.float32r, name="mb1", tag="mb1", bufs=2)
                        nc.sync.dma_start(mb1[:],
                                          mbT_t[c:c + 1, :, :].rearrange("c j i -> c (j i)"))
                        psb = qps.tile([128, TPC], dt.float32, name="ps_b", tag="ps", bufs=3)
                        nc.tensor.matmul(psb[:], ones_r[0:1, :], mb1[:], start=True, stop=True)
                        nc.vector.tensor_copy(mbb_t[:, c, :], psb[:])

                # ---- attention: scoresT = K^T-major, mask via rank-1, exp, AV ----
                with tc.tile_pool(name=f"att{li}", bufs=1) as ap, \
                     tc.tile_pool(name=f"attw{li}", bufs=2) as awp, \
                     tc.tile_pool(name=f"attqk{li}", bufs=2, space="PSUM") as aps_qk, \
                     tc.tile_pool(name=f"attrs{li}", bufs=1, space="PSUM") as aps_rs, \
                     tc.tile_pool(name=f"attao{li}", bufs=2, space="PSUM") as aps_ao, \
                     tc.tile_pool(name=f"attrb{li}", bufs=1, space="PSUM") as aps_rb:
                    for hh in range(NH):
                        kT_h = awp.tile([128, S], dt.bfloat16, name="kT_h", tag="kT_h", bufs=2)
                        for r in range(4):
                            nc.scalar.dma_start(
                                kT_h[:, r * TPC:(r + 1) * TPC],
                                k_io[hh][1][r:r + 1, :, :].rearrange("r p t -> (r p) t"))
                        v_h = awp.tile([128, NKT, HD], dt.bfloat16, name="v_h", tag="v_h", bufs=2)
                        for r in range(4):
                            nc.scalar.dma_start(
                                v_h[:, r * 4:(r + 1) * 4, :],
                                v_io[hh // 4][1][r:r + 1, :, :].rearrange(
                                    "r (a p) b -> p (r a) b", p=128)[:, :, (hh % 4) * HD:(hh % 4 + 1) * HD])
                        weT = ap.tile([128, NKT, TPC], dt.bfloat16, name="weT", tag="weT", bufs=2)
                        rs_ps = aps_rs.tile([1, TPC], dt.float32, name="rs_ps", tag="rs")
                        pao = aps_ao.tile([128, TPC], dt.float32, name="pao", tag="ao")
                        for cp in range(NKT // 2):
                            ps_s = aps_qk.tile([128, 2, TPC], dt.float32, name="ps_s", tag="qk")
                            for u in range(2):
                                nc.tensor.matmul(ps_s[:, u, :],
                                                 kT_h[:, (2 * cp + u) * 128:(2 * cp + u + 1) * 128],
                                                 qT_t[:, hh, :], start=True, stop=True)
                            nc.vector.tensor_tensor(ps_s[:], ps_s[:],
                                                    mbb_t[:, 2 * cp:2 * cp + 2, :], ALU.add)
                            nc.scalar.activation(weT[:, 2 * cp:2 * cp + 2, :], ps_s[:],
                                                 AF.Exp, scale=SCALE)
                            for u in range(2):
                                c = 2 * cp + u
                                nc.tensor.matmul(rs_ps[:], ones_b[:, 0:1], weT[:, c, :],
                                                 start=(c == 0), stop=(c == NKT - 1))
                                nc.tensor.matmul(pao[:], v_h[:, c, :], weT[:, c, :],
                                                 start=(c == 0), stop=(c == NKT - 1))
                        rinv_sb = ap.tile([1, TPC], dt.float32, name="rinv", tag="rinv", bufs=2)
                        nc.vector.reciprocal(rinv_sb[:], rs_ps[:])
                        ps_rb = aps_rb.tile([128, TPC], dt.float32, name="ps_rb", tag="rb")
                        nc.tensor.matmul(ps_rb[:], ones_t[0:1, :], rinv_sb[:],
                                         start=True, stop=True)
                        rinvb = ap.tile([128, TPC], dt.float32, name="rinvb", tag="rinvb", bufs=2)
                        nc.vector.tensor_copy(rinvb[:], ps_rb[:])
                        nc.vector.tensor_tensor(aoT_t[:, hh, :], pao[:], rinvb[:], ALU.mult)

                # ---- o-projection direct [tok, feat] + residual add ----
                with tc.tile_pool(name=f"opj{li}", bufs=1) as op, \
                     tc.tile_pool(name=f"opjps{li}", bufs=4, space="PSUM") as ops:
                    wo_sb = op.tile([128, 8, H], dt.float32r, name="wo_sb")
                    nc.sync.dma_start(wo_sb[:], _col3(ow_d[li], 0, H))
                    for j in range(NQT):
                        for mh in range(2):
                            ps_o = ops.tile([128, 512], dt.float32, name="ps_o", tag="ps")
                            for kt in range(8):
                                nc.tensor.matmul(ps_o[:], aoT_t[:, kt, j * 128:(j + 1) * 128],
                                                 wo_sb[:, kt, mh * 512:(mh + 1) * 512],
                                                 start=(kt == 0), stop=(kt == 7))
                            nc.vector.tensor_tensor(h_t[:, j, mh * 512:(mh + 1) * 512],
                                                    h_t[:, j, mh * 512:(mh + 1) * 512],
                                                    ps_o[:], ALU.add)
                qp_cm.__exit__(None, None, None)

                # ---- LN2 + x2T ----
                x2T_t = LP.tile([128, 8, TPC], dt.float32r, name="x2T", tag="xT")
                with tc.tile_pool(name=f"ln2_{li}", bufs=2) as lp2, \
                     tc.tile_pool(name=f"ln2ps{li}", bufs=4, space="PSUM") as lps2:
                    for j in range(NQT):
                        x2 = lp2.tile([128, H], dt.float32, name="x2", tag="x2")
                        _emit_ln(nc, lp2, h_t[:, j, :], x2)
                        for kt in range(8):
                            tp = lps2.tile([128, 128], dt.float32, name="tp_x2", tag="tp")
                            nc.tensor.transpose(tp[:], x2[:, kt * 128:(kt + 1) * 128], idn_t[:])
                            nc.vector.tensor_copy(x2T_t[:, kt, j * 128:(j + 1) * 128], tp[:])

                # ---- FFN: w1 -> gelu -> w2 direct [tok, feat] ----
                with tc.tile_pool(name=f"ffn{li}", bufs=1) as fp, \
                     tc.tile_pool(name=f"ffnw{li}", bufs=2) as fwp, \
                     tc.tile_pool(name=f"ffnps{li}", bufs=3, space="PSUM") as fps, \
                     tc.tile_pool(name=f"ffnps2{li}", bufs=4, space="PSUM") as fps2:
                    gl_sb = fp.tile([128, 32, TPC], dt.float32r, name="gl_sb")
                    for ms in range(32):
                        w1s = fwp.tile([128, 8, 128], dt.float32r, name="w1s", tag="w1s")
                        nc.sync.dma_start(w1s[:], _col3(fw1_d[li], ms * 128, (ms + 1) * 128))
                        psg = fps.tile([128, TPC], dt.float32, name="ps_g", tag="psg")
                        for kt in range(8):
                            nc.tensor.matmul(psg[:], w1s[:, kt, :], x2T_t[:, kt, :],
                                             start=(kt == 0), stop=(kt == 7))
                        nc.scalar.activation(gl_sb[:, ms, :], psg[:], AF.Gelu)
                    for mq in range(4):
                        w2q = fwp.tile([128, 32, 256], dt.float32r, name="w2q", tag="w2q", bufs=2)
                        nc.sync.dma_start(w2q[:], _col3(fw2_d[li], mq * 256, (mq + 1) * 256))
                        for j in range(NQT):
                            ps_f = fps2.tile([128, 256], dt.float32, name="ps_f", tag="psf")
                            for kt in range(32):
                                nc.tensor.matmul(ps_f[:], gl_sb[:, kt, j * 128:(j + 1) * 128],
                                                 w2q[:, kt, :], start=(kt == 0), stop=(kt == 31))
                            nc.vector.tensor_tensor(h_t[:, j, mq * 256:(mq + 1) * 256],
                                                    h_t[:, j, mq * 256:(mq + 1) * 256],
                                                    ps_f[:], ALU.add)

        # ---------------- logits: h all-gather + vocab-sharded matmul ----------------
        with tc.tile_pool(name="lg", bufs=1) as gp, \
             tc.tile_pool(name="lgw", bufs=2) as gwp, \
             tc.tile_pool(name="lgps", bufs=2, space="PSUM") as gps, \
             tc.tile_pool(name="lgps2", bufs=6, space="PSUM") as gps2:
            hTf = gp.tile([128, 8, TPC], dt.float32r, name="hTf")
            for kt in range(8):
                for j in range(NQT):
                    tp = gps.tile([128, 128], dt.float32, name="tp_hf", tag="tp")
                    nc.tensor.transpose(tp[:], h_t[:, j, kt * 128:(kt + 1) * 128], idn_t[:])
                    nc.vector.tensor_copy(hTf[:, kt, j * 128:(j + 1) * 128], tp[:])
            nc.sync.dma_start(hag_in[:], hTf[:].rearrange("p a b -> p (a b)"))
            nc.gpsimd.collective_compute("AllGather", ALU.bypass, replica_groups=GROUPS,
                                         ins=[hag_in[:].opt()], outs=[hag_out[:].opt()])
            hT_full = gp.tile([128, 8, S], dt.float32r, name="hT_full")
            for r in range(4):
                nc.scalar.dma_start(
                    hT_full[:, :, r * TPC:(r + 1) * TPC],
                    hag_out[r:r + 1, :, :].rearrange("r p (a b) -> (r p) a b", a=8))
            ntiles = [(n * 512, 512) for n in range(VS // 512)]
            if VS % 512:
                ntiles.append((VS - VS % 512, VS % 512))
            for (noff, nsz) in ntiles:
                wf = gwp.tile([128, 8, 512], dt.float32r, name="ow_f", tag="ow", bufs=2)
                nc.sync.dma_start(wf[:, :, :nsz],
                                  outw_d.rearrange("(kt kp) n -> kp kt n", kp=128)[:, :, noff:noff + nsz])
                for tt in range(S // 128):
                    ps = gps2.tile([128, 512], dt.float32, name="ps_lg", tag="ps")
                    for kt in range(8):
                        nc.tensor.matmul(ps[:, :nsz], hT_full[:, kt, tt * 128:(tt + 1) * 128],
                                         wf[:, kt, :nsz], start=(kt == 0), stop=(kt == 7))
                    ot = gp.tile([128, 512], dt.float32, name="ot", tag="ot", bufs=6)
                    nc.vector.tensor_copy(ot[:, :nsz], ps[:, :nsz])
                    nc.scalar.dma_start(logits_d[tt * 128:(tt + 1) * 128, noff:noff + nsz],
                                        ot[:, :nsz])

    nc.compile()
    return nc


def _prep_inputs(inputs):
    f32 = lambda x: np.ascontiguousarray(np.asarray(x, dtype=np.float32))
    ids = np.asarray(inputs["input_ids"]).astype(np.int32)
    common = {
        "tok_emb": f32(inputs["tok_emb"]), "in_w": f32(inputs["in_w"]),
        "qe_w1": f32(inputs["qe_w1"]), "qe_w2": f32(inputs["qe_w2"]),
        "qe_w1r": f32(inputs["qe_w1"]), "qe_w2r": f32(inputs["qe_w2"]),
        "ch_w1": f32(inputs["ch_w1"]), "ch_w2": f32(inputs["ch_w2"]),
        "idn": np.eye(128, dtype=np.float32),
        "ones": np.ones((128, 128), dtype=np.float32),
        "c_mean": np.full((128, 1), 1.0 / CS, dtype=np.float32),
        "rank_init": np.ascontiguousarray(
            np.broadcast_to(NC - 1 - np.arange(NC, dtype=np.float32), (128, NC))),
    }
    for i in range(L):
        for nm in ["q_w", "k_w", "v_w", "o_w", "f_w1", "f_w2"]:
            common[f"l{i}_{nm}"] = f32(np.asarray(inputs[nm])[i])
    pos = f32(inputs["pos_emb"])
    outw = f32(inputs["out_w"])
    in_maps = []
    for c in range(8):
        b, q = c // 4, c % 4
        off = q * TPC
        m = dict(common)
        m["ids_col"] = np.ascontiguousarray(ids[b, off:off + TPC].reshape(NQT, 128).T)
        m["pos"] = np.ascontiguousarray(pos[off:off + TPC])
        m["out_w_sl"] = np.ascontiguousarray(outw[:, q * VS:(q + 1) * VS])
        in_maps.append(m)
    return in_maps


def kernel(**inputs) -> np.ndarray:
    # biases / LN affine params are zero / one for this model; the kernel
    # implements that fast path (verified here).
    for k in ["in_b", "ch_b1", "ch_b2", "qe_b1", "qe_b2", "q_b", "k_b", "v_b",
              "o_b", "f_b1", "f_b2", "ln1_b", "ln2_b", "out_b"]:
        assert not np.any(np.asarray(inputs[k])), f"nonzero bias {k} unsupported"
    for k in ["ln1_g", "ln2_g"]:
        assert np.all(np.asarray(inputs[k]) == 1.0), f"non-unit {k} unsupported"

    if "nc" not in _CACHE:
        _CACHE["nc"] = _build()
    nc = _CACHE["nc"]
    in_maps = _prep_inputs(inputs)
    res = run_bass_kernel_spmd(nc, in_maps, list(range(8)))
    out = np.empty((B, S, V), dtype=np.float32)
    for c in range(8):
        b, q = c // 4, c % 4
        out[b, :, q * VS:(q + 1) * VS] = res.results[c]["logits"]
    return out
